# revision 47
# baseline (speedup 1.0000x reference)
"""Grouped-Query Attention block (RMSNorm + RoPE + causal GQA + o_proj) on 8 trn2 NeuronCores.

Sharding: data-parallel over batch (2) x tensor-parallel over kv-head groups (4).
Core c = b*4 + g handles batch b, kv heads {2g, 2g+1}, q heads {8g..8g+7}.
Each core computes a partial o_proj output (T, D) over its 768 head-dims;
host sums the 4 group partials per batch.

v3 structure:
  * bf16 inputs/weights/activations; accumulation + norm/softmax stats fp32.
  * All matmuls 512-wide moving dim: PE sequencer dispatch (~170ns/matmul)
    stays under engine execution (~213ns) -> engine-bound, not dispatch-bound.
  * DMAs batched (x: 4 single-DMA token-quarters, o_proj weights: 1 DMA,
    output: 1 DMA per 128-token row) and spread across the SP and Pool
    dispatch queues so no sequencer saturates.
  * x + all projection weights SBUF-resident; first two x quarters land on
    different queues in parallel -> compute starts ~5us in.
  * RMSNorm / softmax broadcast matmuls issue-deferred behind the next unit's
    accumulation matmuls so PE never stalls on the Act/DVE scalar chains.
  * o_proj contraction packed: 8 heads x 96 rows repacked (SBUF DMA) into
    6 full 128-row groups -> 6 instead of 8 matmuls per output tile; o_proj
    row-tiles for tokens 0:512 interleave into the attention stream as PE
    gap fillers while Act churns exps.
"""

import os
import sys

import numpy as np

sys.path.insert(0, "/opt/trn_rl_repo")

B, T, D = 2, 1024, 3072
NH, NKV, HD = 32, 8, 96
G = 4                 # tensor-parallel groups
QH = NH // G          # q heads per core (8)
KVH = NKV // G        # kv heads per core (2)
NCORES = 8
EPS = 1e-6
SCALE = 1.0 / float(np.sqrt(HD))
KT = D // 128         # 24 contraction tiles over d_model
KTOK = T // 128       # 8 token tiles
QCS = 256             # q chunk in phase 2
NQC = T // QCS        # 4
NKP = QCS // 128      # k tiles per q chunk
D_PIPE2 = 3           # phase-2 score matmuls issued ahead of P@V
NJ = D // 512         # 6 output column chunks
NG3 = QH * HD // 128  # 6 packed o_proj contraction groups
D_PIPE = 2            # phase-2 score matmuls issued ahead of P@V

_BUILD_CACHE = {}


def _build_nc():
    from contextlib import ExitStack
    from concourse import bacc, tile, mybir
    from concourse.masks import make_identity

    f32 = mybir.dt.float32
    f32r = mybir.dt.float32r
    bf16 = mybir.dt.bfloat16
    AF = mybir.ActivationFunctionType

    nc = bacc.Bacc("TRN2", target_bir_lowering=False, debug=False,
                   num_devices=NCORES)

    xt_d = nc.dram_tensor("xt", (128, KT, T), bf16, kind="ExternalInput").ap()
    wqt_d = nc.dram_tensor("wqt", (6, 128, KT, 128), bf16, kind="ExternalInput").ap()
    wkt_d = nc.dram_tensor("wkt", (KVH, 128, KT, 128), bf16, kind="ExternalInput").ap()
    wvt_d = nc.dram_tensor("wvt", (KVH, 128, KT, HD), bf16, kind="ExternalInput").ap()
    wot_d = nc.dram_tensor("wot", (128, NJ, NG3, 512), bf16, kind="ExternalInput").ap()
    tabaq_d = nc.dram_tensor("tabaq", (3, 128, T), bf16, kind="ExternalInput").ap()
    tabbq_d = nc.dram_tensor("tabbq", (3, 128, T), bf16, kind="ExternalInput").ap()
    sels_d = nc.dram_tensor("sels", (3, 128, 4), f32r, kind="ExternalInput").ap()
    selst_d = nc.dram_tensor("selst", (3, 4, 128), f32r, kind="ExternalInput").ap()
    tabak_d = nc.dram_tensor("tabak", (128, T), bf16, kind="ExternalInput").ap()
    tabbk_d = nc.dram_tensor("tabbk", (128, T), bf16, kind="ExternalInput").ap()
    o128_d = nc.dram_tensor("o128", (128, 1), f32r, kind="ExternalInput").ap()
    o96_d = nc.dram_tensor("o96", (1, HD), f32r, kind="ExternalInput").ap()
    o128T_d = nc.dram_tensor("o128T", (1, 128), f32r, kind="ExternalInput").ap()
    ocol_d = nc.dram_tensor("ocol", (128, KTOK), bf16, kind="ExternalInput").ap()
    out_d = nc.dram_tensor("out", (T, D), f32, kind="ExternalOutput").ap()

    with tile.TileContext(nc) as tc:
        with nc.allow_low_precision(reason="bf16 compute, fp32 accumulate"), \
             ExitStack() as ctx:
            const = ctx.enter_context(tc.tile_pool(name="const", bufs=1))
            p_qkv = ctx.enter_context(tc.tile_pool(name="p_qkv", bufs=1))

            eps_t = const.tile([1, 1], f32, tag="eps")
            nc.vector.memset(eps_t[:], EPS)
            eps4_t = const.tile([4, 1], f32, tag="eps4")
            nc.vector.memset(eps4_t[:], EPS)
            ones128 = const.tile([128, 1], f32r, tag="ones128")
            ones96 = const.tile([1, HD], f32r, tag="ones96")
            ones128T = const.tile([1, 128], f32r, tag="ones128T")
            ident = const.tile([128, 128], f32, tag="ident")

            qt = [p_qkv.tile([128, T], bf16, tag=f"qt{h}", name=f"qt{h}")
                  for h in range(QH)]
            for h in range(QH):
                nc.vector.memset(qt[h][32:64, :], 0.0)
                nc.vector.memset(qt[h][96:128, :], 0.0)
            ktl = [p_qkv.tile([128, T], bf16, tag=f"kt{g2}", name=f"kt{g2}")
                   for g2 in range(KVH)]
            vext = [p_qkv.tile([128, KTOK, HD + 1], bf16, tag=f"vx{g2}",
                               name=f"vx{g2}") for g2 in range(KVH)]
            atp = [p_qkv.tile([128, T], bf16, tag=f"atp{t}", name=f"atp{t}")
                   for t in range(NG3)]

            # ---------------- Phase 1: projections + RMSNorm + RoPE ---------
            with ExitStack() as s1:
                xt_pool = s1.enter_context(tc.tile_pool(name="xt", bufs=1))
                wkv_pool = s1.enter_context(tc.tile_pool(name="wkv", bufs=1))
                wq_pool = s1.enter_context(tc.tile_pool(name="wq", bufs=3))
                tab_pool = s1.enter_context(tc.tile_pool(name="tabs", bufs=1))
                tmp_pool = s1.enter_context(tc.tile_pool(name="tmp1", bufs=3))
                tmp4_pool = s1.enter_context(tc.tile_pool(name="tmp4", bufs=4))
                tmpb_pool = s1.enter_context(tc.tile_pool(name="tmpb", bufs=2))
                psk_pool = s1.enter_context(
                    tc.tile_pool(name="psk", bufs=1, space="PSUM"))
                psq_pool = s1.enter_context(
                    tc.tile_pool(name="psq", bufs=3, space="PSUM"))
                vtr_pool = s1.enter_context(
                    tc.tile_pool(name="vtr", bufs=1, space="PSUM"))
                ssq_pool = s1.enter_context(
                    tc.tile_pool(name="ssq", bufs=1, space="PSUM"))
                bc_pool = s1.enter_context(
                    tc.tile_pool(name="bc1", bufs=1, space="PSUM"))

                xt_t = [xt_pool.tile([128, KT, 512], bf16, tag=f"xh{hf}",
                                     name=f"xh{hf}") for hf in range(2)]
                wk_t = [wkv_pool.tile([128, KT, 128], bf16, tag=f"wk{i}",
                                      name=f"wk{i}") for i in range(KVH)]


                tabaq_t = [tab_pool.tile([128, T], bf16, tag=f"tabaq{p}",
                                          name=f"tabaq{p}") for p in range(3)]
                tabbq_t = [tab_pool.tile([128, T], bf16, tag=f"tabbq{p}",
                                          name=f"tabbq{p}") for p in range(3)]
                sels_t = [tab_pool.tile([128, 4], f32r, tag=f"sels{p}",
                                        name=f"sels{p}") for p in range(3)]
                selst_t = [tab_pool.tile([4, 128], f32r, tag=f"selst{p}",
                                         name=f"selst{p}") for p in range(3)]
                tabak_t = tab_pool.tile([128, T], bf16, tag="tabak")
                tabbk_t = tab_pool.tile([128, T], bf16, tag="tabbk")
                # static load order; wk races xh0 on the two HWDGE queues
                nc.scalar.dma_start(wk_t[0][:], wkt_d[0])
                nc.sync.dma_start(xt_t[0][:], xt_d[:, :, 0:512])
                nc.scalar.dma_start(wk_t[1][:], wkt_d[1])
                nc.sync.dma_start(xt_t[1][:], xt_d[:, :, 512:1024])
                nc.sync.dma_start(ones128[:], o128_d[:])
                nc.sync.dma_start(ones96[:], o96_d[:])
                nc.sync.dma_start(ones128T[:], o128T_d[:])
                make_identity(nc, ident[:])
                nc.scalar.dma_start(tabak_t[:], tabak_d[:])
                nc.scalar.dma_start(tabbk_t[:], tabbk_d[:])
                for p in range(3):
                    nc.scalar.dma_start(tabaq_t[p][:], tabaq_d[p])
                    nc.scalar.dma_start(tabbq_t[p][:], tabbq_d[p])
                    nc.sync.dma_start(sels_t[p][:], sels_d[p])
                    nc.sync.dma_start(selst_t[p][:], selst_d[p])
                wq_tiles = {}

                def load_wq(j):
                    if j >= 6:
                        return
                    t = wq_pool.tile([128, KT, 128], bf16, tag="wq",
                                     name="wq")
                    nc.scalar.dma_start(t[:], wqt_d[j])
                    wq_tiles[j] = t

                for j in range(3):
                    load_wq(j)
                for g2 in range(KVH):
                    nc.sync.dma_start(vext[g2][:, :, HD:HD + 1], ocol_d[:])

                def load_rest():
                    pass

                # --- norm + RoPE chain, split into two issue stages ---------
                def stage_a(ps, sq, rinv):
                    """ssq matmul (PE) + sqrt (Act) + recip (DVE)."""
                    ssq = ssq_pool.tile([4, 512], f32, tag="ssq")
                    nc.tensor.matmul(ssq[0:1, :], ones128[:], sq[:],
                                     start=True, stop=True)
                    rms = tmpb_pool.tile([1, 512], f32, tag="rms")
                    nc.scalar.activation(rms[:], ssq[0:1, :], AF.Sqrt,
                                         bias=eps_t[:], scale=1.0 / HD)
                    nc.vector.reciprocal(rinv[:], rms[:])

                def stage_b(ps, qsh, rinv, dst, hsl, ta, tb, ve):
                    """rinv broadcast (PE) + normalize + RoPE (Act/DVE).
                    ps/qsh are SBUF copies of the pre-norm projection and its
                    RoPE partner-row swap (DMA'd off the critical DVE chain).
                    The RMSNorm weights are folded into the RoPE tables
                    host-side, so one plain 1/rms broadcast serves both.
                    """
                    bc = bc_pool.tile([128, 512], f32, tag="bc")
                    nc.tensor.matmul(bc[:], ones128T[:], rinv[:],
                                     start=True, stop=True)
                    bcs = tmpb_pool.tile([128, 512], f32, tag="bcs")
                    nc.scalar.copy(bcs[:], bc[:])
                    sl = tmpb_pool.tile([128, 512], bf16, tag="slt")
                    ve.tensor_mul(sl[:], ps[:], bcs[:])
                    qsn = tmpb_pool.tile([128, 512], bf16, tag="qsn")
                    ve.tensor_mul(qsn[:], qsh[:], bcs[:])
                    # dst = sl*P + qsn*Q, P/Q full-height tables with the
                    # norm weights and the rotation signs folded in host-side
                    tm1 = tmpb_pool.tile([128, 512], bf16, tag="tm1")
                    ve.tensor_mul(tm1[:], sl[:], ta[:, hsl])
                    tm2 = tmpb_pool.tile([128, 512], bf16, tag="tm2")
                    ve.tensor_mul(tm2[:], qsn[:], tb[:, hsl])
                    ve.tensor_add(dst[:, hsl], tm1[:], tm2[:])

                # Deferred two-stage pipeline over accumulation units: the
                # PE ops of stage A/B for unit u are emitted after the accum
                # matmuls of units u+1 / u+2, so PE never waits on Act/DVE.
                chainq = []   # entries: [stage, a_thunk, b_thunk]

                def step_chain_b():
                    if chainq and chainq[0][0] == 1:
                        e = chainq.pop(0)
                        e[2]()

                def step_chain():
                    for e in chainq:
                        if e[0] == 0:
                            e[0] = 1
                            e[1]()
                            return

                def flush_chains():
                    while chainq:
                        step_chain_b()
                        step_chain()
                        if chainq and chainq[0][0] == 1:
                            e = chainq.pop(0)
                            e[2]()

                chain_no = [0]

                def make_chain(ps, dst, hsl, ta, tb):
                    # alternate DVE/Pool for RoPE; last chains stay on DVE so
                    # the Pool queue is clear for phase-2 affine_selects
                    ve = (nc.gpsimd if (chain_no[0] % 2 == 1
                                        and chain_no[0] < 14) else nc.vector)
                    chain_no[0] += 1
                    sq = tmp_pool.tile([128, 512], f32r, tag="sq")
                    nc.scalar.square(sq[:], ps[:])
                    # SBUF copy frees the PSUM tile and lets the RoPE partner
                    # swap run as a DMA concurrent with the norm chain
                    psb = tmp_pool.tile([128, 512], f32, tag="psb")
                    nc.scalar.copy(psb[:], ps[:])
                    qsh = tmp_pool.tile([128, 512], f32, tag="qsh")
                    nc.sync.dma_start(qsh[0:64, :], psb[64:128, :])
                    nc.sync.dma_start(qsh[64:128, :], psb[0:64, :])
                    rinv = tmp_pool.tile([1, 512], f32r, tag="rinv")
                    push = [0, lambda: stage_a(psb, sq, rinv),
                            lambda: stage_b(psb, qsh, rinv, dst, hsl,
                                            ta, tb, ve)]
                    chainq.append(push)

                # ---- k phase (first: x-paced; half 0 in token-quarters) ----
                psk = [psk_pool.tile([128, 512], f32, tag=f"psk{i}",
                                     name=f"psk{i}") for i in range(KVH)]
                for hf in range(2):
                    hsl = slice(hf * 512, (hf + 1) * 512)
                    for i in range(KVH):
                        for kt in range(KT):
                            nc.tensor.matmul(
                                psk[i][:], wk_t[i][:, kt, :],
                                xt_t[hf][:, kt, :],
                                start=(kt == 0), stop=(kt == KT - 1))
                        if hf == 0 and i == 0:
                            load_rest()
                        step_chain_b()
                        step_chain()
                        make_chain(psk[i], ktl[i], hsl, tabak_t, tabbk_t)

                # ---- q phase: heads packed 8x96 -> 6x128 ------------------
                # Two sets of 4 heads, 3 full 128-row groups each: 288 accum
                # matmuls instead of 384.  Per-head RMSNorm sums via selector
                # matmuls; RoPE in packed layout with per-group-pattern P/Q
                # tables; results DMA-repacked into the per-head padded qt
                # tiles so phase 2 is unchanged (zero k-pads keep the unset
                # qt pad rows harmless in the QK contraction).
                thunkq = []   # (append_step, fn) run one per step, lag >= 1
                stepc = [0]

                def step_thunk():
                    stepc[0] += 1
                    if thunkq and thunkq[0][0] < stepc[0]:
                        thunkq.pop(0)[1]()

                RUNS48 = []   # (start, partner_start, head4, is_odd)
                for h4 in range(4):
                    RUNS48.append((96 * h4, 96 * h4 + 48, h4, 0))
                    RUNS48.append((96 * h4 + 48, 96 * h4, h4, 1))

                for s in range(2):
                    for hf in range(2):
                        hsl = slice(hf * 512, (hf + 1) * 512)
                        sqs, psbs, qshs = [], [], []
                        for g in range(3):
                            ps = psq_pool.tile([128, 512], f32, tag="psq",
                                               name="ps")
                            wqg = wq_tiles[3 * s + g] if hf == 0 else \
                                wq_tiles[3 * s + g]
                            for kt in range(KT):
                                nc.tensor.matmul(
                                    ps[:], wqg[:, kt, :],
                                    xt_t[hf][:, kt, :],
                                    start=(kt == 0), stop=(kt == KT - 1))
                            if hf == 1:
                                load_wq(3 * s + g + 3)
                            step_chain_b()
                            step_chain()
                            step_thunk()
                            sq = tmp_pool.tile([128, 512], f32r, tag="sq",
                                               name="sq")
                            nc.scalar.square(sq[:], ps[:])
                            psb = tmp4_pool.tile([128, 512], f32, tag="psb",
                                                 name="psb")
                            nc.scalar.copy(psb[:], ps[:])
                            sqs.append(sq)
                            psbs.append(psb)
                        for g in range(3):
                            qshs.append(tmp4_pool.tile(
                                [128, 512], f32, tag="qsh", name="qsh"))
                        # partner-row swap, cross-group pieces
                        for (a, ap, h4, odd) in RUNS48:
                            pos = 0
                            while pos < 48:
                                dg, dp = divmod(a + pos, 128)
                                sg, sp = divmod(ap + pos, 128)
                                ln = min(48 - pos, 128 - dp, 128 - sp)
                                nc.sync.dma_start(
                                    qshs[dg][dp:dp + ln, :],
                                    psbs[sg][sp:sp + ln, :])
                                pos += ln
                        rinv4 = tmp_pool.tile([4, 512], f32r, tag="rinv4",
                                              name="rinv4")

                        def a_set(sqs=sqs, rinv4=rinv4):
                            ssq = ssq_pool.tile([4, 512], f32, tag="ssq",
                                                name="ssq")
                            for g in range(3):
                                nc.tensor.matmul(
                                    ssq[:], sels_t[g][:], sqs[g][:],
                                    start=(g == 0), stop=(g == 2))
                            rms4 = tmpb_pool.tile([4, 512], f32, tag="rms4",
                                                  name="rms4")
                            nc.scalar.activation(rms4[:], ssq[:], AF.Sqrt,
                                                 bias=eps4_t[:],
                                                 scale=1.0 / HD)
                            nc.vector.reciprocal(rinv4[:], rms4[:])
                        thunkq.append((stepc[0], a_set))

                        for g in range(3):
                            def b_g(g=g, s=s, hsl=hsl, psb=psbs[g],
                                    qsh=qshs[g], rinv4=rinv4):
                                bc = bc_pool.tile([128, 512], f32, tag="bc",
                                                  name="bc")
                                nc.tensor.matmul(bc[:], selst_t[g][:],
                                                 rinv4[:],
                                                 start=True, stop=True)
                                bcs = tmpb_pool.tile([128, 512], f32,
                                                     tag="bcs", name="bcs")
                                nc.scalar.copy(bcs[:], bc[:])
                                sl = tmpb_pool.tile([128, 512], bf16,
                                                    tag="slt", name="sl")
                                nc.vector.tensor_mul(sl[:], psb[:], bcs[:])
                                qsn = tmpb_pool.tile([128, 512], bf16,
                                                     tag="qsn", name="qsn")
                                nc.vector.tensor_mul(qsn[:], qsh[:], bcs[:])
                                tm1 = tmpb_pool.tile([128, 512], bf16,
                                                     tag="tm1", name="tm1")
                                nc.vector.tensor_mul(tm1[:], sl[:],
                                                     tabaq_t[g][:, hsl])
                                tm2 = tmpb_pool.tile([128, 512], bf16,
                                                     tag="tm2", name="tm2")
                                nc.vector.tensor_mul(tm2[:], qsn[:],
                                                     tabbq_t[g][:, hsl])
                                qp = tmpb_pool.tile([128, 512], bf16,
                                                    tag="slt", name="qp")
                                nc.vector.tensor_add(qp[:], tm1[:], tm2[:])
                                # repack into per-head padded qt layout
                                base = 128 * g
                                for (a, ap, h4, odd) in RUNS48:
                                    lo = max(a, base)
                                    hi = min(a + 48, base + 128)
                                    if lo >= hi:
                                        continue
                                    o0 = lo - a
                                    d0 = (64 + o0) if odd else o0
                                    nc.sync.dma_start(
                                        qt[4 * s + h4][d0:d0 + hi - lo, hsl],
                                        qp[lo - base:hi - base, :])
                            thunkq.append((stepc[0], b_g))

                # ---- v phase last: no norm chains -- the trailing q chains
                # drain on Act/DVE behind these accums, so the phase-2 scope
                # switch sees no backlog
                wv_t = []
                for i in range(KVH):
                    t = wq_pool.tile([128, KT, 128], bf16, tag="wq",
                                     name="wv")
                    nc.scalar.dma_start(t[:, :, 0:HD], wvt_d[i])
                    wv_t.append(t)
                for hf in range(2):
                    hsl = slice(hf * 512, (hf + 1) * 512)
                    for i in range(KVH):
                        vp = psq_pool.tile([128, 512], f32, tag="psq")
                        vps = vp[0:HD, :]
                        for kt in range(KT):
                            nc.tensor.matmul(
                                vps, wv_t[i][:, kt, 0:HD],
                                xt_t[hf][:, kt, :],
                                start=(kt == 0), stop=(kt == KT - 1))
                        vt = tab_pool.tile([HD, 512], f32, tag="vt",
                                           name="vt")
                        nc.scalar.copy(vt[:], vp[0:HD, :])
                        step_chain_b()
                        step_chain()
                        step_thunk()
                        step_thunk()
                        for c in range(4):
                            tp = vtr_pool.tile([128, HD], f32, tag="tp")
                            nc.tensor.transpose(
                                tp[:], vt[:, c * 128:(c + 1) * 128],
                                ident[0:HD, 0:HD])
                            itok = hf * 4 + c
                            nc.scalar.copy(vext[i][:, itok, 0:HD], tp[:])
                        step_thunk()
                flush_chains()
                while thunkq:
                    thunkq.pop(0)[1]()

            # -------- Phase 2+3: attention with o_proj interleaved ----------
            # q-chunks outer, heads inner: once all 8 heads finish chunk qc,
            # tokens [512qc, 512qc+512) are fully attended and their o_proj
            # row-tiles are emitted as PE gap-fillers while the Act engine
            # works through the next chunk group's exps.
            with ExitStack() as s2:
                pt_pool = s2.enter_context(tc.tile_pool(name="pt", bufs=5))
                tmp2_pool = s2.enter_context(tc.tile_pool(name="tmp2", bufs=3))
                wo_pool = s2.enter_context(tc.tile_pool(name="wo3", bufs=1))
                ob_pool = s2.enter_context(tc.tile_pool(name="ob", bufs=4))
                sc_pool = s2.enter_context(
                    tc.tile_pool(name="sc", bufs=3, space="PSUM"))
                po_pool = s2.enter_context(
                    tc.tile_pool(name="po", bufs=2, space="PSUM"))
                bc2_pool = s2.enter_context(
                    tc.tile_pool(name="bc2", bufs=1, space="PSUM"))
                ps3_pool = s2.enter_context(
                    tc.tile_pool(name="ps3", bufs=2, space="PSUM"))

                wo_t = [wo_pool.tile([128, NG3, 512], bf16, tag=f"wo3{j}",
                                     name=f"wo3{j}") for j in range(NJ)]

                # flattened (qc, h, kt2) item list; sc/exp/select emitted
                # D_PIPE items ahead of the corresponding P@V matmul.
                items = []
                for qc in range(NQC):
                    for h in range(QH):
                        for kt2 in range(NKP * qc + NKP):
                            items.append((h, qc, kt2))

                po_t = {}
                pts = {}
                norm_q = []
                ph3_q = []
                ob_t = {}

                def emit_sc(idx):
                    h, qc, kt2 = items[idx]
                    g2 = h // (QH // KVH)
                    jsl = slice(qc * QCS, (qc + 1) * QCS)
                    sc = sc_pool.tile([128, QCS], f32, tag="sc")
                    nc.tensor.matmul(
                        sc[:], ktl[g2][:, kt2 * 128:(kt2 + 1) * 128],
                        qt[h][:, jsl], start=True, stop=True)
                    pt = pt_pool.tile([128, QCS], bf16, tag="pt")
                    nc.scalar.activation(pt[:], sc[:], AF.Exp, scale=SCALE)
                    if kt2 >= NKP * qc:
                        nc.gpsimd.affine_select(
                            pt[:], pt[:], pattern=[[1, QCS]],
                            compare_op=mybir.AluOpType.is_ge,
                            fill=0.0,
                            base=qc * QCS - kt2 * 128,
                            channel_multiplier=-1)
                    pts[idx] = pt

                def emit_pv(idx):
                    h, qc, kt2 = items[idx]
                    g2 = h // (QH // KVH)
                    if kt2 == 0:
                        po_t[(h, qc)] = po_pool.tile([HD + 1, QCS], f32,
                                                     tag="po", name="po")
                    po = po_t[(h, qc)]
                    nc.tensor.matmul(
                        po[:], vext[g2][:, kt2, :], pts.pop(idx)[:],
                        start=(kt2 == 0),
                        stop=(kt2 == NKP * qc + NKP - 1))
                    if kt2 == NKP * qc + NKP - 1:
                        rinv2 = tmp2_pool.tile([1, QCS], f32r, tag="rinv2")
                        nc.vector.reciprocal(rinv2[:], po[HD:HD + 1, :])
                        norm_q.append((h, qc, rinv2))

                def emit_norm():
                    if not norm_q:
                        return
                    h, qc, rinv2 = norm_q.pop(0)
                    jsl = slice(qc * QCS, (qc + 1) * QCS)
                    po = po_t.pop((h, qc))
                    bc2 = bc2_pool.tile([HD, QCS], f32, tag="bc2")
                    nc.tensor.matmul(bc2[:], ones96[:], rinv2[:],
                                     start=True, stop=True)
                    bc2s = tmp2_pool.tile([HD, QCS], f32, tag="bc2s")
                    nc.scalar.copy(bc2s[:], bc2[:])
                    at_tmp = tmp2_pool.tile([HD, QCS], bf16, tag="at_tmp")
                    nc.vector.tensor_mul(at_tmp[:], po[0:HD, :], bc2s[:])
                    # repack rows 96h..96h+96 into the 6x128 contraction tiles
                    r0 = h * HD
                    while r0 < (h + 1) * HD:
                        t, p = divmod(r0, 128)
                        rows = min(128 - p, (h + 1) * HD - r0)
                        nc.sync.dma_start(
                            atp[t][p:p + rows, jsl],
                            at_tmp[r0 - h * HD:r0 - h * HD + rows, :])
                        r0 += rows
                    if h == QH - 1:
                        for i in range(NKP * qc, NKP * qc + NKP):
                            for j3 in range(NJ):
                                ph3_q.append((i, j3))

                def emit_ph3():
                    if not ph3_q:
                        return
                    i, j3 = ph3_q.pop(0)
                    isl = slice(i * 128, (i + 1) * 128)
                    ps3 = ps3_pool.tile([128, 512], f32, tag="ps3")
                    for t in range(NG3):
                        nc.tensor.matmul(
                            ps3[:], atp[t][:, isl], wo_t[j3][:, t, :],
                            start=(t == 0), stop=(t == NG3 - 1))
                    jsl = slice(j3 * 512, (j3 + 1) * 512)
                    ob = ob_pool.tile([128, 512], f32, tag="ob", name="ob")
                    nc.vector.tensor_copy(ob[:], ps3[:])
                    nc.sync.dma_start(out_d[isl, jsl], ob[:])

                for idx in range(len(items)):
                    if idx % 4 == 0 and idx // 4 < NJ:
                        j = idx // 4
                        nc.scalar.dma_start(wo_t[j][:], wot_d[:, j])
                    emit_sc(idx)
                    if idx >= D_PIPE2:
                        emit_pv(idx - D_PIPE2)
                        emit_norm()
                    if idx % 3 == 2:
                        emit_ph3()
                for idx in range(len(items) - D_PIPE2, len(items)):
                    emit_pv(idx)
                    emit_norm()
                while norm_q:
                    emit_norm()
                while ph3_q:
                    emit_ph3()

    nc.compile()
    return nc


def get_nc():
    if "nc" not in _BUILD_CACHE:
        _BUILD_CACHE["nc"] = _build_nc()
    return _BUILD_CACHE["nc"]


def _permpad_rows(w96):
    """(96, N) head rows -> (128, N): evens at 0:48, odds at 64:112, pad 0."""
    out = np.zeros((128, w96.shape[1]), np.float32)
    out[0:48] = w96[0::2]
    out[64:112] = w96[1::2]
    return out


def _lhsT_tiles(wT, m):
    """(D, m) -> (128, KT, m) lhsT tile layout (contraction on partitions)."""
    return np.ascontiguousarray(
        wT.reshape(KT, 128, m).transpose(1, 0, 2)).astype(np.float32)


def prepare_in_maps(x, wq, wk, wv, wo, q_norm_w, k_norm_w, cos, sin):
    import ml_dtypes
    bf16 = ml_dtypes.bfloat16

    x = np.asarray(x, np.float32)
    wq = np.asarray(wq, np.float32)
    wk = np.asarray(wk, np.float32)
    wv = np.asarray(wv, np.float32)
    wo = np.asarray(wo, np.float32)
    cos = np.asarray(cos, np.float32)
    sin = np.asarray(sin, np.float32)
    q_norm_w = np.asarray(q_norm_w, np.float32)
    k_norm_w = np.asarray(k_norm_w, np.float32)

    def _fold_tabs(nw):
        # P multiplies the in-place operand sl, Q the partner-swapped qsn:
        #   evens rows: out = a*we*ce - b*wo*se -> P=we*ce, Q=-wo*se
        #   odds rows:  out = b*wo*co + a*we*so -> P=wo*co, Q=+we*so
        nwe = nw[0::2][:, None]
        nwo = nw[1::2][:, None]
        ta = np.zeros((128, T), np.float32)
        tb = np.zeros((128, T), np.float32)
        ta[0:48] = nwe * cos[:, 0::2].T
        ta[64:112] = nwo * cos[:, 1::2].T
        tb[0:48] = -nwo * sin[:, 0::2].T
        tb[64:112] = nwe * sin[:, 1::2].T
        return ta, tb

    tabak, tabbk = _fold_tabs(k_norm_w)

    # packed-layout q tables/selectors: set-local row R (of 384) -> head
    # h4 = R//96, local l = R%96; l<48 = even component e=l, else odd o=l-48
    tabaq = np.zeros((3, 128, T), np.float32)
    tabbq = np.zeros((3, 128, T), np.float32)
    sels = np.zeros((3, 128, 4), np.float32)
    for p in range(3):
        for r in range(128):
            R = 128 * p + r
            h4, l = divmod(R, 96)
            sels[p, r, h4] = 1.0
            if l < 48:
                e = l
                tabaq[p, r] = q_norm_w[2 * e] * cos[:, 2 * e]
                tabbq[p, r] = -q_norm_w[2 * e + 1] * sin[:, 2 * e]
            else:
                o = l - 48
                tabaq[p, r] = q_norm_w[2 * o + 1] * cos[:, 2 * o + 1]
                tabbq[p, r] = q_norm_w[2 * o] * sin[:, 2 * o + 1]
    selst = np.ascontiguousarray(sels.transpose(0, 2, 1))

    xts = []
    for b in range(B):
        xT = np.ascontiguousarray(x[b].T)  # (D, T)
        xts.append(np.ascontiguousarray(
            xT.reshape(KT, 128, T).transpose(1, 0, 2)).astype(bf16))

    in_maps = []
    for c in range(NCORES):
        b, g = divmod(c, G)
        # packed q weights: 8 heads x 96 rows (evens then odds per head)
        # -> 6 full 128-row groups
        wqp = np.zeros((QH * HD, D), np.float32)
        for i in range(QH):
            hw_ = wq[(g * QH + i) * HD:(g * QH + i + 1) * HD]
            wqp[96 * i:96 * i + 48] = hw_[0::2]
            wqp[96 * i + 48:96 * i + 96] = hw_[1::2]
        wqt = np.stack([
            _lhsT_tiles(np.ascontiguousarray(
                wqp[128 * j:128 * (j + 1)].T), 128)
            for j in range(6)]).astype(bf16)
        wkt = np.stack([
            _lhsT_tiles(_permpad_rows(
                wk[(g * KVH + i) * HD:(g * KVH + i + 1) * HD]).T, 128)
            for i in range(KVH)]).astype(bf16)
        wvt = np.stack([
            _lhsT_tiles(np.ascontiguousarray(
                wv[(g * KVH + i) * HD:(g * KVH + i + 1) * HD].T), HD)
            for i in range(KVH)]).astype(bf16)
        # packed o_proj weights, partition-major: (128, NJ, NG3, 512)
        woT = np.ascontiguousarray(
            wo[:, g * QH * HD:(g + 1) * QH * HD].T)  # (768, 3072)
        wot = np.ascontiguousarray(
            woT.reshape(NG3, 128, NJ, 512).transpose(1, 2, 0, 3)).astype(bf16)
        in_maps.append({
            "xt": xts[b], "wqt": wqt, "wkt": wkt, "wvt": wvt, "wot": wot,
            "tabaq": tabaq.astype(bf16), "tabbq": tabbq.astype(bf16),
            "tabak": tabak.astype(bf16), "tabbk": tabbk.astype(bf16),
            "sels": sels, "selst": selst,

            "o128": np.ones((128, 1), np.float32),
            "o128T": np.ones((1, 128), np.float32),
            "o96": np.ones((1, HD), np.float32),
            "ocol": np.ones((128, KTOK), bf16),
        })
    return in_maps


def kernel(**inputs):
    from concourse import bass_utils

    nc = get_nc()
    in_maps = prepare_in_maps(
        inputs["x"], inputs["wq"], inputs["wk"], inputs["wv"], inputs["wo"],
        inputs["q_norm_w"], inputs["k_norm_w"], inputs["cos"], inputs["sin"])
    trace = bool(int(os.environ.get("BASS_KERNEL_TRACE", "0")))
    res = bass_utils.run_bass_kernel_spmd(
        nc, in_maps, core_ids=list(range(NCORES)), trace=trace)
    _BUILD_CACHE["last_result"] = res
    partials = [np.asarray(r["out"]) for r in res.results]
    out = np.empty((B, T, D), np.float32)
    for b in range(B):
        out[b] = np.sum(np.stack(partials[b * G:(b + 1) * G]), axis=0,
                        dtype=np.float64).astype(np.float32)
    return out


# revision 56
# speedup vs baseline: 1.0135x; 1.0135x over previous
"""Grouped-Query Attention block (RMSNorm + RoPE + causal GQA + o_proj) on 8 trn2 NeuronCores.

Sharding: data-parallel over batch (2) x tensor-parallel over kv-head groups (4).
Core c = b*4 + g handles batch b, kv heads {2g, 2g+1}, q heads {8g..8g+7}.
Each core computes a partial o_proj output (T, D) over its 768 head-dims;
host sums the 4 group partials per batch.

v3 structure:
  * bf16 inputs/weights/activations; accumulation + norm/softmax stats fp32.
  * All matmuls 512-wide moving dim: PE sequencer dispatch (~170ns/matmul)
    stays under engine execution (~213ns) -> engine-bound, not dispatch-bound.
  * DMAs batched (x: 4 single-DMA token-quarters, o_proj weights: 1 DMA,
    output: 1 DMA per 128-token row) and spread across the SP and Pool
    dispatch queues so no sequencer saturates.
  * x + all projection weights SBUF-resident; first two x quarters land on
    different queues in parallel -> compute starts ~5us in.
  * RMSNorm / softmax broadcast matmuls issue-deferred behind the next unit's
    accumulation matmuls so PE never stalls on the Act/DVE scalar chains.
  * o_proj contraction packed: 8 heads x 96 rows repacked (SBUF DMA) into
    6 full 128-row groups -> 6 instead of 8 matmuls per output tile; o_proj
    row-tiles for tokens 0:512 interleave into the attention stream as PE
    gap fillers while Act churns exps.
"""

import os
import sys

import numpy as np

sys.path.insert(0, "/opt/trn_rl_repo")

B, T, D = 2, 1024, 3072
NH, NKV, HD = 32, 8, 96
G = 4                 # tensor-parallel groups
QH = NH // G          # q heads per core (8)
KVH = NKV // G        # kv heads per core (2)
NCORES = 8
EPS = 1e-6
SCALE = 1.0 / float(np.sqrt(HD))
KT = D // 128         # 24 contraction tiles over d_model
KTOK = T // 128       # 8 token tiles
QCS = 256             # q chunk in phase 2
NQC = T // QCS        # 4
NKP = QCS // 128      # k tiles per q chunk
D_PIPE2 = 3           # phase-2 score matmuls issued ahead of P@V
NJ = D // 512         # 6 output column chunks
NG3 = QH * HD // 128  # 6 packed o_proj contraction groups
D_PIPE = 2            # phase-2 score matmuls issued ahead of P@V

_BUILD_CACHE = {}


def _build_nc():
    from contextlib import ExitStack
    from concourse import bacc, tile, mybir
    from concourse.masks import make_identity

    f32 = mybir.dt.float32
    f32r = mybir.dt.float32r
    bf16 = mybir.dt.bfloat16
    AF = mybir.ActivationFunctionType

    nc = bacc.Bacc("TRN2", target_bir_lowering=False, debug=False,
                   num_devices=NCORES)

    xt_d = nc.dram_tensor("xt", (128, KT, T), bf16, kind="ExternalInput").ap()
    wqt_d = nc.dram_tensor("wqt", (6, 128, KT, 128), bf16, kind="ExternalInput").ap()
    wkt_d = nc.dram_tensor("wkt", (KVH, 128, KT, 128), bf16, kind="ExternalInput").ap()
    wvt_d = nc.dram_tensor("wvt", (KVH, 128, KT, HD), bf16, kind="ExternalInput").ap()
    wot_d = nc.dram_tensor("wot", (128, NJ, NG3, 512), bf16, kind="ExternalInput").ap()
    tabaq_d = nc.dram_tensor("tabaq", (3, 128, T), bf16, kind="ExternalInput").ap()
    tabbq_d = nc.dram_tensor("tabbq", (3, 128, T), bf16, kind="ExternalInput").ap()
    sels_d = nc.dram_tensor("sels", (3, 128, 4), f32r, kind="ExternalInput").ap()
    selst_d = nc.dram_tensor("selst", (3, 4, 128), f32r, kind="ExternalInput").ap()
    tabak_d = nc.dram_tensor("tabak", (128, T), bf16, kind="ExternalInput").ap()
    tabbk_d = nc.dram_tensor("tabbk", (128, T), bf16, kind="ExternalInput").ap()
    o128_d = nc.dram_tensor("o128", (128, 1), f32r, kind="ExternalInput").ap()
    o96_d = nc.dram_tensor("o96", (1, HD), f32r, kind="ExternalInput").ap()
    o128T_d = nc.dram_tensor("o128T", (1, 128), f32r, kind="ExternalInput").ap()
    ocol_d = nc.dram_tensor("ocol", (128, KTOK), bf16, kind="ExternalInput").ap()
    out_d = nc.dram_tensor("out", (T, D), f32, kind="ExternalOutput").ap()

    with tile.TileContext(nc) as tc:
        with nc.allow_low_precision(reason="bf16 compute, fp32 accumulate"), \
             ExitStack() as ctx:
            const = ctx.enter_context(tc.tile_pool(name="const", bufs=1))
            p_qkv = ctx.enter_context(tc.tile_pool(name="p_qkv", bufs=1))

            eps_t = const.tile([1, 1], f32, tag="eps")
            nc.vector.memset(eps_t[:], EPS)
            warm_t = const.tile([1, 1], f32, tag="warm")
            nc.scalar.activation(warm_t[:], eps_t[:], AF.Exp, scale=1.0)
            eps4_t = const.tile([4, 1], f32, tag="eps4")
            nc.vector.memset(eps4_t[:], EPS)
            ones128 = const.tile([128, 1], f32r, tag="ones128")
            ones96 = const.tile([1, HD], f32r, tag="ones96")
            ones128T = const.tile([1, 128], f32r, tag="ones128T")
            ident = const.tile([128, 128], f32, tag="ident")

            qt = [p_qkv.tile([128, T], bf16, tag=f"qt{h}", name=f"qt{h}")
                  for h in range(QH)]
            for h in range(QH):
                nc.vector.memset(qt[h][32:64, :], 0.0)
                nc.vector.memset(qt[h][96:128, :], 0.0)
            ktl = [p_qkv.tile([128, T], bf16, tag=f"kt{g2}", name=f"kt{g2}")
                   for g2 in range(KVH)]
            vext = [p_qkv.tile([128, KTOK, HD + 1], bf16, tag=f"vx{g2}",
                               name=f"vx{g2}") for g2 in range(KVH)]
            atp = [p_qkv.tile([128, T], bf16, tag=f"atp{t}", name=f"atp{t}")
                   for t in range(NG3)]

            # ---------------- Phase 1: projections + RMSNorm + RoPE ---------
            with ExitStack() as s1:
                xt_pool = s1.enter_context(tc.tile_pool(name="xt", bufs=1))
                wkv_pool = s1.enter_context(tc.tile_pool(name="wkv", bufs=1))
                wq_pool = s1.enter_context(tc.tile_pool(name="wq", bufs=3))
                tab_pool = s1.enter_context(tc.tile_pool(name="tabs", bufs=1))
                tmp_pool = s1.enter_context(tc.tile_pool(name="tmp1", bufs=3))
                tmp4_pool = s1.enter_context(tc.tile_pool(name="tmp4", bufs=4))
                tmpb_pool = s1.enter_context(tc.tile_pool(name="tmpb", bufs=2))
                psk_pool = s1.enter_context(
                    tc.tile_pool(name="psk", bufs=1, space="PSUM"))
                psq_pool = s1.enter_context(
                    tc.tile_pool(name="psq", bufs=3, space="PSUM"))
                vtr_pool = s1.enter_context(
                    tc.tile_pool(name="vtr", bufs=1, space="PSUM"))
                ssq_pool = s1.enter_context(
                    tc.tile_pool(name="ssq", bufs=1, space="PSUM"))
                bc_pool = s1.enter_context(
                    tc.tile_pool(name="bc1", bufs=1, space="PSUM"))

                xt_t = [xt_pool.tile([128, KT, 512], bf16, tag=f"xh{hf}",
                                     name=f"xh{hf}") for hf in range(2)]
                wk_t = [wkv_pool.tile([128, KT, 128], bf16, tag=f"wk{i}",
                                      name=f"wk{i}") for i in range(KVH)]


                tabaq_t = [tab_pool.tile([128, T], bf16, tag=f"tabaq{p}",
                                          name=f"tabaq{p}") for p in range(3)]
                tabbq_t = [tab_pool.tile([128, T], bf16, tag=f"tabbq{p}",
                                          name=f"tabbq{p}") for p in range(3)]
                sels_t = [tab_pool.tile([128, 4], f32r, tag=f"sels{p}",
                                        name=f"sels{p}") for p in range(3)]
                selst_t = [tab_pool.tile([4, 128], f32r, tag=f"selst{p}",
                                         name=f"selst{p}") for p in range(3)]
                tabak_t = tab_pool.tile([128, T], bf16, tag="tabak")
                tabbk_t = tab_pool.tile([128, T], bf16, tag="tabbk")
                # static load order; wk races xh0 on the two HWDGE queues
                nc.scalar.dma_start(wk_t[0][:], wkt_d[0])
                nc.sync.dma_start(xt_t[0][:], xt_d[:, :, 0:512])
                nc.scalar.dma_start(wk_t[1][:], wkt_d[1])
                nc.sync.dma_start(xt_t[1][:], xt_d[:, :, 512:1024])
                nc.sync.dma_start(ones128[:], o128_d[:])
                nc.sync.dma_start(ones96[:], o96_d[:])
                nc.sync.dma_start(ones128T[:], o128T_d[:])
                make_identity(nc, ident[:])
                nc.scalar.dma_start(tabak_t[:], tabak_d[:])
                nc.scalar.dma_start(tabbk_t[:], tabbk_d[:])
                wq_tiles = {}

                def load_wq(j):
                    if j >= 6:
                        return
                    t = wq_pool.tile([128, KT, 128], bf16, tag="wq",
                                     name="wq")
                    nc.scalar.dma_start(t[:], wqt_d[j])
                    wq_tiles[j] = t

                for j in range(3):
                    load_wq(j)
                for p in range(3):
                    nc.scalar.dma_start(tabaq_t[p][:], tabaq_d[p])
                    nc.scalar.dma_start(tabbq_t[p][:], tabbq_d[p])
                    nc.sync.dma_start(sels_t[p][:], sels_d[p])
                    nc.sync.dma_start(selst_t[p][:], selst_d[p])
                for g2 in range(KVH):
                    nc.sync.dma_start(vext[g2][:, :, HD:HD + 1], ocol_d[:])

                def load_rest():
                    pass

                # --- norm + RoPE chain, split into two issue stages ---------
                def stage_a(ps, sq, rinv):
                    """ssq matmul (PE) + sqrt (Act) + recip (DVE)."""
                    ssq = ssq_pool.tile([4, 512], f32, tag="ssq")
                    nc.tensor.matmul(ssq[0:1, :], ones128[:], sq[:],
                                     start=True, stop=True)
                    rms = tmpb_pool.tile([1, 512], f32, tag="rms")
                    nc.scalar.activation(rms[:], ssq[0:1, :], AF.Sqrt,
                                         bias=eps_t[:], scale=1.0 / HD)
                    nc.vector.reciprocal(rinv[:], rms[:])

                def stage_b(ps, qsh, rinv, dst, hsl, ta, tb, ve):
                    """rinv broadcast (PE) + normalize + RoPE (Act/DVE).
                    ps/qsh are SBUF copies of the pre-norm projection and its
                    RoPE partner-row swap (DMA'd off the critical DVE chain).
                    The RMSNorm weights are folded into the RoPE tables
                    host-side, so one plain 1/rms broadcast serves both.
                    """
                    bc = bc_pool.tile([128, 512], f32, tag="bc")
                    nc.tensor.matmul(bc[:], ones128T[:], rinv[:],
                                     start=True, stop=True)
                    bcs = tmpb_pool.tile([128, 512], f32, tag="bcs")
                    nc.scalar.copy(bcs[:], bc[:])
                    sl = tmpb_pool.tile([128, 512], bf16, tag="slt")
                    ve.tensor_mul(sl[:], ps[:], bcs[:])
                    qsn = tmpb_pool.tile([128, 512], bf16, tag="qsn")
                    ve.tensor_mul(qsn[:], qsh[:], bcs[:])
                    # dst = sl*P + qsn*Q, P/Q full-height tables with the
                    # norm weights and the rotation signs folded in host-side
                    tm1 = tmpb_pool.tile([128, 512], bf16, tag="tm1")
                    ve.tensor_mul(tm1[:], sl[:], ta[:, hsl])
                    tm2 = tmpb_pool.tile([128, 512], bf16, tag="tm2")
                    ve.tensor_mul(tm2[:], qsn[:], tb[:, hsl])
                    ve.tensor_add(dst[:, hsl], tm1[:], tm2[:])

                # Deferred two-stage pipeline over accumulation units: the
                # PE ops of stage A/B for unit u are emitted after the accum
                # matmuls of units u+1 / u+2, so PE never waits on Act/DVE.
                chainq = []   # entries: [stage, a_thunk, b_thunk]

                def step_chain_b():
                    if chainq and chainq[0][0] == 1:
                        e = chainq.pop(0)
                        e[2]()

                def step_chain():
                    for e in chainq:
                        if e[0] == 0:
                            e[0] = 1
                            e[1]()
                            return

                def flush_chains():
                    while chainq:
                        step_chain_b()
                        step_chain()
                        if chainq and chainq[0][0] == 1:
                            e = chainq.pop(0)
                            e[2]()

                chain_no = [0]

                def make_chain(ps, dst, hsl, ta, tb):
                    # alternate DVE/Pool for RoPE; last chains stay on DVE so
                    # the Pool queue is clear for phase-2 affine_selects
                    ve = (nc.gpsimd if (chain_no[0] % 2 == 1
                                        and chain_no[0] < 14) else nc.vector)
                    chain_no[0] += 1
                    sq = tmp_pool.tile([128, 512], f32r, tag="sq")
                    nc.scalar.square(sq[:], ps[:])
                    # SBUF copy frees the PSUM tile and lets the RoPE partner
                    # swap run as a DMA concurrent with the norm chain
                    psb = tmp_pool.tile([128, 512], f32, tag="psb")
                    nc.scalar.copy(psb[:], ps[:])
                    qsh = tmp_pool.tile([128, 512], f32, tag="qsh")
                    nc.sync.dma_start(qsh[0:64, :], psb[64:128, :])
                    nc.sync.dma_start(qsh[64:128, :], psb[0:64, :])
                    rinv = tmp_pool.tile([1, 512], f32r, tag="rinv")
                    push = [0, lambda: stage_a(psb, sq, rinv),
                            lambda: stage_b(psb, qsh, rinv, dst, hsl,
                                            ta, tb, ve)]
                    chainq.append(push)

                # ---- k phase (first: x-paced; half 0 in token-quarters) ----
                psk = [psk_pool.tile([128, 512], f32, tag=f"psk{i}",
                                     name=f"psk{i}") for i in range(KVH)]
                for hf in range(2):
                    hsl = slice(hf * 512, (hf + 1) * 512)
                    for i in range(KVH):
                        for kt in range(KT):
                            nc.tensor.matmul(
                                psk[i][:], wk_t[i][:, kt, :],
                                xt_t[hf][:, kt, :],
                                start=(kt == 0), stop=(kt == KT - 1))
                        if hf == 0 and i == 0:
                            load_rest()
                        step_chain_b()
                        step_chain()
                        make_chain(psk[i], ktl[i], hsl, tabak_t, tabbk_t)

                # ---- q phase: heads packed 8x96 -> 6x128 ------------------
                # Two sets of 4 heads, 3 full 128-row groups each: 288 accum
                # matmuls instead of 384.  Per-head RMSNorm sums via selector
                # matmuls; RoPE in packed layout with per-group-pattern P/Q
                # tables; results DMA-repacked into the per-head padded qt
                # tiles so phase 2 is unchanged (zero k-pads keep the unset
                # qt pad rows harmless in the QK contraction).
                thunkq = []   # (append_step, fn) run one per step, lag >= 1
                stepc = [0]

                def step_thunk():
                    stepc[0] += 1
                    if thunkq and thunkq[0][0] < stepc[0]:
                        thunkq.pop(0)[1]()

                RUNS48 = []   # (start, partner_start, head4, is_odd)
                for h4 in range(4):
                    RUNS48.append((96 * h4, 96 * h4 + 48, h4, 0))
                    RUNS48.append((96 * h4 + 48, 96 * h4, h4, 1))

                for s in range(2):
                    for hf in range(2):
                        hsl = slice(hf * 512, (hf + 1) * 512)
                        sqs, psbs, qshs = [], [], []
                        for g in range(3):
                            ps = psq_pool.tile([128, 512], f32, tag="psq",
                                               name="ps")
                            wqg = wq_tiles[3 * s + g] if hf == 0 else \
                                wq_tiles[3 * s + g]
                            for kt in range(KT):
                                nc.tensor.matmul(
                                    ps[:], wqg[:, kt, :],
                                    xt_t[hf][:, kt, :],
                                    start=(kt == 0), stop=(kt == KT - 1))
                            if hf == 1:
                                load_wq(3 * s + g + 3)
                            step_chain_b()
                            step_chain()
                            step_thunk()
                            sq = tmp_pool.tile([128, 512], f32r, tag="sq",
                                               name="sq")
                            nc.scalar.square(sq[:], ps[:])
                            psb = tmp4_pool.tile([128, 512], f32, tag="psb",
                                                 name="psb")
                            nc.scalar.copy(psb[:], ps[:])
                            sqs.append(sq)
                            psbs.append(psb)
                        for g in range(3):
                            qshs.append(tmp4_pool.tile(
                                [128, 512], f32, tag="qsh", name="qsh"))
                        # partner-row swap, cross-group pieces
                        for (a, ap, h4, odd) in RUNS48:
                            pos = 0
                            while pos < 48:
                                dg, dp = divmod(a + pos, 128)
                                sg, sp = divmod(ap + pos, 128)
                                ln = min(48 - pos, 128 - dp, 128 - sp)
                                nc.sync.dma_start(
                                    qshs[dg][dp:dp + ln, :],
                                    psbs[sg][sp:sp + ln, :])
                                pos += ln
                        rinv4 = tmp_pool.tile([4, 512], f32r, tag="rinv4",
                                              name="rinv4")

                        def a_set(sqs=sqs, rinv4=rinv4):
                            ssq = ssq_pool.tile([4, 512], f32, tag="ssq",
                                                name="ssq")
                            for g in range(3):
                                nc.tensor.matmul(
                                    ssq[:], sels_t[g][:], sqs[g][:],
                                    start=(g == 0), stop=(g == 2))
                            rms4 = tmpb_pool.tile([4, 512], f32, tag="rms4",
                                                  name="rms4")
                            nc.scalar.activation(rms4[:], ssq[:], AF.Sqrt,
                                                 bias=eps4_t[:],
                                                 scale=1.0 / HD)
                            nc.vector.reciprocal(rinv4[:], rms4[:])
                        thunkq.append((stepc[0], a_set))

                        for g in range(3):
                            def b_g(g=g, s=s, hsl=hsl, psb=psbs[g],
                                    qsh=qshs[g], rinv4=rinv4):
                                bc = bc_pool.tile([128, 512], f32, tag="bc",
                                                  name="bc")
                                nc.tensor.matmul(bc[:], selst_t[g][:],
                                                 rinv4[:],
                                                 start=True, stop=True)
                                bcs = tmpb_pool.tile([128, 512], f32,
                                                     tag="bcs", name="bcs")
                                nc.scalar.copy(bcs[:], bc[:])
                                sl = tmpb_pool.tile([128, 512], bf16,
                                                    tag="slt", name="sl")
                                nc.vector.tensor_mul(sl[:], psb[:], bcs[:])
                                qsn = tmpb_pool.tile([128, 512], bf16,
                                                     tag="qsn", name="qsn")
                                nc.vector.tensor_mul(qsn[:], qsh[:], bcs[:])
                                tm1 = tmpb_pool.tile([128, 512], bf16,
                                                     tag="tm1", name="tm1")
                                nc.vector.tensor_mul(tm1[:], sl[:],
                                                     tabaq_t[g][:, hsl])
                                tm2 = tmpb_pool.tile([128, 512], bf16,
                                                     tag="tm2", name="tm2")
                                nc.vector.tensor_mul(tm2[:], qsn[:],
                                                     tabbq_t[g][:, hsl])
                                qp = tmpb_pool.tile([128, 512], bf16,
                                                    tag="slt", name="qp")
                                nc.vector.tensor_add(qp[:], tm1[:], tm2[:])
                                # repack into per-head padded qt layout
                                base = 128 * g
                                for (a, ap, h4, odd) in RUNS48:
                                    lo = max(a, base)
                                    hi = min(a + 48, base + 128)
                                    if lo >= hi:
                                        continue
                                    o0 = lo - a
                                    d0 = (64 + o0) if odd else o0
                                    nc.sync.dma_start(
                                        qt[4 * s + h4][d0:d0 + hi - lo, hsl],
                                        qp[lo - base:hi - base, :])
                            thunkq.append((stepc[0], b_g))

                # ---- v phase last: no norm chains -- the trailing q chains
                # drain on Act/DVE behind these accums, so the phase-2 scope
                # switch sees no backlog
                wv_t = []
                for i in range(KVH):
                    t = wq_pool.tile([128, KT, 128], bf16, tag="wq",
                                     name="wv")
                    nc.scalar.dma_start(t[:, :, 0:HD], wvt_d[i])
                    wv_t.append(t)
                for hf in range(2):
                    hsl = slice(hf * 512, (hf + 1) * 512)
                    for i in range(KVH):
                        vp = psq_pool.tile([128, 512], f32, tag="psq")
                        vps = vp[0:HD, :]
                        for kt in range(KT):
                            nc.tensor.matmul(
                                vps, wv_t[i][:, kt, 0:HD],
                                xt_t[hf][:, kt, :],
                                start=(kt == 0), stop=(kt == KT - 1))
                        vt = tab_pool.tile([HD, 512], f32, tag="vt",
                                           name="vt")
                        nc.scalar.copy(vt[:], vp[0:HD, :])
                        step_chain_b()
                        step_chain()
                        step_thunk()
                        step_thunk()
                        for c in range(4):
                            tp = vtr_pool.tile([128, HD], f32, tag="tp")
                            nc.tensor.transpose(
                                tp[:], vt[:, c * 128:(c + 1) * 128],
                                ident[0:HD, 0:HD])
                            itok = hf * 4 + c
                            nc.scalar.copy(vext[i][:, itok, 0:HD], tp[:])
                        step_thunk()
                flush_chains()
                while thunkq:
                    thunkq.pop(0)[1]()

            # -------- Phase 2+3: attention with o_proj interleaved ----------
            # q-chunks outer, heads inner: once all 8 heads finish chunk qc,
            # tokens [512qc, 512qc+512) are fully attended and their o_proj
            # row-tiles are emitted as PE gap-fillers while the Act engine
            # works through the next chunk group's exps.
            with ExitStack() as s2:
                pt_pool = s2.enter_context(tc.tile_pool(name="pt", bufs=5))
                tmp2_pool = s2.enter_context(tc.tile_pool(name="tmp2", bufs=3))
                wo_pool = s2.enter_context(tc.tile_pool(name="wo3", bufs=1))
                ob_pool = s2.enter_context(tc.tile_pool(name="ob", bufs=4))
                sc_pool = s2.enter_context(
                    tc.tile_pool(name="sc", bufs=3, space="PSUM"))
                po_pool = s2.enter_context(
                    tc.tile_pool(name="po", bufs=2, space="PSUM"))
                bc2_pool = s2.enter_context(
                    tc.tile_pool(name="bc2", bufs=1, space="PSUM"))
                ps3_pool = s2.enter_context(
                    tc.tile_pool(name="ps3", bufs=2, space="PSUM"))

                wo_t = [wo_pool.tile([128, NG3, 512], bf16, tag=f"wo3{j}",
                                     name=f"wo3{j}") for j in range(NJ)]

                # flattened (qc, h, kt2) item list; sc/exp/select emitted
                # D_PIPE items ahead of the corresponding P@V matmul.
                items = []
                for qc in range(NQC):
                    for h in range(QH):
                        for kt2 in range(NKP * qc + NKP):
                            items.append((h, qc, kt2))

                po_t = {}
                pts = {}
                norm_q = []
                ph3_q = []
                ob_t = {}

                def emit_sc(idx):
                    h, qc, kt2 = items[idx]
                    g2 = h // (QH // KVH)
                    jsl = slice(qc * QCS, (qc + 1) * QCS)
                    sc = sc_pool.tile([128, QCS], f32, tag="sc")
                    nc.tensor.matmul(
                        sc[:], ktl[g2][:, kt2 * 128:(kt2 + 1) * 128],
                        qt[h][:, jsl], start=True, stop=True)
                    pt = pt_pool.tile([128, QCS], bf16, tag="pt")
                    nc.scalar.activation(pt[:], sc[:], AF.Exp, scale=SCALE)
                    if kt2 >= NKP * qc:
                        nc.gpsimd.affine_select(
                            pt[:], pt[:], pattern=[[1, QCS]],
                            compare_op=mybir.AluOpType.is_ge,
                            fill=0.0,
                            base=qc * QCS - kt2 * 128,
                            channel_multiplier=-1)
                    pts[idx] = pt

                def emit_pv(idx):
                    h, qc, kt2 = items[idx]
                    g2 = h // (QH // KVH)
                    if kt2 == 0:
                        po_t[(h, qc)] = po_pool.tile([HD + 1, QCS], f32,
                                                     tag="po", name="po")
                    po = po_t[(h, qc)]
                    nc.tensor.matmul(
                        po[:], vext[g2][:, kt2, :], pts.pop(idx)[:],
                        start=(kt2 == 0),
                        stop=(kt2 == NKP * qc + NKP - 1))
                    if kt2 == NKP * qc + NKP - 1:
                        rinv2 = tmp2_pool.tile([1, QCS], f32r, tag="rinv2")
                        nc.vector.reciprocal(rinv2[:], po[HD:HD + 1, :])
                        norm_q.append((h, qc, rinv2))

                def emit_norm():
                    if not norm_q:
                        return
                    h, qc, rinv2 = norm_q.pop(0)
                    jsl = slice(qc * QCS, (qc + 1) * QCS)
                    po = po_t.pop((h, qc))
                    bc2 = bc2_pool.tile([HD, QCS], f32, tag="bc2")
                    nc.tensor.matmul(bc2[:], ones96[:], rinv2[:],
                                     start=True, stop=True)
                    bc2s = tmp2_pool.tile([HD, QCS], f32, tag="bc2s")
                    nc.scalar.copy(bc2s[:], bc2[:])
                    at_tmp = tmp2_pool.tile([HD, QCS], bf16, tag="at_tmp")
                    nc.vector.tensor_mul(at_tmp[:], po[0:HD, :], bc2s[:])
                    # repack rows 96h..96h+96 into the 6x128 contraction tiles
                    r0 = h * HD
                    while r0 < (h + 1) * HD:
                        t, p = divmod(r0, 128)
                        rows = min(128 - p, (h + 1) * HD - r0)
                        nc.sync.dma_start(
                            atp[t][p:p + rows, jsl],
                            at_tmp[r0 - h * HD:r0 - h * HD + rows, :])
                        r0 += rows
                    if h == QH - 1:
                        for j3 in range(NJ):
                            for i in range(NKP * qc, NKP * qc + NKP):
                                ph3_q.append((i, j3))

                def emit_ph3():
                    if not ph3_q:
                        return
                    i, j3 = ph3_q.pop(0)
                    isl = slice(i * 128, (i + 1) * 128)
                    ps3 = ps3_pool.tile([128, 512], f32, tag="ps3")
                    for t in range(NG3):
                        nc.tensor.matmul(
                            ps3[:], atp[t][:, isl], wo_t[j3][:, t, :],
                            start=(t == 0), stop=(t == NG3 - 1))
                    jsl = slice(j3 * 512, (j3 + 1) * 512)
                    ob = ob_pool.tile([128, 512], f32, tag="ob", name="ob")
                    nc.vector.tensor_copy(ob[:], ps3[:])
                    nc.sync.dma_start(out_d[isl, jsl], ob[:])

                for idx in range(len(items)):
                    if idx % 4 == 0 and idx // 4 < NJ:
                        j = idx // 4
                        nc.scalar.dma_start(wo_t[j][:], wot_d[:, j])
                    emit_sc(idx)
                    if idx >= D_PIPE2:
                        emit_pv(idx - D_PIPE2)
                        emit_norm()
                    if idx % 3 == 2 and len(ph3_q) > 2:
                        emit_ph3()
                for idx in range(len(items) - D_PIPE2, len(items)):
                    emit_pv(idx)
                    emit_norm()
                while norm_q:
                    emit_norm()
                while ph3_q:
                    emit_ph3()

    nc.compile()
    return nc


def get_nc():
    if "nc" not in _BUILD_CACHE:
        _BUILD_CACHE["nc"] = _build_nc()
    return _BUILD_CACHE["nc"]


def _permpad_rows(w96):
    """(96, N) head rows -> (128, N): evens at 0:48, odds at 64:112, pad 0."""
    out = np.zeros((128, w96.shape[1]), np.float32)
    out[0:48] = w96[0::2]
    out[64:112] = w96[1::2]
    return out


def _lhsT_tiles(wT, m):
    """(D, m) -> (128, KT, m) lhsT tile layout (contraction on partitions)."""
    return np.ascontiguousarray(
        wT.reshape(KT, 128, m).transpose(1, 0, 2)).astype(np.float32)


def prepare_in_maps(x, wq, wk, wv, wo, q_norm_w, k_norm_w, cos, sin):
    import ml_dtypes
    bf16 = ml_dtypes.bfloat16

    x = np.asarray(x, np.float32)
    wq = np.asarray(wq, np.float32)
    wk = np.asarray(wk, np.float32)
    wv = np.asarray(wv, np.float32)
    wo = np.asarray(wo, np.float32)
    cos = np.asarray(cos, np.float32)
    sin = np.asarray(sin, np.float32)
    q_norm_w = np.asarray(q_norm_w, np.float32)
    k_norm_w = np.asarray(k_norm_w, np.float32)

    def _fold_tabs(nw):
        # P multiplies the in-place operand sl, Q the partner-swapped qsn:
        #   evens rows: out = a*we*ce - b*wo*se -> P=we*ce, Q=-wo*se
        #   odds rows:  out = b*wo*co + a*we*so -> P=wo*co, Q=+we*so
        nwe = nw[0::2][:, None]
        nwo = nw[1::2][:, None]
        ta = np.zeros((128, T), np.float32)
        tb = np.zeros((128, T), np.float32)
        ta[0:48] = nwe * cos[:, 0::2].T
        ta[64:112] = nwo * cos[:, 1::2].T
        tb[0:48] = -nwo * sin[:, 0::2].T
        tb[64:112] = nwe * sin[:, 1::2].T
        return ta, tb

    tabak, tabbk = _fold_tabs(k_norm_w)

    # packed-layout q tables/selectors: set-local row R (of 384) -> head
    # h4 = R//96, local l = R%96; l<48 = even component e=l, else odd o=l-48
    tabaq = np.zeros((3, 128, T), np.float32)
    tabbq = np.zeros((3, 128, T), np.float32)
    sels = np.zeros((3, 128, 4), np.float32)
    for p in range(3):
        for r in range(128):
            R = 128 * p + r
            h4, l = divmod(R, 96)
            sels[p, r, h4] = 1.0
            if l < 48:
                e = l
                tabaq[p, r] = q_norm_w[2 * e] * cos[:, 2 * e]
                tabbq[p, r] = -q_norm_w[2 * e + 1] * sin[:, 2 * e]
            else:
                o = l - 48
                tabaq[p, r] = q_norm_w[2 * o + 1] * cos[:, 2 * o + 1]
                tabbq[p, r] = q_norm_w[2 * o] * sin[:, 2 * o + 1]
    selst = np.ascontiguousarray(sels.transpose(0, 2, 1))

    xts = []
    for b in range(B):
        xT = np.ascontiguousarray(x[b].T)  # (D, T)
        xts.append(np.ascontiguousarray(
            xT.reshape(KT, 128, T).transpose(1, 0, 2)).astype(bf16))

    in_maps = []
    for c in range(NCORES):
        b, g = divmod(c, G)
        # packed q weights: 8 heads x 96 rows (evens then odds per head)
        # -> 6 full 128-row groups
        wqp = np.zeros((QH * HD, D), np.float32)
        for i in range(QH):
            hw_ = wq[(g * QH + i) * HD:(g * QH + i + 1) * HD]
            wqp[96 * i:96 * i + 48] = hw_[0::2]
            wqp[96 * i + 48:96 * i + 96] = hw_[1::2]
        wqt = np.stack([
            _lhsT_tiles(np.ascontiguousarray(
                wqp[128 * j:128 * (j + 1)].T), 128)
            for j in range(6)]).astype(bf16)
        wkt = np.stack([
            _lhsT_tiles(_permpad_rows(
                wk[(g * KVH + i) * HD:(g * KVH + i + 1) * HD]).T, 128)
            for i in range(KVH)]).astype(bf16)
        wvt = np.stack([
            _lhsT_tiles(np.ascontiguousarray(
                wv[(g * KVH + i) * HD:(g * KVH + i + 1) * HD].T), HD)
            for i in range(KVH)]).astype(bf16)
        # packed o_proj weights, partition-major: (128, NJ, NG3, 512)
        woT = np.ascontiguousarray(
            wo[:, g * QH * HD:(g + 1) * QH * HD].T)  # (768, 3072)
        wot = np.ascontiguousarray(
            woT.reshape(NG3, 128, NJ, 512).transpose(1, 2, 0, 3)).astype(bf16)
        in_maps.append({
            "xt": xts[b], "wqt": wqt, "wkt": wkt, "wvt": wvt, "wot": wot,
            "tabaq": tabaq.astype(bf16), "tabbq": tabbq.astype(bf16),
            "tabak": tabak.astype(bf16), "tabbk": tabbk.astype(bf16),
            "sels": sels, "selst": selst,

            "o128": np.ones((128, 1), np.float32),
            "o128T": np.ones((1, 128), np.float32),
            "o96": np.ones((1, HD), np.float32),
            "ocol": np.ones((128, KTOK), bf16),
        })
    return in_maps


def kernel(**inputs):
    from concourse import bass_utils

    nc = get_nc()
    in_maps = prepare_in_maps(
        inputs["x"], inputs["wq"], inputs["wk"], inputs["wv"], inputs["wo"],
        inputs["q_norm_w"], inputs["k_norm_w"], inputs["cos"], inputs["sin"])
    trace = bool(int(os.environ.get("BASS_KERNEL_TRACE", "0")))
    res = bass_utils.run_bass_kernel_spmd(
        nc, in_maps, core_ids=list(range(NCORES)), trace=trace)
    _BUILD_CACHE["last_result"] = res
    partials = [np.asarray(r["out"]) for r in res.results]
    out = np.empty((B, T, D), np.float32)
    for b in range(B):
        out[b] = np.sum(np.stack(partials[b * G:(b + 1) * G]), axis=0,
                        dtype=np.float64).astype(np.float32)
    return out


# revision 59
# speedup vs baseline: 1.0228x; 1.0092x over previous
"""Grouped-Query Attention block (RMSNorm + RoPE + causal GQA + o_proj) on 8 trn2 NeuronCores.

Sharding: data-parallel over batch (2) x tensor-parallel over kv-head groups (4).
Core c = b*4 + g handles batch b, kv heads {2g, 2g+1}, q heads {8g..8g+7}.
Each core computes a partial o_proj output (T, D) over its 768 head-dims;
host sums the 4 group partials per batch.

v3 structure:
  * bf16 inputs/weights/activations; accumulation + norm/softmax stats fp32.
  * All matmuls 512-wide moving dim: PE sequencer dispatch (~170ns/matmul)
    stays under engine execution (~213ns) -> engine-bound, not dispatch-bound.
  * DMAs batched (x: 4 single-DMA token-quarters, o_proj weights: 1 DMA,
    output: 1 DMA per 128-token row) and spread across the SP and Pool
    dispatch queues so no sequencer saturates.
  * x + all projection weights SBUF-resident; first two x quarters land on
    different queues in parallel -> compute starts ~5us in.
  * RMSNorm / softmax broadcast matmuls issue-deferred behind the next unit's
    accumulation matmuls so PE never stalls on the Act/DVE scalar chains.
  * o_proj contraction packed: 8 heads x 96 rows repacked (SBUF DMA) into
    6 full 128-row groups -> 6 instead of 8 matmuls per output tile; o_proj
    row-tiles for tokens 0:512 interleave into the attention stream as PE
    gap fillers while Act churns exps.
"""

import os
import sys

import numpy as np

sys.path.insert(0, "/opt/trn_rl_repo")

B, T, D = 2, 1024, 3072
NH, NKV, HD = 32, 8, 96
G = 4                 # tensor-parallel groups
QH = NH // G          # q heads per core (8)
KVH = NKV // G        # kv heads per core (2)
NCORES = 8
EPS = 1e-6
SCALE = 1.0 / float(np.sqrt(HD))
KT = D // 128         # 24 contraction tiles over d_model
KTOK = T // 128       # 8 token tiles
QCS = 256             # q chunk in phase 2
NQC = T // QCS        # 4
NKP = QCS // 128      # k tiles per q chunk
D_PIPE2 = 3           # phase-2 score matmuls issued ahead of P@V
NJ = D // 512         # 6 output column chunks
NG3 = QH * HD // 128  # 6 packed o_proj contraction groups
D_PIPE = 2            # phase-2 score matmuls issued ahead of P@V

_BUILD_CACHE = {}


def _build_nc():
    from contextlib import ExitStack
    from concourse import bacc, tile, mybir
    from concourse.masks import make_identity

    f32 = mybir.dt.float32
    f32r = mybir.dt.float32r
    bf16 = mybir.dt.bfloat16
    AF = mybir.ActivationFunctionType

    nc = bacc.Bacc("TRN2", target_bir_lowering=False, debug=False,
                   num_devices=NCORES)

    xt_d = nc.dram_tensor("xt", (128, KT, T), bf16, kind="ExternalInput").ap()
    wqt_d = nc.dram_tensor("wqt", (6, 128, KT, 128), bf16, kind="ExternalInput").ap()
    wkt_d = nc.dram_tensor("wkt", (KVH, 128, KT, 128), bf16, kind="ExternalInput").ap()
    wvt_d = nc.dram_tensor("wvt", (KVH, 128, KT, HD), bf16, kind="ExternalInput").ap()
    wot_d = nc.dram_tensor("wot", (128, NJ, NG3, 512), bf16, kind="ExternalInput").ap()
    tabaq_d = nc.dram_tensor("tabaq", (3, 128, T), bf16, kind="ExternalInput").ap()
    tabbq_d = nc.dram_tensor("tabbq", (3, 128, T), bf16, kind="ExternalInput").ap()
    sels_d = nc.dram_tensor("sels", (3, 128, 4), f32r, kind="ExternalInput").ap()
    selst_d = nc.dram_tensor("selst", (3, 4, 128), f32r, kind="ExternalInput").ap()
    tabak_d = nc.dram_tensor("tabak", (128, T), bf16, kind="ExternalInput").ap()
    tabbk_d = nc.dram_tensor("tabbk", (128, T), bf16, kind="ExternalInput").ap()
    o128_d = nc.dram_tensor("o128", (128, 1), f32r, kind="ExternalInput").ap()
    o96_d = nc.dram_tensor("o96", (1, HD), f32r, kind="ExternalInput").ap()
    o128T_d = nc.dram_tensor("o128T", (1, 128), f32r, kind="ExternalInput").ap()
    ocol_d = nc.dram_tensor("ocol", (128, KTOK), bf16, kind="ExternalInput").ap()
    out_d = nc.dram_tensor("out", (T, D), f32, kind="ExternalOutput").ap()

    with tile.TileContext(nc) as tc:
        with nc.allow_low_precision(reason="bf16 compute, fp32 accumulate"), \
             ExitStack() as ctx:
            const = ctx.enter_context(tc.tile_pool(name="const", bufs=1))
            p_qkv = ctx.enter_context(tc.tile_pool(name="p_qkv", bufs=1))

            eps_t = const.tile([1, 1], f32, tag="eps")
            nc.vector.memset(eps_t[:], EPS)
            warm_t = const.tile([1, 1], f32, tag="warm")
            nc.scalar.activation(warm_t[:], eps_t[:], AF.Exp, scale=1.0)
            eps4_t = const.tile([4, 1], f32, tag="eps4")
            nc.vector.memset(eps4_t[:], EPS)
            ones128 = const.tile([128, 1], f32r, tag="ones128")
            ones96 = const.tile([1, HD], f32r, tag="ones96")
            ones128T = const.tile([1, 128], f32r, tag="ones128T")
            ident = const.tile([128, 128], f32, tag="ident")

            qt = [p_qkv.tile([128, T], bf16, tag=f"qt{h}", name=f"qt{h}")
                  for h in range(QH)]
            for h in range(QH):
                nc.vector.memset(qt[h][32:64, :], 0.0)
                nc.vector.memset(qt[h][96:128, :], 0.0)
            ktl = [p_qkv.tile([128, T], bf16, tag=f"kt{g2}", name=f"kt{g2}")
                   for g2 in range(KVH)]
            vext = [p_qkv.tile([128, KTOK, HD + 1], bf16, tag=f"vx{g2}",
                               name=f"vx{g2}") for g2 in range(KVH)]
            atp = [p_qkv.tile([128, T], bf16, tag=f"atp{t}", name=f"atp{t}")
                   for t in range(NG3)]

            # ---------------- Phase 1: projections + RMSNorm + RoPE ---------
            with ExitStack() as s1:
                xt_pool = s1.enter_context(tc.tile_pool(name="xt", bufs=1))
                wkv_pool = s1.enter_context(tc.tile_pool(name="wkv", bufs=1))
                wq_pool = s1.enter_context(tc.tile_pool(name="wq", bufs=3))
                tab_pool = s1.enter_context(tc.tile_pool(name="tabs", bufs=1))
                tmp_pool = s1.enter_context(tc.tile_pool(name="tmp1", bufs=3))
                tmp4_pool = s1.enter_context(tc.tile_pool(name="tmp4", bufs=4))
                tmpb_pool = s1.enter_context(tc.tile_pool(name="tmpb", bufs=2))
                psk_pool = s1.enter_context(
                    tc.tile_pool(name="psk", bufs=1, space="PSUM"))
                psq_pool = s1.enter_context(
                    tc.tile_pool(name="psq", bufs=3, space="PSUM"))
                vtr_pool = s1.enter_context(
                    tc.tile_pool(name="vtr", bufs=1, space="PSUM"))
                ssq_pool = s1.enter_context(
                    tc.tile_pool(name="ssq", bufs=1, space="PSUM"))
                bc_pool = s1.enter_context(
                    tc.tile_pool(name="bc1", bufs=1, space="PSUM"))

                xt_t = [xt_pool.tile([128, KT, 512], bf16, tag=f"xh{hf}",
                                     name=f"xh{hf}") for hf in range(2)]
                wk_t = [wkv_pool.tile([128, KT, 128], bf16, tag=f"wk{i}",
                                      name=f"wk{i}") for i in range(KVH)]


                tabaq_t = [tab_pool.tile([128, T], bf16, tag=f"tabaq{p}",
                                          name=f"tabaq{p}") for p in range(3)]
                tabbq_t = [tab_pool.tile([128, T], bf16, tag=f"tabbq{p}",
                                          name=f"tabbq{p}") for p in range(3)]
                sels_t = [tab_pool.tile([128, 4], f32r, tag=f"sels{p}",
                                        name=f"sels{p}") for p in range(3)]
                selst_t = [tab_pool.tile([4, 128], f32r, tag=f"selst{p}",
                                         name=f"selst{p}") for p in range(3)]
                tabak_t = tab_pool.tile([128, T], bf16, tag="tabak")
                tabbk_t = tab_pool.tile([128, T], bf16, tag="tabbk")
                # static load order; wk races xh0 on the two HWDGE queues
                nc.scalar.dma_start(wk_t[0][:], wkt_d[0])
                nc.sync.dma_start(xt_t[0][:], xt_d[:, :, 0:512])
                nc.scalar.dma_start(wk_t[1][:], wkt_d[1])
                nc.sync.dma_start(xt_t[1][:], xt_d[:, :, 512:1024])
                nc.sync.dma_start(ones128[:], o128_d[:])
                nc.sync.dma_start(ones96[:], o96_d[:])
                nc.sync.dma_start(ones128T[:], o128T_d[:])
                make_identity(nc, ident[:])
                nc.scalar.dma_start(tabak_t[:], tabak_d[:])
                nc.scalar.dma_start(tabbk_t[:], tabbk_d[:])
                wq_tiles = {}

                def load_wq(j):
                    if j >= 6:
                        return
                    t = wq_pool.tile([128, KT, 128], bf16, tag="wq",
                                     name="wq")
                    nc.scalar.dma_start(t[:], wqt_d[j])
                    wq_tiles[j] = t

                for j in range(3):
                    load_wq(j)
                for p in range(3):
                    nc.scalar.dma_start(tabaq_t[p][:], tabaq_d[p])
                    nc.scalar.dma_start(tabbq_t[p][:], tabbq_d[p])
                    nc.sync.dma_start(sels_t[p][:], sels_d[p])
                    nc.sync.dma_start(selst_t[p][:], selst_d[p])
                for g2 in range(KVH):
                    nc.sync.dma_start(vext[g2][:, :, HD:HD + 1], ocol_d[:])

                def load_rest():
                    pass

                # --- norm + RoPE chain, split into two issue stages ---------
                def stage_a(ps, sq, rinv):
                    """ssq matmul (PE) + sqrt (Act) + recip (DVE)."""
                    ssq = ssq_pool.tile([4, 512], f32, tag="ssq")
                    nc.tensor.matmul(ssq[0:1, :], ones128[:], sq[:],
                                     start=True, stop=True)
                    rms = tmpb_pool.tile([1, 512], f32, tag="rms")
                    nc.scalar.activation(rms[:], ssq[0:1, :], AF.Sqrt,
                                         bias=eps_t[:], scale=1.0 / HD)
                    nc.vector.reciprocal(rinv[:], rms[:])

                def stage_b(ps, qsh, rinv, dst, hsl, ta, tb, ve):
                    """rinv broadcast (PE) + normalize + RoPE (Act/DVE).
                    ps/qsh are SBUF copies of the pre-norm projection and its
                    RoPE partner-row swap (DMA'd off the critical DVE chain).
                    The RMSNorm weights are folded into the RoPE tables
                    host-side, so one plain 1/rms broadcast serves both.
                    """
                    bc = bc_pool.tile([128, 512], f32, tag="bc")
                    nc.tensor.matmul(bc[:], ones128T[:], rinv[:],
                                     start=True, stop=True)
                    bcs = tmpb_pool.tile([128, 512], f32, tag="bcs")
                    nc.scalar.copy(bcs[:], bc[:])
                    sl = tmpb_pool.tile([128, 512], bf16, tag="slt")
                    ve.tensor_mul(sl[:], ps[:], bcs[:])
                    qsn = tmpb_pool.tile([128, 512], bf16, tag="qsn")
                    ve.tensor_mul(qsn[:], qsh[:], bcs[:])
                    # dst = sl*P + qsn*Q, P/Q full-height tables with the
                    # norm weights and the rotation signs folded in host-side
                    tm1 = tmpb_pool.tile([128, 512], bf16, tag="tm1")
                    ve.tensor_mul(tm1[:], sl[:], ta[:, hsl])
                    tm2 = tmpb_pool.tile([128, 512], bf16, tag="tm2")
                    ve.tensor_mul(tm2[:], qsn[:], tb[:, hsl])
                    ve.tensor_add(dst[:, hsl], tm1[:], tm2[:])

                # Deferred two-stage pipeline over accumulation units: the
                # PE ops of stage A/B for unit u are emitted after the accum
                # matmuls of units u+1 / u+2, so PE never waits on Act/DVE.
                chainq = []   # entries: [stage, a_thunk, b_thunk]

                def step_chain_b():
                    if chainq and chainq[0][0] == 1:
                        e = chainq.pop(0)
                        e[2]()

                def step_chain():
                    for e in chainq:
                        if e[0] == 0:
                            e[0] = 1
                            e[1]()
                            return

                def flush_chains():
                    while chainq:
                        step_chain_b()
                        step_chain()
                        if chainq and chainq[0][0] == 1:
                            e = chainq.pop(0)
                            e[2]()

                chain_no = [0]

                def make_chain(ps, dst, hsl, ta, tb):
                    # alternate DVE/Pool for RoPE; last chains stay on DVE so
                    # the Pool queue is clear for phase-2 affine_selects
                    ve = (nc.gpsimd if (chain_no[0] % 2 == 1
                                        and chain_no[0] < 14) else nc.vector)
                    chain_no[0] += 1
                    sq = tmp_pool.tile([128, 512], f32r, tag="sq")
                    nc.scalar.square(sq[:], ps[:])
                    # SBUF copy frees the PSUM tile and lets the RoPE partner
                    # swap run as a DMA concurrent with the norm chain
                    psb = tmp_pool.tile([128, 512], f32, tag="psb")
                    nc.scalar.copy(psb[:], ps[:])
                    qsh = tmp_pool.tile([128, 512], f32, tag="qsh")
                    nc.sync.dma_start(qsh[0:64, :], psb[64:128, :])
                    nc.sync.dma_start(qsh[64:128, :], psb[0:64, :])
                    rinv = tmp_pool.tile([1, 512], f32r, tag="rinv")
                    push = [0, lambda: stage_a(psb, sq, rinv),
                            lambda: stage_b(psb, qsh, rinv, dst, hsl,
                                            ta, tb, ve)]
                    chainq.append(push)

                # ---- k phase (first: x-paced; half 0 in token-quarters) ----
                psk = [psk_pool.tile([128, 512], f32, tag=f"psk{i}",
                                     name=f"psk{i}") for i in range(KVH)]
                for hf in range(2):
                    hsl = slice(hf * 512, (hf + 1) * 512)
                    for i in range(KVH):
                        for kt in range(KT):
                            nc.tensor.matmul(
                                psk[i][:], wk_t[i][:, kt, :],
                                xt_t[hf][:, kt, :],
                                start=(kt == 0), stop=(kt == KT - 1))
                        if hf == 0 and i == 0:
                            load_rest()
                        step_chain_b()
                        step_chain()
                        make_chain(psk[i], ktl[i], hsl, tabak_t, tabbk_t)

                # ---- q phase: heads packed 8x96 -> 6x128 ------------------
                # Two sets of 4 heads, 3 full 128-row groups each: 288 accum
                # matmuls instead of 384.  Per-head RMSNorm sums via selector
                # matmuls; RoPE in packed layout with per-group-pattern P/Q
                # tables; results DMA-repacked into the per-head padded qt
                # tiles so phase 2 is unchanged (zero k-pads keep the unset
                # qt pad rows harmless in the QK contraction).
                thunkq = []   # (append_step, fn) run one per step, lag >= 1
                stepc = [0]

                def step_thunk():
                    stepc[0] += 1
                    if thunkq and thunkq[0][0] < stepc[0]:
                        thunkq.pop(0)[1]()

                RUNS48 = []   # (start, partner_start, head4, is_odd)
                for h4 in range(4):
                    RUNS48.append((96 * h4, 96 * h4 + 48, h4, 0))
                    RUNS48.append((96 * h4 + 48, 96 * h4, h4, 1))

                for s in range(2):
                    for hf in range(2):
                        hsl = slice(hf * 512, (hf + 1) * 512)
                        sqs, psbs, qshs = [], [], []
                        for g in range(3):
                            ps = psq_pool.tile([128, 512], f32, tag="psq",
                                               name="ps")
                            wqg = wq_tiles[3 * s + g] if hf == 0 else \
                                wq_tiles[3 * s + g]
                            for kt in range(KT):
                                nc.tensor.matmul(
                                    ps[:], wqg[:, kt, :],
                                    xt_t[hf][:, kt, :],
                                    start=(kt == 0), stop=(kt == KT - 1))
                            if hf == 1:
                                load_wq(3 * s + g + 3)
                            step_chain_b()
                            step_chain()
                            step_thunk()
                            sq = tmp_pool.tile([128, 512], f32r, tag="sq",
                                               name="sq")
                            nc.scalar.square(sq[:], ps[:])
                            psb = tmp4_pool.tile([128, 512], f32, tag="psb",
                                                 name="psb")
                            nc.scalar.copy(psb[:], ps[:])
                            sqs.append(sq)
                            psbs.append(psb)
                        for g in range(3):
                            qshs.append(tmp4_pool.tile(
                                [128, 512], f32, tag="qsh", name="qsh"))
                        # partner-row swap, cross-group pieces
                        for (a, ap, h4, odd) in RUNS48:
                            pos = 0
                            while pos < 48:
                                dg, dp = divmod(a + pos, 128)
                                sg, sp = divmod(ap + pos, 128)
                                ln = min(48 - pos, 128 - dp, 128 - sp)
                                nc.sync.dma_start(
                                    qshs[dg][dp:dp + ln, :],
                                    psbs[sg][sp:sp + ln, :])
                                pos += ln
                        rinv4 = tmp_pool.tile([4, 512], f32r, tag="rinv4",
                                              name="rinv4")

                        def a_set(sqs=sqs, rinv4=rinv4):
                            ssq = ssq_pool.tile([4, 512], f32, tag="ssq",
                                                name="ssq")
                            for g in range(3):
                                nc.tensor.matmul(
                                    ssq[:], sels_t[g][:], sqs[g][:],
                                    start=(g == 0), stop=(g == 2))
                            rms4 = tmpb_pool.tile([4, 512], f32, tag="rms4",
                                                  name="rms4")
                            nc.scalar.activation(rms4[:], ssq[:], AF.Sqrt,
                                                 bias=eps4_t[:],
                                                 scale=1.0 / HD)
                            nc.vector.reciprocal(rinv4[:], rms4[:])
                        thunkq.append((stepc[0], a_set))

                        for g in range(3):
                            def b_g(g=g, s=s, hsl=hsl, psb=psbs[g],
                                    qsh=qshs[g], rinv4=rinv4):
                                bc = bc_pool.tile([128, 512], f32, tag="bc",
                                                  name="bc")
                                nc.tensor.matmul(bc[:], selst_t[g][:],
                                                 rinv4[:],
                                                 start=True, stop=True)
                                bcs = tmpb_pool.tile([128, 512], f32,
                                                     tag="bcs", name="bcs")
                                nc.scalar.copy(bcs[:], bc[:])
                                sl = tmpb_pool.tile([128, 512], bf16,
                                                    tag="slt", name="sl")
                                nc.vector.tensor_mul(sl[:], psb[:], bcs[:])
                                qsn = tmpb_pool.tile([128, 512], bf16,
                                                     tag="qsn", name="qsn")
                                nc.vector.tensor_mul(qsn[:], qsh[:], bcs[:])
                                tm1 = tmpb_pool.tile([128, 512], bf16,
                                                     tag="tm1", name="tm1")
                                nc.vector.tensor_mul(tm1[:], sl[:],
                                                     tabaq_t[g][:, hsl])
                                tm2 = tmpb_pool.tile([128, 512], bf16,
                                                     tag="tm2", name="tm2")
                                nc.vector.tensor_mul(tm2[:], qsn[:],
                                                     tabbq_t[g][:, hsl])
                                qp = tmpb_pool.tile([128, 512], bf16,
                                                    tag="slt", name="qp")
                                nc.vector.tensor_add(qp[:], tm1[:], tm2[:])
                                # repack into per-head padded qt layout
                                base = 128 * g
                                for (a, ap, h4, odd) in RUNS48:
                                    lo = max(a, base)
                                    hi = min(a + 48, base + 128)
                                    if lo >= hi:
                                        continue
                                    o0 = lo - a
                                    d0 = (64 + o0) if odd else o0
                                    nc.sync.dma_start(
                                        qt[4 * s + h4][d0:d0 + hi - lo, hsl],
                                        qp[lo - base:hi - base, :])
                            thunkq.append((stepc[0], b_g))

                # ---- v phase last: no norm chains -- the trailing q chains
                # drain on Act/DVE behind these accums, so the phase-2 scope
                # switch sees no backlog
                wv_t = []
                for i in range(KVH):
                    t = wq_pool.tile([128, KT, 128], bf16, tag="wq",
                                     name="wv")
                    nc.scalar.dma_start(t[:, :, 0:HD], wvt_d[i])
                    wv_t.append(t)
                for hf in range(2):
                    hsl = slice(hf * 512, (hf + 1) * 512)
                    for i in range(KVH):
                        vp = psq_pool.tile([128, 512], f32, tag="psq")
                        vps = vp[0:HD, :]
                        for kt in range(KT):
                            nc.tensor.matmul(
                                vps, wv_t[i][:, kt, 0:HD],
                                xt_t[hf][:, kt, :],
                                start=(kt == 0), stop=(kt == KT - 1))
                        vt = tab_pool.tile([HD, 512], f32, tag="vt",
                                           name="vt")
                        nc.scalar.copy(vt[:], vp[0:HD, :])
                        step_chain_b()
                        step_chain()
                        step_thunk()
                        step_thunk()
                        for c in range(4):
                            tp = vtr_pool.tile([128, HD], f32, tag="tp")
                            nc.tensor.transpose(
                                tp[:], vt[:, c * 128:(c + 1) * 128],
                                ident[0:HD, 0:HD])
                            itok = hf * 4 + c
                            nc.scalar.copy(vext[i][:, itok, 0:HD], tp[:])
                        step_thunk()
                flush_chains()
                while thunkq:
                    thunkq.pop(0)[1]()

            # -------- Phase 2+3: attention with o_proj interleaved ----------
            # q-chunks outer, heads inner: once all 8 heads finish chunk qc,
            # tokens [512qc, 512qc+512) are fully attended and their o_proj
            # row-tiles are emitted as PE gap-fillers while the Act engine
            # works through the next chunk group's exps.
            with ExitStack() as s2:
                pt_pool = s2.enter_context(tc.tile_pool(name="pt", bufs=5))
                tmp2_pool = s2.enter_context(tc.tile_pool(name="tmp2", bufs=3))
                wo_pool = s2.enter_context(tc.tile_pool(name="wo3", bufs=1))
                ob_pool = s2.enter_context(tc.tile_pool(name="ob", bufs=4))
                sc_pool = s2.enter_context(
                    tc.tile_pool(name="sc", bufs=3, space="PSUM"))
                po_pool = s2.enter_context(
                    tc.tile_pool(name="po", bufs=2, space="PSUM"))
                bc2_pool = s2.enter_context(
                    tc.tile_pool(name="bc2", bufs=1, space="PSUM"))
                ps3_pool = s2.enter_context(
                    tc.tile_pool(name="ps3", bufs=2, space="PSUM"))

                wo_t = [wo_pool.tile([128, NG3, 512], bf16, tag=f"wo3{j}",
                                     name=f"wo3{j}") for j in range(NJ)]

                # flattened (qc, h, kt2) item list; sc/exp/select emitted
                # D_PIPE items ahead of the corresponding P@V matmul.
                items = []
                for qc in range(NQC):
                    for h in range(QH):
                        for kt2 in range(NKP * qc + NKP):
                            items.append((h, qc, kt2))

                po_t = {}
                pts = {}
                norm_q = []
                ph3_q = []
                ob_t = {}

                def emit_sc(idx):
                    h, qc, kt2 = items[idx]
                    g2 = h // (QH // KVH)
                    jsl = slice(qc * QCS, (qc + 1) * QCS)
                    sc = sc_pool.tile([128, QCS], f32, tag="sc")
                    nc.tensor.matmul(
                        sc[:], ktl[g2][:, kt2 * 128:(kt2 + 1) * 128],
                        qt[h][:, jsl], start=True, stop=True)
                    pt = pt_pool.tile([128, QCS], bf16, tag="pt")
                    nc.scalar.activation(pt[:], sc[:], AF.Exp, scale=SCALE)
                    if kt2 >= NKP * qc:
                        nc.gpsimd.affine_select(
                            pt[:], pt[:], pattern=[[1, QCS]],
                            compare_op=mybir.AluOpType.is_ge,
                            fill=0.0,
                            base=qc * QCS - kt2 * 128,
                            channel_multiplier=-1)
                    pts[idx] = pt

                def emit_pv(idx):
                    h, qc, kt2 = items[idx]
                    g2 = h // (QH // KVH)
                    if kt2 == 0:
                        po_t[(h, qc)] = po_pool.tile([HD + 1, QCS], f32,
                                                     tag="po", name="po")
                    po = po_t[(h, qc)]
                    nc.tensor.matmul(
                        po[:], vext[g2][:, kt2, :], pts.pop(idx)[:],
                        start=(kt2 == 0),
                        stop=(kt2 == NKP * qc + NKP - 1))
                    if kt2 == NKP * qc + NKP - 1:
                        rinv2 = tmp2_pool.tile([1, QCS], f32r, tag="rinv2")
                        nc.vector.reciprocal(rinv2[:], po[HD:HD + 1, :])
                        norm_q.append((h, qc, rinv2))

                def emit_norm():
                    if not norm_q:
                        return
                    h, qc, rinv2 = norm_q.pop(0)
                    jsl = slice(qc * QCS, (qc + 1) * QCS)
                    po = po_t.pop((h, qc))
                    bc2 = bc2_pool.tile([HD, QCS], f32, tag="bc2")
                    nc.tensor.matmul(bc2[:], ones96[:], rinv2[:],
                                     start=True, stop=True)
                    bc2s = tmp2_pool.tile([HD, QCS], f32, tag="bc2s")
                    nc.scalar.copy(bc2s[:], bc2[:])
                    at_tmp = tmp2_pool.tile([HD, QCS], bf16, tag="at_tmp")
                    nc.vector.tensor_mul(at_tmp[:], po[0:HD, :], bc2s[:])
                    # repack rows 96h..96h+96 into the 6x128 contraction tiles
                    r0 = h * HD
                    while r0 < (h + 1) * HD:
                        t, p = divmod(r0, 128)
                        rows = min(128 - p, (h + 1) * HD - r0)
                        nc.sync.dma_start(
                            atp[t][p:p + rows, jsl],
                            at_tmp[r0 - h * HD:r0 - h * HD + rows, :])
                        r0 += rows
                    if h == QH - 1:
                        for j3 in range(NJ):
                            for i in range(NKP * qc, NKP * qc + NKP):
                                ph3_q.append((i, j3))

                def emit_ph3():
                    if not ph3_q:
                        return
                    i, j3 = ph3_q.pop(0)
                    isl = slice(i * 128, (i + 1) * 128)
                    ps3 = ps3_pool.tile([128, 512], f32, tag="ps3")
                    for t in range(NG3):
                        nc.tensor.matmul(
                            ps3[:], atp[t][:, isl], wo_t[j3][:, t, :],
                            start=(t == 0), stop=(t == NG3 - 1))
                    jsl = slice(j3 * 512, (j3 + 1) * 512)
                    ob = ob_pool.tile([128, 512], f32, tag="ob", name="ob")
                    nc.vector.tensor_copy(ob[:], ps3[:])
                    nc.sync.dma_start(out_d[isl, jsl], ob[:])

                for idx in range(len(items)):
                    if idx % 6 == 0 and idx // 6 < NJ:
                        j = idx // 6
                        nc.scalar.dma_start(wo_t[j][:], wot_d[:, j])
                    emit_sc(idx)
                    if idx >= D_PIPE2:
                        emit_pv(idx - D_PIPE2)
                        emit_norm()
                    if idx % 3 == 2 and len(ph3_q) > 2:
                        emit_ph3()
                for idx in range(len(items) - D_PIPE2, len(items)):
                    emit_pv(idx)
                    emit_norm()
                while norm_q:
                    emit_norm()
                while ph3_q:
                    emit_ph3()

    nc.compile()
    return nc


def get_nc():
    if "nc" not in _BUILD_CACHE:
        _BUILD_CACHE["nc"] = _build_nc()
    return _BUILD_CACHE["nc"]


def _permpad_rows(w96):
    """(96, N) head rows -> (128, N): evens at 0:48, odds at 64:112, pad 0."""
    out = np.zeros((128, w96.shape[1]), np.float32)
    out[0:48] = w96[0::2]
    out[64:112] = w96[1::2]
    return out


def _lhsT_tiles(wT, m):
    """(D, m) -> (128, KT, m) lhsT tile layout (contraction on partitions)."""
    return np.ascontiguousarray(
        wT.reshape(KT, 128, m).transpose(1, 0, 2)).astype(np.float32)


def prepare_in_maps(x, wq, wk, wv, wo, q_norm_w, k_norm_w, cos, sin):
    import ml_dtypes
    bf16 = ml_dtypes.bfloat16

    x = np.asarray(x, np.float32)
    wq = np.asarray(wq, np.float32)
    wk = np.asarray(wk, np.float32)
    wv = np.asarray(wv, np.float32)
    wo = np.asarray(wo, np.float32)
    cos = np.asarray(cos, np.float32)
    sin = np.asarray(sin, np.float32)
    q_norm_w = np.asarray(q_norm_w, np.float32)
    k_norm_w = np.asarray(k_norm_w, np.float32)

    def _fold_tabs(nw):
        # P multiplies the in-place operand sl, Q the partner-swapped qsn:
        #   evens rows: out = a*we*ce - b*wo*se -> P=we*ce, Q=-wo*se
        #   odds rows:  out = b*wo*co + a*we*so -> P=wo*co, Q=+we*so
        nwe = nw[0::2][:, None]
        nwo = nw[1::2][:, None]
        ta = np.zeros((128, T), np.float32)
        tb = np.zeros((128, T), np.float32)
        ta[0:48] = nwe * cos[:, 0::2].T
        ta[64:112] = nwo * cos[:, 1::2].T
        tb[0:48] = -nwo * sin[:, 0::2].T
        tb[64:112] = nwe * sin[:, 1::2].T
        return ta, tb

    tabak, tabbk = _fold_tabs(k_norm_w)

    # packed-layout q tables/selectors: set-local row R (of 384) -> head
    # h4 = R//96, local l = R%96; l<48 = even component e=l, else odd o=l-48
    tabaq = np.zeros((3, 128, T), np.float32)
    tabbq = np.zeros((3, 128, T), np.float32)
    sels = np.zeros((3, 128, 4), np.float32)
    for p in range(3):
        for r in range(128):
            R = 128 * p + r
            h4, l = divmod(R, 96)
            sels[p, r, h4] = 1.0
            if l < 48:
                e = l
                tabaq[p, r] = q_norm_w[2 * e] * cos[:, 2 * e]
                tabbq[p, r] = -q_norm_w[2 * e + 1] * sin[:, 2 * e]
            else:
                o = l - 48
                tabaq[p, r] = q_norm_w[2 * o + 1] * cos[:, 2 * o + 1]
                tabbq[p, r] = q_norm_w[2 * o] * sin[:, 2 * o + 1]
    selst = np.ascontiguousarray(sels.transpose(0, 2, 1))

    xts = []
    for b in range(B):
        xT = np.ascontiguousarray(x[b].T)  # (D, T)
        xts.append(np.ascontiguousarray(
            xT.reshape(KT, 128, T).transpose(1, 0, 2)).astype(bf16))

    in_maps = []
    for c in range(NCORES):
        b, g = divmod(c, G)
        # packed q weights: 8 heads x 96 rows (evens then odds per head)
        # -> 6 full 128-row groups
        wqp = np.zeros((QH * HD, D), np.float32)
        for i in range(QH):
            hw_ = wq[(g * QH + i) * HD:(g * QH + i + 1) * HD]
            wqp[96 * i:96 * i + 48] = hw_[0::2]
            wqp[96 * i + 48:96 * i + 96] = hw_[1::2]
        wqt = np.stack([
            _lhsT_tiles(np.ascontiguousarray(
                wqp[128 * j:128 * (j + 1)].T), 128)
            for j in range(6)]).astype(bf16)
        wkt = np.stack([
            _lhsT_tiles(_permpad_rows(
                wk[(g * KVH + i) * HD:(g * KVH + i + 1) * HD]).T, 128)
            for i in range(KVH)]).astype(bf16)
        wvt = np.stack([
            _lhsT_tiles(np.ascontiguousarray(
                wv[(g * KVH + i) * HD:(g * KVH + i + 1) * HD].T), HD)
            for i in range(KVH)]).astype(bf16)
        # packed o_proj weights, partition-major: (128, NJ, NG3, 512)
        woT = np.ascontiguousarray(
            wo[:, g * QH * HD:(g + 1) * QH * HD].T)  # (768, 3072)
        wot = np.ascontiguousarray(
            woT.reshape(NG3, 128, NJ, 512).transpose(1, 2, 0, 3)).astype(bf16)
        in_maps.append({
            "xt": xts[b], "wqt": wqt, "wkt": wkt, "wvt": wvt, "wot": wot,
            "tabaq": tabaq.astype(bf16), "tabbq": tabbq.astype(bf16),
            "tabak": tabak.astype(bf16), "tabbk": tabbk.astype(bf16),
            "sels": sels, "selst": selst,

            "o128": np.ones((128, 1), np.float32),
            "o128T": np.ones((1, 128), np.float32),
            "o96": np.ones((1, HD), np.float32),
            "ocol": np.ones((128, KTOK), bf16),
        })
    return in_maps


def kernel(**inputs):
    from concourse import bass_utils

    nc = get_nc()
    in_maps = prepare_in_maps(
        inputs["x"], inputs["wq"], inputs["wk"], inputs["wv"], inputs["wo"],
        inputs["q_norm_w"], inputs["k_norm_w"], inputs["cos"], inputs["sin"])
    trace = bool(int(os.environ.get("BASS_KERNEL_TRACE", "0")))
    res = bass_utils.run_bass_kernel_spmd(
        nc, in_maps, core_ids=list(range(NCORES)), trace=trace)
    _BUILD_CACHE["last_result"] = res
    partials = [np.asarray(r["out"]) for r in res.results]
    out = np.empty((B, T, D), np.float32)
    for b in range(B):
        out[b] = np.sum(np.stack(partials[b * G:(b + 1) * G]), axis=0,
                        dtype=np.float64).astype(np.float32)
    return out


# revision 62
# speedup vs baseline: 1.0267x; 1.0038x over previous
"""Grouped-Query Attention block (RMSNorm + RoPE + causal GQA + o_proj) on 8 trn2 NeuronCores.

Sharding: data-parallel over batch (2) x tensor-parallel over kv-head groups (4).
Core c = b*4 + g handles batch b, kv heads {2g, 2g+1}, q heads {8g..8g+7}.
Each core computes a partial o_proj output (T, D) over its 768 head-dims;
host sums the 4 group partials per batch.

v3 structure:
  * bf16 inputs/weights/activations; accumulation + norm/softmax stats fp32.
  * All matmuls 512-wide moving dim: PE sequencer dispatch (~170ns/matmul)
    stays under engine execution (~213ns) -> engine-bound, not dispatch-bound.
  * DMAs batched (x: 4 single-DMA token-quarters, o_proj weights: 1 DMA,
    output: 1 DMA per 128-token row) and spread across the SP and Pool
    dispatch queues so no sequencer saturates.
  * x + all projection weights SBUF-resident; first two x quarters land on
    different queues in parallel -> compute starts ~5us in.
  * RMSNorm / softmax broadcast matmuls issue-deferred behind the next unit's
    accumulation matmuls so PE never stalls on the Act/DVE scalar chains.
  * o_proj contraction packed: 8 heads x 96 rows repacked (SBUF DMA) into
    6 full 128-row groups -> 6 instead of 8 matmuls per output tile; o_proj
    row-tiles for tokens 0:512 interleave into the attention stream as PE
    gap fillers while Act churns exps.
"""

import os
import sys

import numpy as np

sys.path.insert(0, "/opt/trn_rl_repo")

B, T, D = 2, 1024, 3072
NH, NKV, HD = 32, 8, 96
G = 4                 # tensor-parallel groups
QH = NH // G          # q heads per core (8)
KVH = NKV // G        # kv heads per core (2)
NCORES = 8
EPS = 1e-6
SCALE = 1.0 / float(np.sqrt(HD))
KT = D // 128         # 24 contraction tiles over d_model
KTOK = T // 128       # 8 token tiles
QCS = 256             # q chunk in phase 2
NQC = T // QCS        # 4
NKP = QCS // 128      # k tiles per q chunk
D_PIPE2 = 3           # phase-2 score matmuls issued ahead of P@V
NJ = D // 512         # 6 output column chunks
NG3 = QH * HD // 128  # 6 packed o_proj contraction groups
D_PIPE = 2            # phase-2 score matmuls issued ahead of P@V

_BUILD_CACHE = {}


def _build_nc():
    from contextlib import ExitStack
    from concourse import bacc, tile, mybir
    from concourse.masks import make_identity

    f32 = mybir.dt.float32
    f32r = mybir.dt.float32r
    bf16 = mybir.dt.bfloat16
    AF = mybir.ActivationFunctionType

    nc = bacc.Bacc("TRN2", target_bir_lowering=False, debug=False,
                   num_devices=NCORES)

    xt_d = nc.dram_tensor("xt", (128, KT, T), bf16, kind="ExternalInput").ap()
    wqt_d = nc.dram_tensor("wqt", (6, 128, KT, 128), bf16, kind="ExternalInput").ap()
    wkt_d = nc.dram_tensor("wkt", (KVH, 128, KT, 128), bf16, kind="ExternalInput").ap()
    wvt_d = nc.dram_tensor("wvt", (KVH, 128, KT, HD), bf16, kind="ExternalInput").ap()
    wot_d = nc.dram_tensor("wot", (128, NJ, NG3, 512), bf16, kind="ExternalInput").ap()
    tabaq_d = nc.dram_tensor("tabaq", (3, 128, T), bf16, kind="ExternalInput").ap()
    tabbq_d = nc.dram_tensor("tabbq", (3, 128, T), bf16, kind="ExternalInput").ap()
    sels_d = nc.dram_tensor("sels", (3, 128, 4), f32r, kind="ExternalInput").ap()
    selst_d = nc.dram_tensor("selst", (3, 4, 128), f32r, kind="ExternalInput").ap()
    tabak_d = nc.dram_tensor("tabak", (128, T), bf16, kind="ExternalInput").ap()
    tabbk_d = nc.dram_tensor("tabbk", (128, T), bf16, kind="ExternalInput").ap()
    o128_d = nc.dram_tensor("o128", (128, 1), f32r, kind="ExternalInput").ap()
    o96_d = nc.dram_tensor("o96", (1, HD), f32r, kind="ExternalInput").ap()
    o128T_d = nc.dram_tensor("o128T", (1, 128), f32r, kind="ExternalInput").ap()
    ocol_d = nc.dram_tensor("ocol", (128, KTOK), bf16, kind="ExternalInput").ap()
    out_d = nc.dram_tensor("out", (T, D), f32, kind="ExternalOutput").ap()

    with tile.TileContext(nc) as tc:
        with nc.allow_low_precision(reason="bf16 compute, fp32 accumulate"), \
             ExitStack() as ctx:
            const = ctx.enter_context(tc.tile_pool(name="const", bufs=1))
            p_qkv = ctx.enter_context(tc.tile_pool(name="p_qkv", bufs=1))

            eps_t = const.tile([1, 1], f32, tag="eps")
            nc.vector.memset(eps_t[:], EPS)
            warm_t = const.tile([1, 1], f32, tag="warm")
            nc.scalar.activation(warm_t[:], eps_t[:], AF.Exp, scale=1.0)
            eps4_t = const.tile([4, 1], f32, tag="eps4")
            nc.vector.memset(eps4_t[:], EPS)
            ones128 = const.tile([128, 1], f32r, tag="ones128")
            ones96 = const.tile([1, HD], f32r, tag="ones96")
            ones128T = const.tile([1, 128], f32r, tag="ones128T")
            ident = const.tile([128, 128], f32, tag="ident")

            qt = [p_qkv.tile([128, T], bf16, tag=f"qt{h}", name=f"qt{h}")
                  for h in range(QH)]
            for h in range(QH):
                nc.vector.memset(qt[h][32:64, :], 0.0)
                nc.vector.memset(qt[h][96:128, :], 0.0)
            ktl = [p_qkv.tile([128, T], bf16, tag=f"kt{g2}", name=f"kt{g2}")
                   for g2 in range(KVH)]
            vext = [p_qkv.tile([128, KTOK, HD + 1], bf16, tag=f"vx{g2}",
                               name=f"vx{g2}") for g2 in range(KVH)]
            atp = [p_qkv.tile([128, T], bf16, tag=f"atp{t}", name=f"atp{t}")
                   for t in range(NG3)]

            # ---------------- Phase 1: projections + RMSNorm + RoPE ---------
            with ExitStack() as s1:
                xt_pool = s1.enter_context(tc.tile_pool(name="xt", bufs=1))
                wkv_pool = s1.enter_context(tc.tile_pool(name="wkv", bufs=1))
                wq_pool = s1.enter_context(tc.tile_pool(name="wq", bufs=3))
                tab_pool = s1.enter_context(tc.tile_pool(name="tabs", bufs=1))
                tmp_pool = s1.enter_context(tc.tile_pool(name="tmp1", bufs=3))
                tmp4_pool = s1.enter_context(tc.tile_pool(name="tmp4", bufs=4))
                tmpb_pool = s1.enter_context(tc.tile_pool(name="tmpb", bufs=2))
                psk_pool = s1.enter_context(
                    tc.tile_pool(name="psk", bufs=1, space="PSUM"))
                psq_pool = s1.enter_context(
                    tc.tile_pool(name="psq", bufs=3, space="PSUM"))
                vtr_pool = s1.enter_context(
                    tc.tile_pool(name="vtr", bufs=1, space="PSUM"))
                ssq_pool = s1.enter_context(
                    tc.tile_pool(name="ssq", bufs=1, space="PSUM"))
                bc_pool = s1.enter_context(
                    tc.tile_pool(name="bc1", bufs=1, space="PSUM"))

                xt_t = [xt_pool.tile([128, KT, 512], bf16, tag=f"xh{hf}",
                                     name=f"xh{hf}") for hf in range(2)]
                wk_t = [wkv_pool.tile([128, KT, 128], bf16, tag=f"wk{i}",
                                      name=f"wk{i}") for i in range(KVH)]


                tabaq_t = [tab_pool.tile([128, T], bf16, tag=f"tabaq{p}",
                                          name=f"tabaq{p}") for p in range(3)]
                tabbq_t = [tab_pool.tile([128, T], bf16, tag=f"tabbq{p}",
                                          name=f"tabbq{p}") for p in range(3)]
                sels_t = [tab_pool.tile([128, 4], f32r, tag=f"sels{p}",
                                        name=f"sels{p}") for p in range(3)]
                selst_t = [tab_pool.tile([4, 128], f32r, tag=f"selst{p}",
                                         name=f"selst{p}") for p in range(3)]
                tabak_t = tab_pool.tile([128, T], bf16, tag="tabak")
                tabbk_t = tab_pool.tile([128, T], bf16, tag="tabbk")
                # static load order; wk races xh0 on the two HWDGE queues
                nc.scalar.dma_start(wk_t[0][:], wkt_d[0])
                nc.sync.dma_start(xt_t[0][:], xt_d[:, :, 0:512])
                nc.scalar.dma_start(wk_t[1][:], wkt_d[1])
                nc.sync.dma_start(xt_t[1][:], xt_d[:, :, 512:1024])
                nc.sync.dma_start(ones128[:], o128_d[:])
                nc.sync.dma_start(ones96[:], o96_d[:])
                nc.sync.dma_start(ones128T[:], o128T_d[:])
                make_identity(nc, ident[:])
                nc.scalar.dma_start(tabak_t[:], tabak_d[:])
                nc.scalar.dma_start(tabbk_t[:], tabbk_d[:])
                wq_tiles = {}

                def load_wq(j):
                    if j >= 6:
                        return
                    t = wq_pool.tile([128, KT, 128], bf16, tag="wq",
                                     name="wq")
                    nc.scalar.dma_start(t[:], wqt_d[j])
                    wq_tiles[j] = t

                for j in range(3):
                    load_wq(j)
                for p in range(3):
                    nc.scalar.dma_start(tabaq_t[p][:], tabaq_d[p])
                    nc.scalar.dma_start(tabbq_t[p][:], tabbq_d[p])
                    nc.sync.dma_start(sels_t[p][:], sels_d[p])
                    nc.sync.dma_start(selst_t[p][:], selst_d[p])
                for g2 in range(KVH):
                    nc.sync.dma_start(vext[g2][:, :, HD:HD + 1], ocol_d[:])

                def load_rest():
                    pass

                # --- norm + RoPE chain, split into two issue stages ---------
                def stage_a(ps, sq, rinv):
                    """ssq matmul (PE) + sqrt (Act) + recip (DVE)."""
                    ssq = ssq_pool.tile([4, 512], f32, tag="ssq")
                    nc.tensor.matmul(ssq[0:1, :], ones128[:], sq[:],
                                     start=True, stop=True)
                    rms = tmpb_pool.tile([1, 512], f32, tag="rms")
                    nc.scalar.activation(rms[:], ssq[0:1, :], AF.Sqrt,
                                         bias=eps_t[:], scale=1.0 / HD)
                    nc.vector.reciprocal(rinv[:], rms[:])

                def stage_b(ps, qsh, rinv, dst, hsl, ta, tb, ve):
                    """rinv broadcast (PE) + normalize + RoPE (Act/DVE).
                    ps/qsh are SBUF copies of the pre-norm projection and its
                    RoPE partner-row swap (DMA'd off the critical DVE chain).
                    The RMSNorm weights are folded into the RoPE tables
                    host-side, so one plain 1/rms broadcast serves both.
                    """
                    bc = bc_pool.tile([128, 512], f32, tag="bc")
                    nc.tensor.matmul(bc[:], ones128T[:], rinv[:],
                                     start=True, stop=True)
                    bcs = tmpb_pool.tile([128, 512], f32, tag="bcs")
                    nc.scalar.copy(bcs[:], bc[:])
                    sl = tmpb_pool.tile([128, 512], bf16, tag="slt")
                    ve.tensor_mul(sl[:], ps[:], bcs[:])
                    qsn = tmpb_pool.tile([128, 512], bf16, tag="qsn")
                    ve.tensor_mul(qsn[:], qsh[:], bcs[:])
                    # dst = sl*P + qsn*Q, P/Q full-height tables with the
                    # norm weights and the rotation signs folded in host-side
                    tm1 = tmpb_pool.tile([128, 512], bf16, tag="tm1")
                    ve.tensor_mul(tm1[:], sl[:], ta[:, hsl])
                    tm2 = tmpb_pool.tile([128, 512], bf16, tag="tm2")
                    ve.tensor_mul(tm2[:], qsn[:], tb[:, hsl])
                    ve.tensor_add(dst[:, hsl], tm1[:], tm2[:])

                # Deferred two-stage pipeline over accumulation units: the
                # PE ops of stage A/B for unit u are emitted after the accum
                # matmuls of units u+1 / u+2, so PE never waits on Act/DVE.
                chainq = []   # entries: [stage, a_thunk, b_thunk]

                def step_chain_b():
                    if chainq and chainq[0][0] == 1:
                        e = chainq.pop(0)
                        e[2]()

                def step_chain():
                    for e in chainq:
                        if e[0] == 0:
                            e[0] = 1
                            e[1]()
                            return

                def flush_chains():
                    while chainq:
                        step_chain_b()
                        step_chain()
                        if chainq and chainq[0][0] == 1:
                            e = chainq.pop(0)
                            e[2]()

                chain_no = [0]

                def make_chain(ps, dst, hsl, ta, tb):
                    # alternate DVE/Pool for RoPE; last chains stay on DVE so
                    # the Pool queue is clear for phase-2 affine_selects
                    ve = (nc.gpsimd if (chain_no[0] % 2 == 1
                                        and chain_no[0] < 14) else nc.vector)
                    chain_no[0] += 1
                    sq = tmp_pool.tile([128, 512], f32r, tag="sq")
                    nc.scalar.square(sq[:], ps[:])
                    # SBUF copy frees the PSUM tile and lets the RoPE partner
                    # swap run as a DMA concurrent with the norm chain
                    psb = tmp_pool.tile([128, 512], f32, tag="psb")
                    nc.scalar.copy(psb[:], ps[:])
                    qsh = tmp_pool.tile([128, 512], f32, tag="qsh")
                    nc.sync.dma_start(qsh[0:64, :], psb[64:128, :])
                    nc.sync.dma_start(qsh[64:128, :], psb[0:64, :])
                    rinv = tmp_pool.tile([1, 512], f32r, tag="rinv")
                    push = [0, lambda: stage_a(psb, sq, rinv),
                            lambda: stage_b(psb, qsh, rinv, dst, hsl,
                                            ta, tb, ve)]
                    chainq.append(push)

                # ---- k phase (first: x-paced; half 0 in token-quarters) ----
                psk = [psk_pool.tile([128, 512], f32, tag=f"psk{i}",
                                     name=f"psk{i}") for i in range(KVH)]
                for hf in range(2):
                    hsl = slice(hf * 512, (hf + 1) * 512)
                    for i in range(KVH):
                        for kt in range(KT):
                            nc.tensor.matmul(
                                psk[i][:], wk_t[i][:, kt, :],
                                xt_t[hf][:, kt, :],
                                start=(kt == 0), stop=(kt == KT - 1))
                        if hf == 0 and i == 0:
                            load_rest()
                        step_chain_b()
                        step_chain()
                        make_chain(psk[i], ktl[i], hsl, tabak_t, tabbk_t)

                # ---- q phase: heads packed 8x96 -> 6x128 ------------------
                # Two sets of 4 heads, 3 full 128-row groups each: 288 accum
                # matmuls instead of 384.  Per-head RMSNorm sums via selector
                # matmuls; RoPE in packed layout with per-group-pattern P/Q
                # tables; results DMA-repacked into the per-head padded qt
                # tiles so phase 2 is unchanged (zero k-pads keep the unset
                # qt pad rows harmless in the QK contraction).
                thunkq = []   # (append_step, fn) run one per step, lag >= 1
                stepc = [0]

                def step_thunk():
                    stepc[0] += 1
                    if thunkq and thunkq[0][0] < stepc[0]:
                        thunkq.pop(0)[1]()

                RUNS48 = []   # (start, partner_start, head4, is_odd)
                for h4 in range(4):
                    RUNS48.append((96 * h4, 96 * h4 + 48, h4, 0))
                    RUNS48.append((96 * h4 + 48, 96 * h4, h4, 1))

                for s in range(2):
                    for hf in range(2):
                        hsl = slice(hf * 512, (hf + 1) * 512)
                        sqs, psbs, qshs = [], [], []
                        for g in range(3):
                            ps = psq_pool.tile([128, 512], f32, tag="psq",
                                               name="ps")
                            wqg = wq_tiles[3 * s + g] if hf == 0 else \
                                wq_tiles[3 * s + g]
                            for kt in range(KT):
                                nc.tensor.matmul(
                                    ps[:], wqg[:, kt, :],
                                    xt_t[hf][:, kt, :],
                                    start=(kt == 0), stop=(kt == KT - 1))
                            if hf == 1:
                                load_wq(3 * s + g + 3)
                            step_chain_b()
                            step_chain()
                            step_thunk()
                            sq = tmp_pool.tile([128, 512], f32r, tag="sq",
                                               name="sq")
                            nc.scalar.square(sq[:], ps[:])
                            psb = tmp4_pool.tile([128, 512], f32, tag="psb",
                                                 name="psb")
                            nc.scalar.copy(psb[:], ps[:])
                            sqs.append(sq)
                            psbs.append(psb)
                        for g in range(3):
                            qshs.append(tmp4_pool.tile(
                                [128, 512], f32, tag="qsh", name="qsh"))
                        # partner-row swap, cross-group pieces
                        for (a, ap, h4, odd) in RUNS48:
                            pos = 0
                            while pos < 48:
                                dg, dp = divmod(a + pos, 128)
                                sg, sp = divmod(ap + pos, 128)
                                ln = min(48 - pos, 128 - dp, 128 - sp)
                                nc.sync.dma_start(
                                    qshs[dg][dp:dp + ln, :],
                                    psbs[sg][sp:sp + ln, :])
                                pos += ln
                        rinv4 = tmp_pool.tile([4, 512], f32r, tag="rinv4",
                                              name="rinv4")

                        def a_set(sqs=sqs, rinv4=rinv4):
                            ssq = ssq_pool.tile([4, 512], f32, tag="ssq",
                                                name="ssq")
                            for g in range(3):
                                nc.tensor.matmul(
                                    ssq[:], sels_t[g][:], sqs[g][:],
                                    start=(g == 0), stop=(g == 2))
                            rms4 = tmpb_pool.tile([4, 512], f32, tag="rms4",
                                                  name="rms4")
                            nc.scalar.activation(rms4[:], ssq[:], AF.Sqrt,
                                                 bias=eps4_t[:],
                                                 scale=1.0 / HD)
                            nc.vector.reciprocal(rinv4[:], rms4[:])
                        thunkq.append((stepc[0], a_set))

                        for g in range(3):
                            def b_g(g=g, s=s, hsl=hsl, psb=psbs[g],
                                    qsh=qshs[g], rinv4=rinv4):
                                bc = bc_pool.tile([128, 512], f32, tag="bc",
                                                  name="bc")
                                nc.tensor.matmul(bc[:], selst_t[g][:],
                                                 rinv4[:],
                                                 start=True, stop=True)
                                bcs = tmpb_pool.tile([128, 512], f32,
                                                     tag="bcs", name="bcs")
                                nc.scalar.copy(bcs[:], bc[:])
                                sl = tmpb_pool.tile([128, 512], bf16,
                                                    tag="slt", name="sl")
                                nc.vector.tensor_mul(sl[:], psb[:], bcs[:])
                                qsn = tmpb_pool.tile([128, 512], bf16,
                                                     tag="qsn", name="qsn")
                                nc.vector.tensor_mul(qsn[:], qsh[:], bcs[:])
                                tm1 = tmpb_pool.tile([128, 512], bf16,
                                                     tag="tm1", name="tm1")
                                nc.vector.tensor_mul(tm1[:], sl[:],
                                                     tabaq_t[g][:, hsl])
                                tm2 = tmpb_pool.tile([128, 512], bf16,
                                                     tag="tm2", name="tm2")
                                nc.vector.tensor_mul(tm2[:], qsn[:],
                                                     tabbq_t[g][:, hsl])
                                qp = tmpb_pool.tile([128, 512], bf16,
                                                    tag="slt", name="qp")
                                nc.vector.tensor_add(qp[:], tm1[:], tm2[:])
                                # repack into per-head padded qt layout
                                base = 128 * g
                                for (a, ap, h4, odd) in RUNS48:
                                    lo = max(a, base)
                                    hi = min(a + 48, base + 128)
                                    if lo >= hi:
                                        continue
                                    o0 = lo - a
                                    d0 = (64 + o0) if odd else o0
                                    nc.sync.dma_start(
                                        qt[4 * s + h4][d0:d0 + hi - lo, hsl],
                                        qp[lo - base:hi - base, :])
                            thunkq.append((stepc[0], b_g))

                # ---- v phase last: no norm chains -- the trailing q chains
                # drain on Act/DVE behind these accums, so the phase-2 scope
                # switch sees no backlog
                wv_t = []
                for i in range(KVH):
                    t = wq_pool.tile([128, KT, 128], bf16, tag="wq",
                                     name="wv")
                    nc.scalar.dma_start(t[:, :, 0:HD], wvt_d[i])
                    wv_t.append(t)
                for hf in range(2):
                    hsl = slice(hf * 512, (hf + 1) * 512)
                    for i in range(KVH):
                        vp = psq_pool.tile([128, 512], f32, tag="psq")
                        vps = vp[0:HD, :]
                        for kt in range(KT):
                            nc.tensor.matmul(
                                vps, wv_t[i][:, kt, 0:HD],
                                xt_t[hf][:, kt, :],
                                start=(kt == 0), stop=(kt == KT - 1))
                        vt = tab_pool.tile([HD, 512], f32, tag="vt",
                                           name="vt")
                        nc.scalar.copy(vt[:], vp[0:HD, :])
                        step_chain_b()
                        step_chain()
                        step_thunk()
                        step_thunk()
                        for c in range(4):
                            tp = vtr_pool.tile([128, HD], f32, tag="tp")
                            nc.tensor.transpose(
                                tp[:], vt[:, c * 128:(c + 1) * 128],
                                ident[0:HD, 0:HD])
                            itok = hf * 4 + c
                            nc.scalar.copy(vext[i][:, itok, 0:HD], tp[:])
                        step_thunk()
                flush_chains()
                while thunkq:
                    thunkq.pop(0)[1]()

            # -------- Phase 2+3: attention with o_proj interleaved ----------
            # q-chunks outer, heads inner: once all 8 heads finish chunk qc,
            # tokens [512qc, 512qc+512) are fully attended and their o_proj
            # row-tiles are emitted as PE gap-fillers while the Act engine
            # works through the next chunk group's exps.
            with ExitStack() as s2:
                pt_pool = s2.enter_context(tc.tile_pool(name="pt", bufs=5))
                tmp2_pool = s2.enter_context(tc.tile_pool(name="tmp2", bufs=3))
                wo_pool = s2.enter_context(tc.tile_pool(name="wo3", bufs=1))
                ob_pool = s2.enter_context(tc.tile_pool(name="ob", bufs=4))
                sc_pool = s2.enter_context(
                    tc.tile_pool(name="sc", bufs=3, space="PSUM"))
                po_pool = s2.enter_context(
                    tc.tile_pool(name="po", bufs=2, space="PSUM"))
                bc2_pool = s2.enter_context(
                    tc.tile_pool(name="bc2", bufs=1, space="PSUM"))
                ps3_pool = s2.enter_context(
                    tc.tile_pool(name="ps3", bufs=2, space="PSUM"))

                wo_t = [wo_pool.tile([128, NG3, 512], bf16, tag=f"wo3{j}",
                                     name=f"wo3{j}") for j in range(NJ)]

                # flattened (qc, h, kt2) item list; sc/exp/select emitted
                # D_PIPE items ahead of the corresponding P@V matmul.
                items = []
                for qc in range(NQC):
                    for h in range(QH):
                        for kt2 in range(NKP * qc + NKP):
                            items.append((h, qc, kt2))

                po_t = {}
                pts = {}
                norm_q = []
                ph3_q = []
                ob_t = {}

                def emit_sc(idx):
                    h, qc, kt2 = items[idx]
                    g2 = h // (QH // KVH)
                    jsl = slice(qc * QCS, (qc + 1) * QCS)
                    sc = sc_pool.tile([128, QCS], f32, tag="sc")
                    nc.tensor.matmul(
                        sc[:], ktl[g2][:, kt2 * 128:(kt2 + 1) * 128],
                        qt[h][:, jsl], start=True, stop=True)
                    pt = pt_pool.tile([128, QCS], bf16, tag="pt")
                    nc.scalar.activation(pt[:], sc[:], AF.Exp, scale=SCALE)
                    if kt2 >= NKP * qc:
                        nc.gpsimd.affine_select(
                            pt[:], pt[:], pattern=[[1, QCS]],
                            compare_op=mybir.AluOpType.is_ge,
                            fill=0.0,
                            base=qc * QCS - kt2 * 128,
                            channel_multiplier=-1)
                    pts[idx] = pt

                def emit_pv(idx):
                    h, qc, kt2 = items[idx]
                    g2 = h // (QH // KVH)
                    if kt2 == 0:
                        po_t[(h, qc)] = po_pool.tile([HD + 1, QCS], f32,
                                                     tag="po", name="po")
                    po = po_t[(h, qc)]
                    nc.tensor.matmul(
                        po[:], vext[g2][:, kt2, :], pts.pop(idx)[:],
                        start=(kt2 == 0),
                        stop=(kt2 == NKP * qc + NKP - 1))
                    if kt2 == NKP * qc + NKP - 1:
                        rinv2 = tmp2_pool.tile([1, QCS], f32r, tag="rinv2")
                        nc.vector.reciprocal(rinv2[:], po[HD:HD + 1, :])
                        norm_q.append((h, qc, rinv2))

                def emit_norm():
                    if not norm_q:
                        return
                    h, qc, rinv2 = norm_q.pop(0)
                    jsl = slice(qc * QCS, (qc + 1) * QCS)
                    po = po_t.pop((h, qc))
                    bc2 = bc2_pool.tile([HD, QCS], f32, tag="bc2")
                    nc.tensor.matmul(bc2[:], ones96[:], rinv2[:],
                                     start=True, stop=True)
                    bc2s = tmp2_pool.tile([HD, QCS], f32, tag="bc2s")
                    nc.scalar.copy(bc2s[:], bc2[:])
                    at_tmp = tmp2_pool.tile([HD, QCS], bf16, tag="at_tmp")
                    nc.vector.tensor_mul(at_tmp[:], po[0:HD, :], bc2s[:])
                    # repack rows 96h..96h+96 into the 6x128 contraction tiles
                    r0 = h * HD
                    while r0 < (h + 1) * HD:
                        t, p = divmod(r0, 128)
                        rows = min(128 - p, (h + 1) * HD - r0)
                        nc.sync.dma_start(
                            atp[t][p:p + rows, jsl],
                            at_tmp[r0 - h * HD:r0 - h * HD + rows, :])
                        r0 += rows
                    if h == QH - 1:
                        for j3 in range(NJ):
                            for i in range(NKP * qc, NKP * qc + NKP):
                                ph3_q.append((i, j3))

                def emit_ph3():
                    if not ph3_q:
                        return
                    i, j3 = ph3_q.pop(0)
                    isl = slice(i * 128, (i + 1) * 128)
                    ps3 = ps3_pool.tile([128, 512], f32, tag="ps3")
                    for t in range(NG3):
                        nc.tensor.matmul(
                            ps3[:], atp[t][:, isl], wo_t[j3][:, t, :],
                            start=(t == 0), stop=(t == NG3 - 1))
                    jsl = slice(j3 * 512, (j3 + 1) * 512)
                    ob = ob_pool.tile([128, 512], f32, tag="ob", name="ob")
                    nc.vector.tensor_copy(ob[:], ps3[:])
                    nc.sync.dma_start(out_d[isl, jsl], ob[:])

                for idx in range(len(items)):
                    if idx % 8 == 0 and idx // 8 < NJ:
                        j = idx // 8
                        nc.scalar.dma_start(wo_t[j][:], wot_d[:, j])
                    emit_sc(idx)
                    if idx >= D_PIPE2:
                        emit_pv(idx - D_PIPE2)
                        emit_norm()
                    if idx % 3 == 2 and len(ph3_q) > 2:
                        emit_ph3()
                for idx in range(len(items) - D_PIPE2, len(items)):
                    emit_pv(idx)
                    emit_norm()
                while norm_q:
                    emit_norm()
                while ph3_q:
                    emit_ph3()

    nc.compile()
    return nc


def get_nc():
    if "nc" not in _BUILD_CACHE:
        _BUILD_CACHE["nc"] = _build_nc()
    return _BUILD_CACHE["nc"]


def _permpad_rows(w96):
    """(96, N) head rows -> (128, N): evens at 0:48, odds at 64:112, pad 0."""
    out = np.zeros((128, w96.shape[1]), np.float32)
    out[0:48] = w96[0::2]
    out[64:112] = w96[1::2]
    return out


def _lhsT_tiles(wT, m):
    """(D, m) -> (128, KT, m) lhsT tile layout (contraction on partitions)."""
    return np.ascontiguousarray(
        wT.reshape(KT, 128, m).transpose(1, 0, 2)).astype(np.float32)


def prepare_in_maps(x, wq, wk, wv, wo, q_norm_w, k_norm_w, cos, sin):
    import ml_dtypes
    bf16 = ml_dtypes.bfloat16

    x = np.asarray(x, np.float32)
    wq = np.asarray(wq, np.float32)
    wk = np.asarray(wk, np.float32)
    wv = np.asarray(wv, np.float32)
    wo = np.asarray(wo, np.float32)
    cos = np.asarray(cos, np.float32)
    sin = np.asarray(sin, np.float32)
    q_norm_w = np.asarray(q_norm_w, np.float32)
    k_norm_w = np.asarray(k_norm_w, np.float32)

    def _fold_tabs(nw):
        # P multiplies the in-place operand sl, Q the partner-swapped qsn:
        #   evens rows: out = a*we*ce - b*wo*se -> P=we*ce, Q=-wo*se
        #   odds rows:  out = b*wo*co + a*we*so -> P=wo*co, Q=+we*so
        nwe = nw[0::2][:, None]
        nwo = nw[1::2][:, None]
        ta = np.zeros((128, T), np.float32)
        tb = np.zeros((128, T), np.float32)
        ta[0:48] = nwe * cos[:, 0::2].T
        ta[64:112] = nwo * cos[:, 1::2].T
        tb[0:48] = -nwo * sin[:, 0::2].T
        tb[64:112] = nwe * sin[:, 1::2].T
        return ta, tb

    tabak, tabbk = _fold_tabs(k_norm_w)

    # packed-layout q tables/selectors: set-local row R (of 384) -> head
    # h4 = R//96, local l = R%96; l<48 = even component e=l, else odd o=l-48
    tabaq = np.zeros((3, 128, T), np.float32)
    tabbq = np.zeros((3, 128, T), np.float32)
    sels = np.zeros((3, 128, 4), np.float32)
    for p in range(3):
        for r in range(128):
            R = 128 * p + r
            h4, l = divmod(R, 96)
            sels[p, r, h4] = 1.0
            if l < 48:
                e = l
                tabaq[p, r] = q_norm_w[2 * e] * cos[:, 2 * e]
                tabbq[p, r] = -q_norm_w[2 * e + 1] * sin[:, 2 * e]
            else:
                o = l - 48
                tabaq[p, r] = q_norm_w[2 * o + 1] * cos[:, 2 * o + 1]
                tabbq[p, r] = q_norm_w[2 * o] * sin[:, 2 * o + 1]
    selst = np.ascontiguousarray(sels.transpose(0, 2, 1))

    xts = []
    for b in range(B):
        xT = np.ascontiguousarray(x[b].T)  # (D, T)
        xts.append(np.ascontiguousarray(
            xT.reshape(KT, 128, T).transpose(1, 0, 2)).astype(bf16))

    in_maps = []
    for c in range(NCORES):
        b, g = divmod(c, G)
        # packed q weights: 8 heads x 96 rows (evens then odds per head)
        # -> 6 full 128-row groups
        wqp = np.zeros((QH * HD, D), np.float32)
        for i in range(QH):
            hw_ = wq[(g * QH + i) * HD:(g * QH + i + 1) * HD]
            wqp[96 * i:96 * i + 48] = hw_[0::2]
            wqp[96 * i + 48:96 * i + 96] = hw_[1::2]
        wqt = np.stack([
            _lhsT_tiles(np.ascontiguousarray(
                wqp[128 * j:128 * (j + 1)].T), 128)
            for j in range(6)]).astype(bf16)
        wkt = np.stack([
            _lhsT_tiles(_permpad_rows(
                wk[(g * KVH + i) * HD:(g * KVH + i + 1) * HD]).T, 128)
            for i in range(KVH)]).astype(bf16)
        wvt = np.stack([
            _lhsT_tiles(np.ascontiguousarray(
                wv[(g * KVH + i) * HD:(g * KVH + i + 1) * HD].T), HD)
            for i in range(KVH)]).astype(bf16)
        # packed o_proj weights, partition-major: (128, NJ, NG3, 512)
        woT = np.ascontiguousarray(
            wo[:, g * QH * HD:(g + 1) * QH * HD].T)  # (768, 3072)
        wot = np.ascontiguousarray(
            woT.reshape(NG3, 128, NJ, 512).transpose(1, 2, 0, 3)).astype(bf16)
        in_maps.append({
            "xt": xts[b], "wqt": wqt, "wkt": wkt, "wvt": wvt, "wot": wot,
            "tabaq": tabaq.astype(bf16), "tabbq": tabbq.astype(bf16),
            "tabak": tabak.astype(bf16), "tabbk": tabbk.astype(bf16),
            "sels": sels, "selst": selst,

            "o128": np.ones((128, 1), np.float32),
            "o128T": np.ones((1, 128), np.float32),
            "o96": np.ones((1, HD), np.float32),
            "ocol": np.ones((128, KTOK), bf16),
        })
    return in_maps


def kernel(**inputs):
    from concourse import bass_utils

    nc = get_nc()
    in_maps = prepare_in_maps(
        inputs["x"], inputs["wq"], inputs["wk"], inputs["wv"], inputs["wo"],
        inputs["q_norm_w"], inputs["k_norm_w"], inputs["cos"], inputs["sin"])
    trace = bool(int(os.environ.get("BASS_KERNEL_TRACE", "0")))
    res = bass_utils.run_bass_kernel_spmd(
        nc, in_maps, core_ids=list(range(NCORES)), trace=trace)
    _BUILD_CACHE["last_result"] = res
    partials = [np.asarray(r["out"]) for r in res.results]
    out = np.empty((B, T, D), np.float32)
    for b in range(B):
        out[b] = np.sum(np.stack(partials[b * G:(b + 1) * G]), axis=0,
                        dtype=np.float64).astype(np.float32)
    return out


# revision 65
# speedup vs baseline: 1.0279x; 1.0012x over previous
"""Grouped-Query Attention block (RMSNorm + RoPE + causal GQA + o_proj) on 8 trn2 NeuronCores.

Sharding: data-parallel over batch (2) x tensor-parallel over kv-head groups (4).
Core c = b*4 + g handles batch b, kv heads {2g, 2g+1}, q heads {8g..8g+7}.
Each core computes a partial o_proj output (T, D) over its 768 head-dims;
host sums the 4 group partials per batch.

v3 structure:
  * bf16 inputs/weights/activations; accumulation + norm/softmax stats fp32.
  * All matmuls 512-wide moving dim: PE sequencer dispatch (~170ns/matmul)
    stays under engine execution (~213ns) -> engine-bound, not dispatch-bound.
  * DMAs batched (x: 4 single-DMA token-quarters, o_proj weights: 1 DMA,
    output: 1 DMA per 128-token row) and spread across the SP and Pool
    dispatch queues so no sequencer saturates.
  * x + all projection weights SBUF-resident; first two x quarters land on
    different queues in parallel -> compute starts ~5us in.
  * RMSNorm / softmax broadcast matmuls issue-deferred behind the next unit's
    accumulation matmuls so PE never stalls on the Act/DVE scalar chains.
  * o_proj contraction packed: 8 heads x 96 rows repacked (SBUF DMA) into
    6 full 128-row groups -> 6 instead of 8 matmuls per output tile; o_proj
    row-tiles for tokens 0:512 interleave into the attention stream as PE
    gap fillers while Act churns exps.
"""

import os
import sys

import numpy as np

sys.path.insert(0, "/opt/trn_rl_repo")

B, T, D = 2, 1024, 3072
NH, NKV, HD = 32, 8, 96
G = 4                 # tensor-parallel groups
QH = NH // G          # q heads per core (8)
KVH = NKV // G        # kv heads per core (2)
NCORES = 8
EPS = 1e-6
SCALE = 1.0 / float(np.sqrt(HD))
KT = D // 128         # 24 contraction tiles over d_model
KTOK = T // 128       # 8 token tiles
QCS = 256             # q chunk in phase 2
NQC = T // QCS        # 4
NKP = QCS // 128      # k tiles per q chunk
D_PIPE2 = 3           # phase-2 score matmuls issued ahead of P@V
NJ = D // 512         # 6 output column chunks
NG3 = QH * HD // 128  # 6 packed o_proj contraction groups
D_PIPE = 2            # phase-2 score matmuls issued ahead of P@V

_BUILD_CACHE = {}


def _build_nc():
    from contextlib import ExitStack
    from concourse import bacc, tile, mybir
    from concourse.masks import make_identity

    f32 = mybir.dt.float32
    f32r = mybir.dt.float32r
    bf16 = mybir.dt.bfloat16
    AF = mybir.ActivationFunctionType

    nc = bacc.Bacc("TRN2", target_bir_lowering=False, debug=False,
                   num_devices=NCORES)

    xt_d = nc.dram_tensor("xt", (128, KT, T), bf16, kind="ExternalInput").ap()
    wqt_d = nc.dram_tensor("wqt", (6, 128, KT, 128), bf16, kind="ExternalInput").ap()
    wkt_d = nc.dram_tensor("wkt", (KVH, 128, KT, 128), bf16, kind="ExternalInput").ap()
    wvt_d = nc.dram_tensor("wvt", (KVH, 128, KT, HD), bf16, kind="ExternalInput").ap()
    wot_d = nc.dram_tensor("wot", (128, NJ, NG3, 512), bf16, kind="ExternalInput").ap()
    tabaq_d = nc.dram_tensor("tabaq", (3, 128, T), bf16, kind="ExternalInput").ap()
    tabbq_d = nc.dram_tensor("tabbq", (3, 128, T), bf16, kind="ExternalInput").ap()
    sels_d = nc.dram_tensor("sels", (3, 128, 4), f32r, kind="ExternalInput").ap()
    selst_d = nc.dram_tensor("selst", (3, 4, 128), f32r, kind="ExternalInput").ap()
    tabak_d = nc.dram_tensor("tabak", (128, T), bf16, kind="ExternalInput").ap()
    tabbk_d = nc.dram_tensor("tabbk", (128, T), bf16, kind="ExternalInput").ap()
    o128_d = nc.dram_tensor("o128", (128, 1), f32r, kind="ExternalInput").ap()
    o96_d = nc.dram_tensor("o96", (1, HD), f32r, kind="ExternalInput").ap()
    o128T_d = nc.dram_tensor("o128T", (1, 128), f32r, kind="ExternalInput").ap()
    ocol_d = nc.dram_tensor("ocol", (128, KTOK), bf16, kind="ExternalInput").ap()
    out_d = nc.dram_tensor("out", (T, D), f32, kind="ExternalOutput").ap()

    with tile.TileContext(nc) as tc:
        with nc.allow_low_precision(reason="bf16 compute, fp32 accumulate"), \
             ExitStack() as ctx:
            const = ctx.enter_context(tc.tile_pool(name="const", bufs=1))
            p_qkv = ctx.enter_context(tc.tile_pool(name="p_qkv", bufs=1))

            eps_t = const.tile([1, 1], f32, tag="eps")
            nc.vector.memset(eps_t[:], EPS)
            warm_t = const.tile([1, 1], f32, tag="warm")
            nc.scalar.activation(warm_t[:], eps_t[:], AF.Exp, scale=1.0)
            eps4_t = const.tile([4, 1], f32, tag="eps4")
            nc.vector.memset(eps4_t[:], EPS)
            ones128 = const.tile([128, 1], f32r, tag="ones128")
            ones96 = const.tile([1, HD], f32r, tag="ones96")
            ones128T = const.tile([1, 128], f32r, tag="ones128T")
            ident = const.tile([128, 128], f32, tag="ident")

            qt = [p_qkv.tile([128, T], bf16, tag=f"qt{h}", name=f"qt{h}")
                  for h in range(QH)]
            for h in range(QH):
                nc.vector.memset(qt[h][32:64, :], 0.0)
                nc.vector.memset(qt[h][96:128, :], 0.0)
            ktl = [p_qkv.tile([128, T], bf16, tag=f"kt{g2}", name=f"kt{g2}")
                   for g2 in range(KVH)]
            vext = [p_qkv.tile([128, KTOK, HD + 1], bf16, tag=f"vx{g2}",
                               name=f"vx{g2}") for g2 in range(KVH)]
            atp = [p_qkv.tile([128, T], bf16, tag=f"atp{t}", name=f"atp{t}")
                   for t in range(NG3)]

            # ---------------- Phase 1: projections + RMSNorm + RoPE ---------
            with ExitStack() as s1:
                xt_pool = s1.enter_context(tc.tile_pool(name="xt", bufs=1))
                wkv_pool = s1.enter_context(tc.tile_pool(name="wkv", bufs=1))
                wq_pool = s1.enter_context(tc.tile_pool(name="wq", bufs=3))
                tab_pool = s1.enter_context(tc.tile_pool(name="tabs", bufs=1))
                tmp_pool = s1.enter_context(tc.tile_pool(name="tmp1", bufs=3))
                tmp4_pool = s1.enter_context(tc.tile_pool(name="tmp4", bufs=4))
                tmpb_pool = s1.enter_context(tc.tile_pool(name="tmpb", bufs=2))
                psk_pool = s1.enter_context(
                    tc.tile_pool(name="psk", bufs=1, space="PSUM"))
                psq_pool = s1.enter_context(
                    tc.tile_pool(name="psq", bufs=3, space="PSUM"))
                vtr_pool = s1.enter_context(
                    tc.tile_pool(name="vtr", bufs=1, space="PSUM"))
                ssq_pool = s1.enter_context(
                    tc.tile_pool(name="ssq", bufs=1, space="PSUM"))
                bc_pool = s1.enter_context(
                    tc.tile_pool(name="bc1", bufs=1, space="PSUM"))

                xt_t = [xt_pool.tile([128, KT, 512], bf16, tag=f"xh{hf}",
                                     name=f"xh{hf}") for hf in range(2)]
                wk_t = [wkv_pool.tile([128, KT, 128], bf16, tag=f"wk{i}",
                                      name=f"wk{i}") for i in range(KVH)]


                tabaq_t = [tab_pool.tile([128, T], bf16, tag=f"tabaq{p}",
                                          name=f"tabaq{p}") for p in range(3)]
                tabbq_t = [tab_pool.tile([128, T], bf16, tag=f"tabbq{p}",
                                          name=f"tabbq{p}") for p in range(3)]
                sels_t = [tab_pool.tile([128, 4], f32r, tag=f"sels{p}",
                                        name=f"sels{p}") for p in range(3)]
                selst_t = [tab_pool.tile([4, 128], f32r, tag=f"selst{p}",
                                         name=f"selst{p}") for p in range(3)]
                tabak_t = tab_pool.tile([128, T], bf16, tag="tabak")
                tabbk_t = tab_pool.tile([128, T], bf16, tag="tabbk")
                # static load order; wk races xh0 on the two HWDGE queues
                nc.scalar.dma_start(wk_t[0][:], wkt_d[0])
                nc.sync.dma_start(xt_t[0][:], xt_d[:, :, 0:512])
                nc.scalar.dma_start(wk_t[1][:], wkt_d[1])
                nc.sync.dma_start(xt_t[1][:], xt_d[:, :, 512:1024])
                nc.sync.dma_start(ones128[:], o128_d[:])
                nc.sync.dma_start(ones96[:], o96_d[:])
                nc.sync.dma_start(ones128T[:], o128T_d[:])
                make_identity(nc, ident[:])
                nc.scalar.dma_start(tabak_t[:], tabak_d[:])
                nc.scalar.dma_start(tabbk_t[:], tabbk_d[:])
                wq_tiles = {}

                def load_wq(j):
                    if j >= 6:
                        return
                    t = wq_pool.tile([128, KT, 128], bf16, tag="wq",
                                     name="wq")
                    nc.scalar.dma_start(t[:], wqt_d[j])
                    wq_tiles[j] = t

                for j in range(3):
                    load_wq(j)
                for p in range(3):
                    nc.scalar.dma_start(tabaq_t[p][:], tabaq_d[p])
                    nc.scalar.dma_start(tabbq_t[p][:], tabbq_d[p])
                    nc.sync.dma_start(sels_t[p][:], sels_d[p])
                    nc.sync.dma_start(selst_t[p][:], selst_d[p])
                for g2 in range(KVH):
                    nc.sync.dma_start(vext[g2][:, :, HD:HD + 1], ocol_d[:])

                def load_rest():
                    pass

                # --- norm + RoPE chain, split into two issue stages ---------
                def stage_a(ps, sq, rinv):
                    """ssq matmul (PE) + sqrt (Act) + recip (DVE)."""
                    ssq = ssq_pool.tile([4, 512], f32, tag="ssq")
                    nc.tensor.matmul(ssq[0:1, :], ones128[:], sq[:],
                                     start=True, stop=True)
                    rms = tmpb_pool.tile([1, 512], f32, tag="rms")
                    nc.scalar.activation(rms[:], ssq[0:1, :], AF.Sqrt,
                                         bias=eps_t[:], scale=1.0 / HD)
                    nc.vector.reciprocal(rinv[:], rms[:])

                def stage_b(ps, qsh, rinv, dst, hsl, ta, tb, ve):
                    """rinv broadcast (PE) + normalize + RoPE (Act/DVE).
                    ps/qsh are SBUF copies of the pre-norm projection and its
                    RoPE partner-row swap (DMA'd off the critical DVE chain).
                    The RMSNorm weights are folded into the RoPE tables
                    host-side, so one plain 1/rms broadcast serves both.
                    """
                    bc = bc_pool.tile([128, 512], f32, tag="bc")
                    nc.tensor.matmul(bc[:], ones128T[:], rinv[:],
                                     start=True, stop=True)
                    bcs = tmpb_pool.tile([128, 512], f32, tag="bcs")
                    nc.scalar.copy(bcs[:], bc[:])
                    sl = tmpb_pool.tile([128, 512], bf16, tag="slt")
                    ve.tensor_mul(sl[:], ps[:], bcs[:])
                    qsn = tmpb_pool.tile([128, 512], bf16, tag="qsn")
                    ve.tensor_mul(qsn[:], qsh[:], bcs[:])
                    # dst = sl*P + qsn*Q, P/Q full-height tables with the
                    # norm weights and the rotation signs folded in host-side
                    tm1 = tmpb_pool.tile([128, 512], bf16, tag="tm1")
                    ve.tensor_mul(tm1[:], sl[:], ta[:, hsl])
                    tm2 = tmpb_pool.tile([128, 512], bf16, tag="tm2")
                    ve.tensor_mul(tm2[:], qsn[:], tb[:, hsl])
                    ve.tensor_add(dst[:, hsl], tm1[:], tm2[:])

                # Deferred two-stage pipeline over accumulation units: the
                # PE ops of stage A/B for unit u are emitted after the accum
                # matmuls of units u+1 / u+2, so PE never waits on Act/DVE.
                chainq = []   # entries: [stage, a_thunk, b_thunk]

                def step_chain_b():
                    if chainq and chainq[0][0] == 1:
                        e = chainq.pop(0)
                        e[2]()

                def step_chain():
                    for e in chainq:
                        if e[0] == 0:
                            e[0] = 1
                            e[1]()
                            return

                def flush_chains():
                    while chainq:
                        step_chain_b()
                        step_chain()
                        if chainq and chainq[0][0] == 1:
                            e = chainq.pop(0)
                            e[2]()

                chain_no = [0]

                def make_chain(ps, dst, hsl, ta, tb):
                    # alternate DVE/Pool for RoPE; last chains stay on DVE so
                    # the Pool queue is clear for phase-2 affine_selects
                    ve = (nc.gpsimd if (chain_no[0] % 2 == 1
                                        and chain_no[0] < 14) else nc.vector)
                    chain_no[0] += 1
                    # SBUF copy frees the PSUM tile and lets the RoPE partner
                    # swap run as a DMA concurrent with the norm chain; the
                    # square runs on DVE from the copy (keeps Act to
                    # copies+sqrt only -> no activation-table churn)
                    psb = tmp_pool.tile([128, 512], f32, tag="psb")
                    nc.scalar.copy(psb[:], ps[:])
                    sq = tmp_pool.tile([128, 512], f32r, tag="sq")
                    nc.vector.tensor_mul(sq[:], psb[:], psb[:])
                    qsh = tmp_pool.tile([128, 512], f32, tag="qsh")
                    nc.sync.dma_start(qsh[0:64, :], psb[64:128, :])
                    nc.sync.dma_start(qsh[64:128, :], psb[0:64, :])
                    rinv = tmp_pool.tile([1, 512], f32r, tag="rinv")
                    push = [0, lambda: stage_a(psb, sq, rinv),
                            lambda: stage_b(psb, qsh, rinv, dst, hsl,
                                            ta, tb, ve)]
                    chainq.append(push)

                # ---- k phase (first: x-paced; half 0 in token-quarters) ----
                psk = [psk_pool.tile([128, 512], f32, tag=f"psk{i}",
                                     name=f"psk{i}") for i in range(KVH)]
                for hf in range(2):
                    hsl = slice(hf * 512, (hf + 1) * 512)
                    for i in range(KVH):
                        for kt in range(KT):
                            nc.tensor.matmul(
                                psk[i][:], wk_t[i][:, kt, :],
                                xt_t[hf][:, kt, :],
                                start=(kt == 0), stop=(kt == KT - 1))
                        if hf == 0 and i == 0:
                            load_rest()
                        step_chain_b()
                        step_chain()
                        make_chain(psk[i], ktl[i], hsl, tabak_t, tabbk_t)

                # ---- q phase: heads packed 8x96 -> 6x128 ------------------
                # Two sets of 4 heads, 3 full 128-row groups each: 288 accum
                # matmuls instead of 384.  Per-head RMSNorm sums via selector
                # matmuls; RoPE in packed layout with per-group-pattern P/Q
                # tables; results DMA-repacked into the per-head padded qt
                # tiles so phase 2 is unchanged (zero k-pads keep the unset
                # qt pad rows harmless in the QK contraction).
                thunkq = []   # (append_step, fn) run one per step, lag >= 1
                stepc = [0]

                def step_thunk():
                    stepc[0] += 1
                    if thunkq and thunkq[0][0] < stepc[0]:
                        thunkq.pop(0)[1]()

                RUNS48 = []   # (start, partner_start, head4, is_odd)
                for h4 in range(4):
                    RUNS48.append((96 * h4, 96 * h4 + 48, h4, 0))
                    RUNS48.append((96 * h4 + 48, 96 * h4, h4, 1))

                for s in range(2):
                    for hf in range(2):
                        hsl = slice(hf * 512, (hf + 1) * 512)
                        sqs, psbs, qshs = [], [], []
                        for g in range(3):
                            ps = psq_pool.tile([128, 512], f32, tag="psq",
                                               name="ps")
                            wqg = wq_tiles[3 * s + g] if hf == 0 else \
                                wq_tiles[3 * s + g]
                            for kt in range(KT):
                                nc.tensor.matmul(
                                    ps[:], wqg[:, kt, :],
                                    xt_t[hf][:, kt, :],
                                    start=(kt == 0), stop=(kt == KT - 1))
                            if hf == 1:
                                load_wq(3 * s + g + 3)
                            step_chain_b()
                            step_chain()
                            step_thunk()
                            psb = tmp4_pool.tile([128, 512], f32, tag="psb",
                                                 name="psb")
                            nc.scalar.copy(psb[:], ps[:])
                            sq = tmp_pool.tile([128, 512], f32r, tag="sq",
                                               name="sq")
                            nc.vector.tensor_mul(sq[:], psb[:], psb[:])
                            sqs.append(sq)
                            psbs.append(psb)
                        for g in range(3):
                            qshs.append(tmp4_pool.tile(
                                [128, 512], f32, tag="qsh", name="qsh"))
                        # partner-row swap, cross-group pieces
                        for (a, ap, h4, odd) in RUNS48:
                            pos = 0
                            while pos < 48:
                                dg, dp = divmod(a + pos, 128)
                                sg, sp = divmod(ap + pos, 128)
                                ln = min(48 - pos, 128 - dp, 128 - sp)
                                nc.sync.dma_start(
                                    qshs[dg][dp:dp + ln, :],
                                    psbs[sg][sp:sp + ln, :])
                                pos += ln
                        rinv4 = tmp_pool.tile([4, 512], f32r, tag="rinv4",
                                              name="rinv4")

                        def a_set(sqs=sqs, rinv4=rinv4):
                            ssq = ssq_pool.tile([4, 512], f32, tag="ssq",
                                                name="ssq")
                            for g in range(3):
                                nc.tensor.matmul(
                                    ssq[:], sels_t[g][:], sqs[g][:],
                                    start=(g == 0), stop=(g == 2))
                            rms4 = tmpb_pool.tile([4, 512], f32, tag="rms4",
                                                  name="rms4")
                            nc.scalar.activation(rms4[:], ssq[:], AF.Sqrt,
                                                 bias=eps4_t[:],
                                                 scale=1.0 / HD)
                            nc.vector.reciprocal(rinv4[:], rms4[:])
                        thunkq.append((stepc[0], a_set))

                        for g in range(3):
                            def b_g(g=g, s=s, hsl=hsl, psb=psbs[g],
                                    qsh=qshs[g], rinv4=rinv4):
                                bc = bc_pool.tile([128, 512], f32, tag="bc",
                                                  name="bc")
                                nc.tensor.matmul(bc[:], selst_t[g][:],
                                                 rinv4[:],
                                                 start=True, stop=True)
                                bcs = tmpb_pool.tile([128, 512], f32,
                                                     tag="bcs", name="bcs")
                                nc.scalar.copy(bcs[:], bc[:])
                                sl = tmpb_pool.tile([128, 512], bf16,
                                                    tag="slt", name="sl")
                                nc.vector.tensor_mul(sl[:], psb[:], bcs[:])
                                qsn = tmpb_pool.tile([128, 512], bf16,
                                                     tag="qsn", name="qsn")
                                nc.vector.tensor_mul(qsn[:], qsh[:], bcs[:])
                                tm1 = tmpb_pool.tile([128, 512], bf16,
                                                     tag="tm1", name="tm1")
                                nc.vector.tensor_mul(tm1[:], sl[:],
                                                     tabaq_t[g][:, hsl])
                                tm2 = tmpb_pool.tile([128, 512], bf16,
                                                     tag="tm2", name="tm2")
                                nc.vector.tensor_mul(tm2[:], qsn[:],
                                                     tabbq_t[g][:, hsl])
                                qp = tmpb_pool.tile([128, 512], bf16,
                                                    tag="slt", name="qp")
                                nc.vector.tensor_add(qp[:], tm1[:], tm2[:])
                                # repack into per-head padded qt layout
                                base = 128 * g
                                for (a, ap, h4, odd) in RUNS48:
                                    lo = max(a, base)
                                    hi = min(a + 48, base + 128)
                                    if lo >= hi:
                                        continue
                                    o0 = lo - a
                                    d0 = (64 + o0) if odd else o0
                                    nc.sync.dma_start(
                                        qt[4 * s + h4][d0:d0 + hi - lo, hsl],
                                        qp[lo - base:hi - base, :])
                            thunkq.append((stepc[0], b_g))

                # ---- v phase last: no norm chains -- the trailing q chains
                # drain on Act/DVE behind these accums, so the phase-2 scope
                # switch sees no backlog
                wv_t = []
                for i in range(KVH):
                    t = wq_pool.tile([128, KT, 128], bf16, tag="wq",
                                     name="wv")
                    nc.scalar.dma_start(t[:, :, 0:HD], wvt_d[i])
                    wv_t.append(t)
                for hf in range(2):
                    hsl = slice(hf * 512, (hf + 1) * 512)
                    for i in range(KVH):
                        vp = psq_pool.tile([128, 512], f32, tag="psq")
                        vps = vp[0:HD, :]
                        for kt in range(KT):
                            nc.tensor.matmul(
                                vps, wv_t[i][:, kt, 0:HD],
                                xt_t[hf][:, kt, :],
                                start=(kt == 0), stop=(kt == KT - 1))
                        vt = tab_pool.tile([HD, 512], f32, tag="vt",
                                           name="vt")
                        nc.scalar.copy(vt[:], vp[0:HD, :])
                        step_chain_b()
                        step_chain()
                        step_thunk()
                        step_thunk()
                        for c in range(4):
                            tp = vtr_pool.tile([128, HD], f32, tag="tp")
                            nc.tensor.transpose(
                                tp[:], vt[:, c * 128:(c + 1) * 128],
                                ident[0:HD, 0:HD])
                            itok = hf * 4 + c
                            nc.scalar.copy(vext[i][:, itok, 0:HD], tp[:])
                        step_thunk()
                flush_chains()
                while thunkq:
                    thunkq.pop(0)[1]()

            # -------- Phase 2+3: attention with o_proj interleaved ----------
            # q-chunks outer, heads inner: once all 8 heads finish chunk qc,
            # tokens [512qc, 512qc+512) are fully attended and their o_proj
            # row-tiles are emitted as PE gap-fillers while the Act engine
            # works through the next chunk group's exps.
            with ExitStack() as s2:
                pt_pool = s2.enter_context(tc.tile_pool(name="pt", bufs=5))
                tmp2_pool = s2.enter_context(tc.tile_pool(name="tmp2", bufs=3))
                wo_pool = s2.enter_context(tc.tile_pool(name="wo3", bufs=1))
                ob_pool = s2.enter_context(tc.tile_pool(name="ob", bufs=4))
                sc_pool = s2.enter_context(
                    tc.tile_pool(name="sc", bufs=3, space="PSUM"))
                po_pool = s2.enter_context(
                    tc.tile_pool(name="po", bufs=2, space="PSUM"))
                bc2_pool = s2.enter_context(
                    tc.tile_pool(name="bc2", bufs=1, space="PSUM"))
                ps3_pool = s2.enter_context(
                    tc.tile_pool(name="ps3", bufs=2, space="PSUM"))

                wo_t = [wo_pool.tile([128, NG3, 512], bf16, tag=f"wo3{j}",
                                     name=f"wo3{j}") for j in range(NJ)]

                # flattened (qc, h, kt2) item list; sc/exp/select emitted
                # D_PIPE items ahead of the corresponding P@V matmul.
                items = []
                for qc in range(NQC):
                    for h in range(QH):
                        for kt2 in range(NKP * qc + NKP):
                            items.append((h, qc, kt2))

                po_t = {}
                pts = {}
                norm_q = []
                ph3_q = []
                ob_t = {}

                def emit_sc(idx):
                    h, qc, kt2 = items[idx]
                    g2 = h // (QH // KVH)
                    jsl = slice(qc * QCS, (qc + 1) * QCS)
                    sc = sc_pool.tile([128, QCS], f32, tag="sc")
                    nc.tensor.matmul(
                        sc[:], ktl[g2][:, kt2 * 128:(kt2 + 1) * 128],
                        qt[h][:, jsl], start=True, stop=True)
                    pt = pt_pool.tile([128, QCS], bf16, tag="pt")
                    nc.scalar.activation(pt[:], sc[:], AF.Exp, scale=SCALE)
                    if kt2 >= NKP * qc:
                        nc.gpsimd.affine_select(
                            pt[:], pt[:], pattern=[[1, QCS]],
                            compare_op=mybir.AluOpType.is_ge,
                            fill=0.0,
                            base=qc * QCS - kt2 * 128,
                            channel_multiplier=-1)
                    pts[idx] = pt

                def emit_pv(idx):
                    h, qc, kt2 = items[idx]
                    g2 = h // (QH // KVH)
                    if kt2 == 0:
                        po_t[(h, qc)] = po_pool.tile([HD + 1, QCS], f32,
                                                     tag="po", name="po")
                    po = po_t[(h, qc)]
                    nc.tensor.matmul(
                        po[:], vext[g2][:, kt2, :], pts.pop(idx)[:],
                        start=(kt2 == 0),
                        stop=(kt2 == NKP * qc + NKP - 1))
                    if kt2 == NKP * qc + NKP - 1:
                        rinv2 = tmp2_pool.tile([1, QCS], f32r, tag="rinv2")
                        nc.vector.reciprocal(rinv2[:], po[HD:HD + 1, :])
                        norm_q.append((h, qc, rinv2))

                def emit_norm():
                    if not norm_q:
                        return
                    h, qc, rinv2 = norm_q.pop(0)
                    jsl = slice(qc * QCS, (qc + 1) * QCS)
                    po = po_t.pop((h, qc))
                    bc2 = bc2_pool.tile([HD, QCS], f32, tag="bc2")
                    nc.tensor.matmul(bc2[:], ones96[:], rinv2[:],
                                     start=True, stop=True)
                    bc2s = tmp2_pool.tile([HD, QCS], f32, tag="bc2s")
                    nc.scalar.copy(bc2s[:], bc2[:])
                    at_tmp = tmp2_pool.tile([HD, QCS], bf16, tag="at_tmp")
                    nc.vector.tensor_mul(at_tmp[:], po[0:HD, :], bc2s[:])
                    # repack rows 96h..96h+96 into the 6x128 contraction tiles
                    r0 = h * HD
                    while r0 < (h + 1) * HD:
                        t, p = divmod(r0, 128)
                        rows = min(128 - p, (h + 1) * HD - r0)
                        nc.sync.dma_start(
                            atp[t][p:p + rows, jsl],
                            at_tmp[r0 - h * HD:r0 - h * HD + rows, :])
                        r0 += rows
                    if h == QH - 1:
                        for j3 in range(NJ):
                            for i in range(NKP * qc, NKP * qc + NKP):
                                ph3_q.append((i, j3))

                def emit_ph3():
                    if not ph3_q:
                        return
                    i, j3 = ph3_q.pop(0)
                    isl = slice(i * 128, (i + 1) * 128)
                    ps3 = ps3_pool.tile([128, 512], f32, tag="ps3")
                    for t in range(NG3):
                        nc.tensor.matmul(
                            ps3[:], atp[t][:, isl], wo_t[j3][:, t, :],
                            start=(t == 0), stop=(t == NG3 - 1))
                    jsl = slice(j3 * 512, (j3 + 1) * 512)
                    ob = ob_pool.tile([128, 512], f32, tag="ob", name="ob")
                    nc.vector.tensor_copy(ob[:], ps3[:])
                    nc.sync.dma_start(out_d[isl, jsl], ob[:])

                for idx in range(len(items)):
                    if idx % 8 == 0 and idx // 8 < NJ:
                        j = idx // 8
                        nc.scalar.dma_start(wo_t[j][:], wot_d[:, j])
                    emit_sc(idx)
                    if idx >= D_PIPE2:
                        emit_pv(idx - D_PIPE2)
                        emit_norm()
                    if idx % 3 == 2 and len(ph3_q) > 2:
                        emit_ph3()
                for idx in range(len(items) - D_PIPE2, len(items)):
                    emit_pv(idx)
                    emit_norm()
                while norm_q:
                    emit_norm()
                while ph3_q:
                    emit_ph3()

    nc.compile()
    return nc


def get_nc():
    if "nc" not in _BUILD_CACHE:
        _BUILD_CACHE["nc"] = _build_nc()
    return _BUILD_CACHE["nc"]


def _permpad_rows(w96):
    """(96, N) head rows -> (128, N): evens at 0:48, odds at 64:112, pad 0."""
    out = np.zeros((128, w96.shape[1]), np.float32)
    out[0:48] = w96[0::2]
    out[64:112] = w96[1::2]
    return out


def _lhsT_tiles(wT, m):
    """(D, m) -> (128, KT, m) lhsT tile layout (contraction on partitions)."""
    return np.ascontiguousarray(
        wT.reshape(KT, 128, m).transpose(1, 0, 2)).astype(np.float32)


def prepare_in_maps(x, wq, wk, wv, wo, q_norm_w, k_norm_w, cos, sin):
    import ml_dtypes
    bf16 = ml_dtypes.bfloat16

    x = np.asarray(x, np.float32)
    wq = np.asarray(wq, np.float32)
    wk = np.asarray(wk, np.float32)
    wv = np.asarray(wv, np.float32)
    wo = np.asarray(wo, np.float32)
    cos = np.asarray(cos, np.float32)
    sin = np.asarray(sin, np.float32)
    q_norm_w = np.asarray(q_norm_w, np.float32)
    k_norm_w = np.asarray(k_norm_w, np.float32)

    def _fold_tabs(nw):
        # P multiplies the in-place operand sl, Q the partner-swapped qsn:
        #   evens rows: out = a*we*ce - b*wo*se -> P=we*ce, Q=-wo*se
        #   odds rows:  out = b*wo*co + a*we*so -> P=wo*co, Q=+we*so
        nwe = nw[0::2][:, None]
        nwo = nw[1::2][:, None]
        ta = np.zeros((128, T), np.float32)
        tb = np.zeros((128, T), np.float32)
        ta[0:48] = nwe * cos[:, 0::2].T
        ta[64:112] = nwo * cos[:, 1::2].T
        tb[0:48] = -nwo * sin[:, 0::2].T
        tb[64:112] = nwe * sin[:, 1::2].T
        return ta, tb

    tabak, tabbk = _fold_tabs(k_norm_w)

    # packed-layout q tables/selectors: set-local row R (of 384) -> head
    # h4 = R//96, local l = R%96; l<48 = even component e=l, else odd o=l-48
    tabaq = np.zeros((3, 128, T), np.float32)
    tabbq = np.zeros((3, 128, T), np.float32)
    sels = np.zeros((3, 128, 4), np.float32)
    for p in range(3):
        for r in range(128):
            R = 128 * p + r
            h4, l = divmod(R, 96)
            sels[p, r, h4] = 1.0
            if l < 48:
                e = l
                tabaq[p, r] = q_norm_w[2 * e] * cos[:, 2 * e]
                tabbq[p, r] = -q_norm_w[2 * e + 1] * sin[:, 2 * e]
            else:
                o = l - 48
                tabaq[p, r] = q_norm_w[2 * o + 1] * cos[:, 2 * o + 1]
                tabbq[p, r] = q_norm_w[2 * o] * sin[:, 2 * o + 1]
    selst = np.ascontiguousarray(sels.transpose(0, 2, 1))

    xts = []
    for b in range(B):
        xT = np.ascontiguousarray(x[b].T)  # (D, T)
        xts.append(np.ascontiguousarray(
            xT.reshape(KT, 128, T).transpose(1, 0, 2)).astype(bf16))

    in_maps = []
    for c in range(NCORES):
        b, g = divmod(c, G)
        # packed q weights: 8 heads x 96 rows (evens then odds per head)
        # -> 6 full 128-row groups
        wqp = np.zeros((QH * HD, D), np.float32)
        for i in range(QH):
            hw_ = wq[(g * QH + i) * HD:(g * QH + i + 1) * HD]
            wqp[96 * i:96 * i + 48] = hw_[0::2]
            wqp[96 * i + 48:96 * i + 96] = hw_[1::2]
        wqt = np.stack([
            _lhsT_tiles(np.ascontiguousarray(
                wqp[128 * j:128 * (j + 1)].T), 128)
            for j in range(6)]).astype(bf16)
        wkt = np.stack([
            _lhsT_tiles(_permpad_rows(
                wk[(g * KVH + i) * HD:(g * KVH + i + 1) * HD]).T, 128)
            for i in range(KVH)]).astype(bf16)
        wvt = np.stack([
            _lhsT_tiles(np.ascontiguousarray(
                wv[(g * KVH + i) * HD:(g * KVH + i + 1) * HD].T), HD)
            for i in range(KVH)]).astype(bf16)
        # packed o_proj weights, partition-major: (128, NJ, NG3, 512)
        woT = np.ascontiguousarray(
            wo[:, g * QH * HD:(g + 1) * QH * HD].T)  # (768, 3072)
        wot = np.ascontiguousarray(
            woT.reshape(NG3, 128, NJ, 512).transpose(1, 2, 0, 3)).astype(bf16)
        in_maps.append({
            "xt": xts[b], "wqt": wqt, "wkt": wkt, "wvt": wvt, "wot": wot,
            "tabaq": tabaq.astype(bf16), "tabbq": tabbq.astype(bf16),
            "tabak": tabak.astype(bf16), "tabbk": tabbk.astype(bf16),
            "sels": sels, "selst": selst,

            "o128": np.ones((128, 1), np.float32),
            "o128T": np.ones((1, 128), np.float32),
            "o96": np.ones((1, HD), np.float32),
            "ocol": np.ones((128, KTOK), bf16),
        })
    return in_maps


def kernel(**inputs):
    from concourse import bass_utils

    nc = get_nc()
    in_maps = prepare_in_maps(
        inputs["x"], inputs["wq"], inputs["wk"], inputs["wv"], inputs["wo"],
        inputs["q_norm_w"], inputs["k_norm_w"], inputs["cos"], inputs["sin"])
    trace = bool(int(os.environ.get("BASS_KERNEL_TRACE", "0")))
    res = bass_utils.run_bass_kernel_spmd(
        nc, in_maps, core_ids=list(range(NCORES)), trace=trace)
    _BUILD_CACHE["last_result"] = res
    partials = [np.asarray(r["out"]) for r in res.results]
    out = np.empty((B, T, D), np.float32)
    for b in range(B):
        out[b] = np.sum(np.stack(partials[b * G:(b + 1) * G]), axis=0,
                        dtype=np.float64).astype(np.float32)
    return out


# revision 66
# speedup vs baseline: 1.0413x; 1.0130x over previous
"""Grouped-Query Attention block (RMSNorm + RoPE + causal GQA + o_proj) on 8 trn2 NeuronCores.

Sharding: data-parallel over batch (2) x tensor-parallel over kv-head groups (4).
Core c = b*4 + g handles batch b, kv heads {2g, 2g+1}, q heads {8g..8g+7}.
Each core computes a partial o_proj output (T, D) over its 768 head-dims;
host sums the 4 group partials per batch.

v3 structure:
  * bf16 inputs/weights/activations; accumulation + norm/softmax stats fp32.
  * All matmuls 512-wide moving dim: PE sequencer dispatch (~170ns/matmul)
    stays under engine execution (~213ns) -> engine-bound, not dispatch-bound.
  * DMAs batched (x: 4 single-DMA token-quarters, o_proj weights: 1 DMA,
    output: 1 DMA per 128-token row) and spread across the SP and Pool
    dispatch queues so no sequencer saturates.
  * x + all projection weights SBUF-resident; first two x quarters land on
    different queues in parallel -> compute starts ~5us in.
  * RMSNorm / softmax broadcast matmuls issue-deferred behind the next unit's
    accumulation matmuls so PE never stalls on the Act/DVE scalar chains.
  * o_proj contraction packed: 8 heads x 96 rows repacked (SBUF DMA) into
    6 full 128-row groups -> 6 instead of 8 matmuls per output tile; o_proj
    row-tiles for tokens 0:512 interleave into the attention stream as PE
    gap fillers while Act churns exps.
"""

import os
import sys

import numpy as np

sys.path.insert(0, "/opt/trn_rl_repo")

B, T, D = 2, 1024, 3072
NH, NKV, HD = 32, 8, 96
G = 4                 # tensor-parallel groups
QH = NH // G          # q heads per core (8)
KVH = NKV // G        # kv heads per core (2)
NCORES = 8
EPS = 1e-6
SCALE = 1.0 / float(np.sqrt(HD))
KT = D // 128         # 24 contraction tiles over d_model
KTOK = T // 128       # 8 token tiles
QCS = 256             # q chunk in phase 2
NQC = T // QCS        # 4
NKP = QCS // 128      # k tiles per q chunk
D_PIPE2 = 3           # phase-2 score matmuls issued ahead of P@V
NJ = D // 512         # 6 output column chunks
NG3 = QH * HD // 128  # 6 packed o_proj contraction groups
D_PIPE = 2            # phase-2 score matmuls issued ahead of P@V

_BUILD_CACHE = {}


def _build_nc():
    from contextlib import ExitStack
    from concourse import bacc, tile, mybir
    from concourse.masks import make_identity

    f32 = mybir.dt.float32
    f32r = mybir.dt.float32r
    bf16 = mybir.dt.bfloat16
    AF = mybir.ActivationFunctionType

    nc = bacc.Bacc("TRN2", target_bir_lowering=False, debug=False,
                   num_devices=NCORES)

    xt_d = nc.dram_tensor("xt", (128, KT, T), bf16, kind="ExternalInput").ap()
    wqt_d = nc.dram_tensor("wqt", (6, 128, KT, 128), bf16, kind="ExternalInput").ap()
    wkt_d = nc.dram_tensor("wkt", (KVH, 128, KT, 128), bf16, kind="ExternalInput").ap()
    wvt_d = nc.dram_tensor("wvt", (KVH, 128, KT, HD), bf16, kind="ExternalInput").ap()
    wot_d = nc.dram_tensor("wot", (NJ * 2, 128, 3, 512), bf16, kind="ExternalInput").ap()
    tabaq_d = nc.dram_tensor("tabaq", (3, 128, T), bf16, kind="ExternalInput").ap()
    tabbq_d = nc.dram_tensor("tabbq", (3, 128, T), bf16, kind="ExternalInput").ap()
    sels_d = nc.dram_tensor("sels", (3, 128, 4), f32r, kind="ExternalInput").ap()
    selst_d = nc.dram_tensor("selst", (3, 4, 128), f32r, kind="ExternalInput").ap()
    tabak_d = nc.dram_tensor("tabak", (128, T), bf16, kind="ExternalInput").ap()
    tabbk_d = nc.dram_tensor("tabbk", (128, T), bf16, kind="ExternalInput").ap()
    o128_d = nc.dram_tensor("o128", (128, 1), f32r, kind="ExternalInput").ap()
    o96_d = nc.dram_tensor("o96", (1, HD), f32r, kind="ExternalInput").ap()
    o128T_d = nc.dram_tensor("o128T", (1, 128), f32r, kind="ExternalInput").ap()
    ocol_d = nc.dram_tensor("ocol", (128, KTOK), bf16, kind="ExternalInput").ap()
    out_d = nc.dram_tensor("out", (T, D), f32, kind="ExternalOutput").ap()

    with tile.TileContext(nc) as tc:
        with nc.allow_low_precision(reason="bf16 compute, fp32 accumulate"), \
             ExitStack() as ctx:
            const = ctx.enter_context(tc.tile_pool(name="const", bufs=1))
            p_qkv = ctx.enter_context(tc.tile_pool(name="p_qkv", bufs=1))

            eps_t = const.tile([1, 1], f32, tag="eps")
            nc.vector.memset(eps_t[:], EPS)
            warm_t = const.tile([1, 1], f32, tag="warm")
            nc.scalar.activation(warm_t[:], eps_t[:], AF.Exp, scale=1.0)
            eps4_t = const.tile([4, 1], f32, tag="eps4")
            nc.vector.memset(eps4_t[:], EPS)
            ones128 = const.tile([128, 1], f32r, tag="ones128")
            ones96 = const.tile([1, HD], f32r, tag="ones96")
            ones128T = const.tile([1, 128], f32r, tag="ones128T")
            ident = const.tile([128, 128], f32, tag="ident")

            qt = [p_qkv.tile([128, T], bf16, tag=f"qt{h}", name=f"qt{h}")
                  for h in range(QH)]
            for h in range(QH):
                nc.vector.memset(qt[h][32:64, :], 0.0)
                nc.vector.memset(qt[h][96:128, :], 0.0)
            ktl = [p_qkv.tile([128, T], bf16, tag=f"kt{g2}", name=f"kt{g2}")
                   for g2 in range(KVH)]
            vext = [p_qkv.tile([128, KTOK, HD + 1], bf16, tag=f"vx{g2}",
                               name=f"vx{g2}") for g2 in range(KVH)]
            atp = [p_qkv.tile([128, T], bf16, tag=f"atp{t}", name=f"atp{t}")
                   for t in range(NG3)]

            # ---------------- Phase 1: projections + RMSNorm + RoPE ---------
            with ExitStack() as s1:
                xt_pool = s1.enter_context(tc.tile_pool(name="xt", bufs=1))
                wkv_pool = s1.enter_context(tc.tile_pool(name="wkv", bufs=1))
                wq_pool = s1.enter_context(tc.tile_pool(name="wq", bufs=3))
                tab_pool = s1.enter_context(tc.tile_pool(name="tabs", bufs=1))
                tmp_pool = s1.enter_context(tc.tile_pool(name="tmp1", bufs=3))
                tmp4_pool = s1.enter_context(tc.tile_pool(name="tmp4", bufs=4))
                tmpb_pool = s1.enter_context(tc.tile_pool(name="tmpb", bufs=2))
                psk_pool = s1.enter_context(
                    tc.tile_pool(name="psk", bufs=1, space="PSUM"))
                psq_pool = s1.enter_context(
                    tc.tile_pool(name="psq", bufs=3, space="PSUM"))
                vtr_pool = s1.enter_context(
                    tc.tile_pool(name="vtr", bufs=1, space="PSUM"))
                ssq_pool = s1.enter_context(
                    tc.tile_pool(name="ssq", bufs=1, space="PSUM"))
                bc_pool = s1.enter_context(
                    tc.tile_pool(name="bc1", bufs=1, space="PSUM"))

                xt_t = [xt_pool.tile([128, KT, 512], bf16, tag=f"xh{hf}",
                                     name=f"xh{hf}") for hf in range(2)]
                wk_t = [wkv_pool.tile([128, KT, 128], bf16, tag=f"wk{i}",
                                      name=f"wk{i}") for i in range(KVH)]


                tabaq_t = [tab_pool.tile([128, T], bf16, tag=f"tabaq{p}",
                                          name=f"tabaq{p}") for p in range(3)]
                tabbq_t = [tab_pool.tile([128, T], bf16, tag=f"tabbq{p}",
                                          name=f"tabbq{p}") for p in range(3)]
                sels_t = [tab_pool.tile([128, 4], f32r, tag=f"sels{p}",
                                        name=f"sels{p}") for p in range(3)]
                selst_t = [tab_pool.tile([4, 128], f32r, tag=f"selst{p}",
                                         name=f"selst{p}") for p in range(3)]
                tabak_t = tab_pool.tile([128, T], bf16, tag="tabak")
                tabbk_t = tab_pool.tile([128, T], bf16, tag="tabbk")
                # static load order; wk races xh0 on the two HWDGE queues
                nc.scalar.dma_start(wk_t[0][:], wkt_d[0])
                nc.sync.dma_start(xt_t[0][:], xt_d[:, :, 0:512])
                nc.scalar.dma_start(wk_t[1][:], wkt_d[1])
                nc.sync.dma_start(xt_t[1][:], xt_d[:, :, 512:1024])
                nc.sync.dma_start(ones128[:], o128_d[:])
                nc.sync.dma_start(ones96[:], o96_d[:])
                nc.sync.dma_start(ones128T[:], o128T_d[:])
                make_identity(nc, ident[:])
                nc.scalar.dma_start(tabak_t[:], tabak_d[:])
                nc.scalar.dma_start(tabbk_t[:], tabbk_d[:])
                wq_tiles = {}

                def load_wq(j):
                    if j >= 6:
                        return
                    t = wq_pool.tile([128, KT, 128], bf16, tag="wq",
                                     name="wq")
                    nc.scalar.dma_start(t[:], wqt_d[j])
                    wq_tiles[j] = t

                for j in range(3):
                    load_wq(j)
                for p in range(3):
                    nc.scalar.dma_start(tabaq_t[p][:], tabaq_d[p])
                    nc.scalar.dma_start(tabbq_t[p][:], tabbq_d[p])
                    nc.sync.dma_start(sels_t[p][:], sels_d[p])
                    nc.sync.dma_start(selst_t[p][:], selst_d[p])
                for g2 in range(KVH):
                    nc.sync.dma_start(vext[g2][:, :, HD:HD + 1], ocol_d[:])

                def load_rest():
                    pass

                # --- norm + RoPE chain, split into two issue stages ---------
                def stage_a(ps, sq, rinv):
                    """ssq matmul (PE) + sqrt (Act) + recip (DVE)."""
                    ssq = ssq_pool.tile([4, 512], f32, tag="ssq")
                    nc.tensor.matmul(ssq[0:1, :], ones128[:], sq[:],
                                     start=True, stop=True)
                    rms = tmpb_pool.tile([1, 512], f32, tag="rms")
                    nc.scalar.activation(rms[:], ssq[0:1, :], AF.Sqrt,
                                         bias=eps_t[:], scale=1.0 / HD)
                    nc.vector.reciprocal(rinv[:], rms[:])

                def stage_b(ps, qsh, rinv, dst, hsl, ta, tb, ve):
                    """rinv broadcast (PE) + normalize + RoPE (Act/DVE).
                    ps/qsh are SBUF copies of the pre-norm projection and its
                    RoPE partner-row swap (DMA'd off the critical DVE chain).
                    The RMSNorm weights are folded into the RoPE tables
                    host-side, so one plain 1/rms broadcast serves both.
                    """
                    bc = bc_pool.tile([128, 512], f32, tag="bc")
                    nc.tensor.matmul(bc[:], ones128T[:], rinv[:],
                                     start=True, stop=True)
                    bcs = tmpb_pool.tile([128, 512], f32, tag="bcs")
                    nc.scalar.copy(bcs[:], bc[:])
                    sl = tmpb_pool.tile([128, 512], bf16, tag="slt")
                    ve.tensor_mul(sl[:], ps[:], bcs[:])
                    qsn = tmpb_pool.tile([128, 512], bf16, tag="qsn")
                    ve.tensor_mul(qsn[:], qsh[:], bcs[:])
                    # dst = sl*P + qsn*Q, P/Q full-height tables with the
                    # norm weights and the rotation signs folded in host-side
                    tm1 = tmpb_pool.tile([128, 512], bf16, tag="tm1")
                    ve.tensor_mul(tm1[:], sl[:], ta[:, hsl])
                    tm2 = tmpb_pool.tile([128, 512], bf16, tag="tm2")
                    ve.tensor_mul(tm2[:], qsn[:], tb[:, hsl])
                    ve.tensor_add(dst[:, hsl], tm1[:], tm2[:])

                # Deferred two-stage pipeline over accumulation units: the
                # PE ops of stage A/B for unit u are emitted after the accum
                # matmuls of units u+1 / u+2, so PE never waits on Act/DVE.
                chainq = []   # entries: [stage, a_thunk, b_thunk]

                def step_chain_b():
                    if chainq and chainq[0][0] == 1:
                        e = chainq.pop(0)
                        e[2]()

                def step_chain():
                    for e in chainq:
                        if e[0] == 0:
                            e[0] = 1
                            e[1]()
                            return

                def flush_chains():
                    while chainq:
                        step_chain_b()
                        step_chain()
                        if chainq and chainq[0][0] == 1:
                            e = chainq.pop(0)
                            e[2]()

                chain_no = [0]

                def make_chain(ps, dst, hsl, ta, tb):
                    # alternate DVE/Pool for RoPE; last chains stay on DVE so
                    # the Pool queue is clear for phase-2 affine_selects
                    ve = (nc.gpsimd if (chain_no[0] % 2 == 1
                                        and chain_no[0] < 14) else nc.vector)
                    chain_no[0] += 1
                    # SBUF copy frees the PSUM tile and lets the RoPE partner
                    # swap run as a DMA concurrent with the norm chain; the
                    # square runs on DVE from the copy (keeps Act to
                    # copies+sqrt only -> no activation-table churn)
                    psb = tmp_pool.tile([128, 512], f32, tag="psb")
                    nc.scalar.copy(psb[:], ps[:])
                    sq = tmp_pool.tile([128, 512], f32r, tag="sq")
                    nc.vector.tensor_mul(sq[:], psb[:], psb[:])
                    qsh = tmp_pool.tile([128, 512], f32, tag="qsh")
                    nc.sync.dma_start(qsh[0:64, :], psb[64:128, :])
                    nc.sync.dma_start(qsh[64:128, :], psb[0:64, :])
                    rinv = tmp_pool.tile([1, 512], f32r, tag="rinv")
                    push = [0, lambda: stage_a(psb, sq, rinv),
                            lambda: stage_b(psb, qsh, rinv, dst, hsl,
                                            ta, tb, ve)]
                    chainq.append(push)

                # ---- k phase (first: x-paced; half 0 in token-quarters) ----
                psk = [psk_pool.tile([128, 512], f32, tag=f"psk{i}",
                                     name=f"psk{i}") for i in range(KVH)]
                for hf in range(2):
                    hsl = slice(hf * 512, (hf + 1) * 512)
                    for i in range(KVH):
                        for kt in range(KT):
                            nc.tensor.matmul(
                                psk[i][:], wk_t[i][:, kt, :],
                                xt_t[hf][:, kt, :],
                                start=(kt == 0), stop=(kt == KT - 1))
                        if hf == 0 and i == 0:
                            load_rest()
                        step_chain_b()
                        step_chain()
                        make_chain(psk[i], ktl[i], hsl, tabak_t, tabbk_t)

                # ---- q phase: heads packed 8x96 -> 6x128 ------------------
                # Two sets of 4 heads, 3 full 128-row groups each: 288 accum
                # matmuls instead of 384.  Per-head RMSNorm sums via selector
                # matmuls; RoPE in packed layout with per-group-pattern P/Q
                # tables; results DMA-repacked into the per-head padded qt
                # tiles so phase 2 is unchanged (zero k-pads keep the unset
                # qt pad rows harmless in the QK contraction).
                thunkq = []   # (append_step, fn) run one per step, lag >= 1
                stepc = [0]

                def step_thunk():
                    stepc[0] += 1
                    if thunkq and thunkq[0][0] < stepc[0]:
                        thunkq.pop(0)[1]()

                RUNS48 = []   # (start, partner_start, head4, is_odd)
                for h4 in range(4):
                    RUNS48.append((96 * h4, 96 * h4 + 48, h4, 0))
                    RUNS48.append((96 * h4 + 48, 96 * h4, h4, 1))

                for s in range(2):
                    for hf in range(2):
                        hsl = slice(hf * 512, (hf + 1) * 512)
                        sqs, psbs, qshs = [], [], []
                        for g in range(3):
                            ps = psq_pool.tile([128, 512], f32, tag="psq",
                                               name="ps")
                            wqg = wq_tiles[3 * s + g] if hf == 0 else \
                                wq_tiles[3 * s + g]
                            for kt in range(KT):
                                nc.tensor.matmul(
                                    ps[:], wqg[:, kt, :],
                                    xt_t[hf][:, kt, :],
                                    start=(kt == 0), stop=(kt == KT - 1))
                            if hf == 1:
                                load_wq(3 * s + g + 3)
                            step_chain_b()
                            step_chain()
                            step_thunk()
                            psb = tmp4_pool.tile([128, 512], f32, tag="psb",
                                                 name="psb")
                            nc.scalar.copy(psb[:], ps[:])
                            sq = tmp_pool.tile([128, 512], f32r, tag="sq",
                                               name="sq")
                            nc.vector.tensor_mul(sq[:], psb[:], psb[:])
                            sqs.append(sq)
                            psbs.append(psb)
                        for g in range(3):
                            qshs.append(tmp4_pool.tile(
                                [128, 512], f32, tag="qsh", name="qsh"))
                        # partner-row swap, cross-group pieces
                        for (a, ap, h4, odd) in RUNS48:
                            pos = 0
                            while pos < 48:
                                dg, dp = divmod(a + pos, 128)
                                sg, sp = divmod(ap + pos, 128)
                                ln = min(48 - pos, 128 - dp, 128 - sp)
                                nc.sync.dma_start(
                                    qshs[dg][dp:dp + ln, :],
                                    psbs[sg][sp:sp + ln, :])
                                pos += ln
                        rinv4 = tmp_pool.tile([4, 512], f32r, tag="rinv4",
                                              name="rinv4")

                        def a_set(sqs=sqs, rinv4=rinv4):
                            ssq = ssq_pool.tile([4, 512], f32, tag="ssq",
                                                name="ssq")
                            for g in range(3):
                                nc.tensor.matmul(
                                    ssq[:], sels_t[g][:], sqs[g][:],
                                    start=(g == 0), stop=(g == 2))
                            rms4 = tmpb_pool.tile([4, 512], f32, tag="rms4",
                                                  name="rms4")
                            nc.scalar.activation(rms4[:], ssq[:], AF.Sqrt,
                                                 bias=eps4_t[:],
                                                 scale=1.0 / HD)
                            nc.vector.reciprocal(rinv4[:], rms4[:])
                        thunkq.append((stepc[0], a_set))

                        for g in range(3):
                            def b_g(g=g, s=s, hsl=hsl, psb=psbs[g],
                                    qsh=qshs[g], rinv4=rinv4):
                                bc = bc_pool.tile([128, 512], f32, tag="bc",
                                                  name="bc")
                                nc.tensor.matmul(bc[:], selst_t[g][:],
                                                 rinv4[:],
                                                 start=True, stop=True)
                                bcs = tmpb_pool.tile([128, 512], f32,
                                                     tag="bcs", name="bcs")
                                nc.scalar.copy(bcs[:], bc[:])
                                sl = tmpb_pool.tile([128, 512], bf16,
                                                    tag="slt", name="sl")
                                nc.vector.tensor_mul(sl[:], psb[:], bcs[:])
                                qsn = tmpb_pool.tile([128, 512], bf16,
                                                     tag="qsn", name="qsn")
                                nc.vector.tensor_mul(qsn[:], qsh[:], bcs[:])
                                tm1 = tmpb_pool.tile([128, 512], bf16,
                                                     tag="tm1", name="tm1")
                                nc.vector.tensor_mul(tm1[:], sl[:],
                                                     tabaq_t[g][:, hsl])
                                tm2 = tmpb_pool.tile([128, 512], bf16,
                                                     tag="tm2", name="tm2")
                                nc.vector.tensor_mul(tm2[:], qsn[:],
                                                     tabbq_t[g][:, hsl])
                                qp = tmpb_pool.tile([128, 512], bf16,
                                                    tag="slt", name="qp")
                                nc.vector.tensor_add(qp[:], tm1[:], tm2[:])
                                # repack into per-head padded qt layout
                                base = 128 * g
                                for (a, ap, h4, odd) in RUNS48:
                                    lo = max(a, base)
                                    hi = min(a + 48, base + 128)
                                    if lo >= hi:
                                        continue
                                    o0 = lo - a
                                    d0 = (64 + o0) if odd else o0
                                    nc.sync.dma_start(
                                        qt[4 * s + h4][d0:d0 + hi - lo, hsl],
                                        qp[lo - base:hi - base, :])
                            thunkq.append((stepc[0], b_g))

                # ---- v phase last: no norm chains -- the trailing q chains
                # drain on Act/DVE behind these accums, so the phase-2 scope
                # switch sees no backlog
                wv_t = []
                for i in range(KVH):
                    t = wq_pool.tile([128, KT, 128], bf16, tag="wq",
                                     name="wv")
                    nc.scalar.dma_start(t[:, :, 0:HD], wvt_d[i])
                    wv_t.append(t)
                for hf in range(2):
                    hsl = slice(hf * 512, (hf + 1) * 512)
                    for i in range(KVH):
                        vp = psq_pool.tile([128, 512], f32, tag="psq")
                        vps = vp[0:HD, :]
                        for kt in range(KT):
                            nc.tensor.matmul(
                                vps, wv_t[i][:, kt, 0:HD],
                                xt_t[hf][:, kt, :],
                                start=(kt == 0), stop=(kt == KT - 1))
                        vt = tab_pool.tile([HD, 512], f32, tag="vt",
                                           name="vt")
                        nc.scalar.copy(vt[:], vp[0:HD, :])
                        step_chain_b()
                        step_chain()
                        step_thunk()
                        step_thunk()
                        for c in range(4):
                            tp = vtr_pool.tile([128, HD], f32, tag="tp")
                            nc.tensor.transpose(
                                tp[:], vt[:, c * 128:(c + 1) * 128],
                                ident[0:HD, 0:HD])
                            itok = hf * 4 + c
                            nc.scalar.copy(vext[i][:, itok, 0:HD], tp[:])
                        step_thunk()
                flush_chains()
                while thunkq:
                    thunkq.pop(0)[1]()

            # -------- Phase 2+3: attention with o_proj interleaved ----------
            # q-chunks outer, heads inner: once all 8 heads finish chunk qc,
            # tokens [512qc, 512qc+512) are fully attended and their o_proj
            # row-tiles are emitted as PE gap-fillers while the Act engine
            # works through the next chunk group's exps.
            with ExitStack() as s2:
                pt_pool = s2.enter_context(tc.tile_pool(name="pt", bufs=5))
                tmp2_pool = s2.enter_context(tc.tile_pool(name="tmp2", bufs=3))
                wo_pool = s2.enter_context(tc.tile_pool(name="wo3", bufs=1))
                ob_pool = s2.enter_context(tc.tile_pool(name="ob", bufs=4))
                sc_pool = s2.enter_context(
                    tc.tile_pool(name="sc", bufs=3, space="PSUM"))
                po_pool = s2.enter_context(
                    tc.tile_pool(name="po", bufs=2, space="PSUM"))
                bc2_pool = s2.enter_context(
                    tc.tile_pool(name="bc2", bufs=1, space="PSUM"))
                ps3_pool = s2.enter_context(
                    tc.tile_pool(name="ps3", bufs=2, space="PSUM"))

                wo_t = [wo_pool.tile([128, NG3, 512], bf16, tag=f"wo3{j}",
                                     name=f"wo3{j}") for j in range(NJ)]

                # flattened (qc, h, kt2) item list; sc/exp/select emitted
                # D_PIPE items ahead of the corresponding P@V matmul.
                items = []
                for qc in range(NQC):
                    for h in range(QH):
                        for kt2 in range(NKP * qc + NKP):
                            items.append((h, qc, kt2))

                po_t = {}
                pts = {}
                norm_q = []
                ph3_q = []
                ob_t = {}

                def emit_sc(idx):
                    h, qc, kt2 = items[idx]
                    g2 = h // (QH // KVH)
                    jsl = slice(qc * QCS, (qc + 1) * QCS)
                    sc = sc_pool.tile([128, QCS], f32, tag="sc")
                    nc.tensor.matmul(
                        sc[:], ktl[g2][:, kt2 * 128:(kt2 + 1) * 128],
                        qt[h][:, jsl], start=True, stop=True)
                    pt = pt_pool.tile([128, QCS], bf16, tag="pt")
                    nc.scalar.activation(pt[:], sc[:], AF.Exp, scale=SCALE)
                    if kt2 >= NKP * qc:
                        nc.gpsimd.affine_select(
                            pt[:], pt[:], pattern=[[1, QCS]],
                            compare_op=mybir.AluOpType.is_ge,
                            fill=0.0,
                            base=qc * QCS - kt2 * 128,
                            channel_multiplier=-1)
                    pts[idx] = pt

                def emit_pv(idx):
                    h, qc, kt2 = items[idx]
                    g2 = h // (QH // KVH)
                    if kt2 == 0:
                        po_t[(h, qc)] = po_pool.tile([HD + 1, QCS], f32,
                                                     tag="po", name="po")
                    po = po_t[(h, qc)]
                    nc.tensor.matmul(
                        po[:], vext[g2][:, kt2, :], pts.pop(idx)[:],
                        start=(kt2 == 0),
                        stop=(kt2 == NKP * qc + NKP - 1))
                    if kt2 == NKP * qc + NKP - 1:
                        rinv2 = tmp2_pool.tile([1, QCS], f32r, tag="rinv2")
                        nc.vector.reciprocal(rinv2[:], po[HD:HD + 1, :])
                        norm_q.append((h, qc, rinv2))

                def emit_norm():
                    if not norm_q:
                        return
                    h, qc, rinv2 = norm_q.pop(0)
                    jsl = slice(qc * QCS, (qc + 1) * QCS)
                    po = po_t.pop((h, qc))
                    bc2 = bc2_pool.tile([HD, QCS], f32, tag="bc2")
                    nc.tensor.matmul(bc2[:], ones96[:], rinv2[:],
                                     start=True, stop=True)
                    bc2s = tmp2_pool.tile([HD, QCS], f32, tag="bc2s")
                    nc.scalar.copy(bc2s[:], bc2[:])
                    at_tmp = tmp2_pool.tile([HD, QCS], bf16, tag="at_tmp")
                    nc.vector.tensor_mul(at_tmp[:], po[0:HD, :], bc2s[:])
                    # repack rows 96h..96h+96 into the 6x128 contraction tiles
                    r0 = h * HD
                    while r0 < (h + 1) * HD:
                        t, p = divmod(r0, 128)
                        rows = min(128 - p, (h + 1) * HD - r0)
                        nc.sync.dma_start(
                            atp[t][p:p + rows, jsl],
                            at_tmp[r0 - h * HD:r0 - h * HD + rows, :])
                        r0 += rows
                    if h == QH - 1:
                        for j3 in range(NJ):
                            for i in range(NKP * qc, NKP * qc + NKP):
                                ph3_q.append((i, j3))

                def emit_ph3():
                    if not ph3_q:
                        return
                    i, j3 = ph3_q.pop(0)
                    isl = slice(i * 128, (i + 1) * 128)
                    ps3 = ps3_pool.tile([128, 512], f32, tag="ps3")
                    for t in range(NG3):
                        nc.tensor.matmul(
                            ps3[:], atp[t][:, isl], wo_t[j3][:, t, :],
                            start=(t == 0), stop=(t == NG3 - 1))
                    jsl = slice(j3 * 512, (j3 + 1) * 512)
                    ob = ob_pool.tile([128, 512], f32, tag="ob", name="ob")
                    nc.vector.tensor_copy(ob[:], ps3[:])
                    nc.sync.dma_start(out_d[isl, jsl], ob[:])

                for idx in range(len(items)):
                    if idx % 4 == 0 and idx // 4 < NJ * 2:
                        k = idx // 4
                        j, h = divmod(k, 2)
                        nc.scalar.dma_start(wo_t[j][:, 3 * h:3 * h + 3, :],
                                            wot_d[k])
                    emit_sc(idx)
                    if idx >= D_PIPE2:
                        emit_pv(idx - D_PIPE2)
                        emit_norm()
                    if idx % 3 == 2 and len(ph3_q) > 2:
                        emit_ph3()
                for idx in range(len(items) - D_PIPE2, len(items)):
                    emit_pv(idx)
                    emit_norm()
                while norm_q:
                    emit_norm()
                while ph3_q:
                    emit_ph3()

    nc.compile()
    return nc


def get_nc():
    if "nc" not in _BUILD_CACHE:
        _BUILD_CACHE["nc"] = _build_nc()
    return _BUILD_CACHE["nc"]


def _permpad_rows(w96):
    """(96, N) head rows -> (128, N): evens at 0:48, odds at 64:112, pad 0."""
    out = np.zeros((128, w96.shape[1]), np.float32)
    out[0:48] = w96[0::2]
    out[64:112] = w96[1::2]
    return out


def _lhsT_tiles(wT, m):
    """(D, m) -> (128, KT, m) lhsT tile layout (contraction on partitions)."""
    return np.ascontiguousarray(
        wT.reshape(KT, 128, m).transpose(1, 0, 2)).astype(np.float32)


def prepare_in_maps(x, wq, wk, wv, wo, q_norm_w, k_norm_w, cos, sin):
    import ml_dtypes
    bf16 = ml_dtypes.bfloat16

    x = np.asarray(x, np.float32)
    wq = np.asarray(wq, np.float32)
    wk = np.asarray(wk, np.float32)
    wv = np.asarray(wv, np.float32)
    wo = np.asarray(wo, np.float32)
    cos = np.asarray(cos, np.float32)
    sin = np.asarray(sin, np.float32)
    q_norm_w = np.asarray(q_norm_w, np.float32)
    k_norm_w = np.asarray(k_norm_w, np.float32)

    def _fold_tabs(nw):
        # P multiplies the in-place operand sl, Q the partner-swapped qsn:
        #   evens rows: out = a*we*ce - b*wo*se -> P=we*ce, Q=-wo*se
        #   odds rows:  out = b*wo*co + a*we*so -> P=wo*co, Q=+we*so
        nwe = nw[0::2][:, None]
        nwo = nw[1::2][:, None]
        ta = np.zeros((128, T), np.float32)
        tb = np.zeros((128, T), np.float32)
        ta[0:48] = nwe * cos[:, 0::2].T
        ta[64:112] = nwo * cos[:, 1::2].T
        tb[0:48] = -nwo * sin[:, 0::2].T
        tb[64:112] = nwe * sin[:, 1::2].T
        return ta, tb

    tabak, tabbk = _fold_tabs(k_norm_w)

    # packed-layout q tables/selectors: set-local row R (of 384) -> head
    # h4 = R//96, local l = R%96; l<48 = even component e=l, else odd o=l-48
    tabaq = np.zeros((3, 128, T), np.float32)
    tabbq = np.zeros((3, 128, T), np.float32)
    sels = np.zeros((3, 128, 4), np.float32)
    for p in range(3):
        for r in range(128):
            R = 128 * p + r
            h4, l = divmod(R, 96)
            sels[p, r, h4] = 1.0
            if l < 48:
                e = l
                tabaq[p, r] = q_norm_w[2 * e] * cos[:, 2 * e]
                tabbq[p, r] = -q_norm_w[2 * e + 1] * sin[:, 2 * e]
            else:
                o = l - 48
                tabaq[p, r] = q_norm_w[2 * o + 1] * cos[:, 2 * o + 1]
                tabbq[p, r] = q_norm_w[2 * o] * sin[:, 2 * o + 1]
    selst = np.ascontiguousarray(sels.transpose(0, 2, 1))

    xts = []
    for b in range(B):
        xT = np.ascontiguousarray(x[b].T)  # (D, T)
        xts.append(np.ascontiguousarray(
            xT.reshape(KT, 128, T).transpose(1, 0, 2)).astype(bf16))

    in_maps = []
    for c in range(NCORES):
        b, g = divmod(c, G)
        # packed q weights: 8 heads x 96 rows (evens then odds per head)
        # -> 6 full 128-row groups
        wqp = np.zeros((QH * HD, D), np.float32)
        for i in range(QH):
            hw_ = wq[(g * QH + i) * HD:(g * QH + i + 1) * HD]
            wqp[96 * i:96 * i + 48] = hw_[0::2]
            wqp[96 * i + 48:96 * i + 96] = hw_[1::2]
        wqt = np.stack([
            _lhsT_tiles(np.ascontiguousarray(
                wqp[128 * j:128 * (j + 1)].T), 128)
            for j in range(6)]).astype(bf16)
        wkt = np.stack([
            _lhsT_tiles(_permpad_rows(
                wk[(g * KVH + i) * HD:(g * KVH + i + 1) * HD]).T, 128)
            for i in range(KVH)]).astype(bf16)
        wvt = np.stack([
            _lhsT_tiles(np.ascontiguousarray(
                wv[(g * KVH + i) * HD:(g * KVH + i + 1) * HD].T), HD)
            for i in range(KVH)]).astype(bf16)
        # packed o_proj weights as 12 half-tile pieces (NJ*2, 128, 3, 512):
        # piece 2j+h = output column chunk j, contraction groups 3h..3h+2
        woT = np.ascontiguousarray(
            wo[:, g * QH * HD:(g + 1) * QH * HD].T)  # (768, 3072)
        wot = np.zeros((NJ * 2, 128, 3, 512), np.float32)
        for j in range(NJ):
            for h in range(2):
                for tp in range(3):
                    r0 = 128 * (3 * h + tp)
                    wot[2 * j + h][:, tp, :] = \
                        woT[r0:r0 + 128, 512 * j:512 * (j + 1)]
        wot = wot.astype(bf16)
        in_maps.append({
            "xt": xts[b], "wqt": wqt, "wkt": wkt, "wvt": wvt, "wot": wot,
            "tabaq": tabaq.astype(bf16), "tabbq": tabbq.astype(bf16),
            "tabak": tabak.astype(bf16), "tabbk": tabbk.astype(bf16),
            "sels": sels, "selst": selst,

            "o128": np.ones((128, 1), np.float32),
            "o128T": np.ones((1, 128), np.float32),
            "o96": np.ones((1, HD), np.float32),
            "ocol": np.ones((128, KTOK), bf16),
        })
    return in_maps


def kernel(**inputs):
    from concourse import bass_utils

    nc = get_nc()
    in_maps = prepare_in_maps(
        inputs["x"], inputs["wq"], inputs["wk"], inputs["wv"], inputs["wo"],
        inputs["q_norm_w"], inputs["k_norm_w"], inputs["cos"], inputs["sin"])
    trace = bool(int(os.environ.get("BASS_KERNEL_TRACE", "0")))
    res = bass_utils.run_bass_kernel_spmd(
        nc, in_maps, core_ids=list(range(NCORES)), trace=trace)
    _BUILD_CACHE["last_result"] = res
    partials = [np.asarray(r["out"]) for r in res.results]
    out = np.empty((B, T, D), np.float32)
    for b in range(B):
        out[b] = np.sum(np.stack(partials[b * G:(b + 1) * G]), axis=0,
                        dtype=np.float64).astype(np.float32)
    return out


# revision 67
# speedup vs baseline: 1.0725x; 1.0300x over previous
"""Grouped-Query Attention block (RMSNorm + RoPE + causal GQA + o_proj) on 8 trn2 NeuronCores.

Sharding: data-parallel over batch (2) x tensor-parallel over kv-head groups (4).
Core c = b*4 + g handles batch b, kv heads {2g, 2g+1}, q heads {8g..8g+7}.
Each core computes a partial o_proj output (T, D) over its 768 head-dims;
host sums the 4 group partials per batch.

v3 structure:
  * bf16 inputs/weights/activations; accumulation + norm/softmax stats fp32.
  * All matmuls 512-wide moving dim: PE sequencer dispatch (~170ns/matmul)
    stays under engine execution (~213ns) -> engine-bound, not dispatch-bound.
  * DMAs batched (x: 4 single-DMA token-quarters, o_proj weights: 1 DMA,
    output: 1 DMA per 128-token row) and spread across the SP and Pool
    dispatch queues so no sequencer saturates.
  * x + all projection weights SBUF-resident; first two x quarters land on
    different queues in parallel -> compute starts ~5us in.
  * RMSNorm / softmax broadcast matmuls issue-deferred behind the next unit's
    accumulation matmuls so PE never stalls on the Act/DVE scalar chains.
  * o_proj contraction packed: 8 heads x 96 rows repacked (SBUF DMA) into
    6 full 128-row groups -> 6 instead of 8 matmuls per output tile; o_proj
    row-tiles for tokens 0:512 interleave into the attention stream as PE
    gap fillers while Act churns exps.
"""

import os
import sys

import numpy as np

sys.path.insert(0, "/opt/trn_rl_repo")

B, T, D = 2, 1024, 3072
NH, NKV, HD = 32, 8, 96
G = 4                 # tensor-parallel groups
QH = NH // G          # q heads per core (8)
KVH = NKV // G        # kv heads per core (2)
NCORES = 8
EPS = 1e-6
SCALE = 1.0 / float(np.sqrt(HD))
KT = D // 128         # 24 contraction tiles over d_model
KTOK = T // 128       # 8 token tiles
QCS = 256             # q chunk in phase 2
NQC = T // QCS        # 4
NKP = QCS // 128      # k tiles per q chunk
D_PIPE2 = 3           # phase-2 score matmuls issued ahead of P@V
NJ = D // 512         # 6 output column chunks
NG3 = QH * HD // 128  # 6 packed o_proj contraction groups
D_PIPE = 2            # phase-2 score matmuls issued ahead of P@V

_BUILD_CACHE = {}


def _build_nc():
    from contextlib import ExitStack
    from concourse import bacc, tile, mybir
    from concourse.masks import make_identity

    f32 = mybir.dt.float32
    f32r = mybir.dt.float32r
    bf16 = mybir.dt.bfloat16
    AF = mybir.ActivationFunctionType

    nc = bacc.Bacc("TRN2", target_bir_lowering=False, debug=False,
                   num_devices=NCORES)

    xt_d = nc.dram_tensor("xt", (128, KT, T), bf16, kind="ExternalInput").ap()
    wqt_d = nc.dram_tensor("wqt", (6, 128, KT, 128), bf16, kind="ExternalInput").ap()
    wkt_d = nc.dram_tensor("wkt", (KVH, 128, KT, 128), bf16, kind="ExternalInput").ap()
    wvt_d = nc.dram_tensor("wvt", (KVH, 128, KT, HD), bf16, kind="ExternalInput").ap()
    wot_d = nc.dram_tensor("wot", (NJ * 2, 128, 3, 512), bf16, kind="ExternalInput").ap()
    tabaq_d = nc.dram_tensor("tabaq", (3, 128, T), bf16, kind="ExternalInput").ap()
    tabbq_d = nc.dram_tensor("tabbq", (3, 128, T), bf16, kind="ExternalInput").ap()
    sels_d = nc.dram_tensor("sels", (3, 128, 4), f32r, kind="ExternalInput").ap()
    selst_d = nc.dram_tensor("selst", (3, 4, 128), f32r, kind="ExternalInput").ap()
    tabak_d = nc.dram_tensor("tabak", (128, T), bf16, kind="ExternalInput").ap()
    tabbk_d = nc.dram_tensor("tabbk", (128, T), bf16, kind="ExternalInput").ap()
    o128_d = nc.dram_tensor("o128", (128, 1), f32r, kind="ExternalInput").ap()
    o96_d = nc.dram_tensor("o96", (1, HD), f32r, kind="ExternalInput").ap()
    o128T_d = nc.dram_tensor("o128T", (1, 128), f32r, kind="ExternalInput").ap()
    ocol_d = nc.dram_tensor("ocol", (128, KTOK), bf16, kind="ExternalInput").ap()
    out_d = nc.dram_tensor("out", (T, D), f32, kind="ExternalOutput").ap()

    with tile.TileContext(nc) as tc:
        with nc.allow_low_precision(reason="bf16 compute, fp32 accumulate"), \
             ExitStack() as ctx:
            const = ctx.enter_context(tc.tile_pool(name="const", bufs=1))
            p_qkv = ctx.enter_context(tc.tile_pool(name="p_qkv", bufs=1))

            eps_t = const.tile([1, 1], f32, tag="eps")
            nc.vector.memset(eps_t[:], EPS)
            warm_t = const.tile([1, 1], f32, tag="warm")
            nc.scalar.activation(warm_t[:], eps_t[:], AF.Exp, scale=1.0)
            eps4_t = const.tile([4, 1], f32, tag="eps4")
            nc.vector.memset(eps4_t[:], EPS)
            ones128 = const.tile([128, 1], f32r, tag="ones128")
            ones96 = const.tile([1, HD], f32r, tag="ones96")
            ones128T = const.tile([1, 128], f32r, tag="ones128T")
            ident = const.tile([128, 128], f32, tag="ident")

            qt = [p_qkv.tile([128, T], bf16, tag=f"qt{h}", name=f"qt{h}")
                  for h in range(QH)]
            for h in range(QH):
                nc.vector.memset(qt[h][32:64, :], 0.0)
                nc.vector.memset(qt[h][96:128, :], 0.0)
            ktl = [p_qkv.tile([128, T], bf16, tag=f"kt{g2}", name=f"kt{g2}")
                   for g2 in range(KVH)]
            vext = [p_qkv.tile([128, KTOK, HD + 1], bf16, tag=f"vx{g2}",
                               name=f"vx{g2}") for g2 in range(KVH)]
            atp = [p_qkv.tile([128, T], bf16, tag=f"atp{t}", name=f"atp{t}")
                   for t in range(NG3)]

            # ---------------- Phase 1: projections + RMSNorm + RoPE ---------
            with ExitStack() as s1:
                xt_pool = s1.enter_context(tc.tile_pool(name="xt", bufs=1))
                wkv_pool = s1.enter_context(tc.tile_pool(name="wkv", bufs=1))
                wq_pool = s1.enter_context(tc.tile_pool(name="wq", bufs=3))
                tab_pool = s1.enter_context(tc.tile_pool(name="tabs", bufs=1))
                tmp_pool = s1.enter_context(tc.tile_pool(name="tmp1", bufs=3))
                tmp4_pool = s1.enter_context(tc.tile_pool(name="tmp4", bufs=4))
                tmpb_pool = s1.enter_context(tc.tile_pool(name="tmpb", bufs=2))
                psk_pool = s1.enter_context(
                    tc.tile_pool(name="psk", bufs=1, space="PSUM"))
                psq_pool = s1.enter_context(
                    tc.tile_pool(name="psq", bufs=3, space="PSUM"))
                vtr_pool = s1.enter_context(
                    tc.tile_pool(name="vtr", bufs=1, space="PSUM"))
                ssq_pool = s1.enter_context(
                    tc.tile_pool(name="ssq", bufs=1, space="PSUM"))
                bc_pool = s1.enter_context(
                    tc.tile_pool(name="bc1", bufs=1, space="PSUM"))

                xt_t = [xt_pool.tile([128, T], bf16, tag=f"xk{kt}",
                                     name=f"xk{kt}") for kt in range(KT)]
                wk_t = [wkv_pool.tile([128, KT, 128], bf16, tag=f"wk{i}",
                                      name=f"wk{i}") for i in range(KVH)]


                tabaq_t = [tab_pool.tile([128, T], bf16, tag=f"tabaq{p}",
                                          name=f"tabaq{p}") for p in range(3)]
                tabbq_t = [tab_pool.tile([128, T], bf16, tag=f"tabbq{p}",
                                          name=f"tabbq{p}") for p in range(3)]
                sels_t = [tab_pool.tile([128, 4], f32r, tag=f"sels{p}",
                                        name=f"sels{p}") for p in range(3)]
                selst_t = [tab_pool.tile([4, 128], f32r, tag=f"selst{p}",
                                         name=f"selst{p}") for p in range(3)]
                tabak_t = tab_pool.tile([128, T], bf16, tag="tabak")
                tabbk_t = tab_pool.tile([128, T], bf16, tag="tabbk")
                wq_tiles = {}

                def load_wq(j):
                    if j >= 6:
                        return
                    t = wq_pool.tile([128, KT, 128], bf16, tag="wq",
                                     name="wq")
                    nc.scalar.dma_start(t[:], wqt_d[j])
                    wq_tiles[j] = t

                # kt-streamed x: weights interleave the chunk stream so the
                # fused opening loop is fed at ~DMA rate from ~3us
                nc.scalar.dma_start(wk_t[0][:], wkt_d[0])
                nc.sync.dma_start(xt_t[0][:], xt_d[:, 0, :])
                nc.scalar.dma_start(wk_t[1][:], wkt_d[1])
                nc.sync.dma_start(xt_t[1][:], xt_d[:, 1, :])
                load_wq(0)
                nc.sync.dma_start(xt_t[2][:], xt_d[:, 2, :])
                load_wq(1)
                nc.sync.dma_start(xt_t[3][:], xt_d[:, 3, :])
                load_wq(2)
                for kt in range(4, KT):
                    nc.sync.dma_start(xt_t[kt][:], xt_d[:, kt, :])
                nc.sync.dma_start(ones128[:], o128_d[:])
                nc.sync.dma_start(ones96[:], o96_d[:])
                nc.sync.dma_start(ones128T[:], o128T_d[:])
                make_identity(nc, ident[:])
                nc.scalar.dma_start(tabak_t[:], tabak_d[:])
                nc.scalar.dma_start(tabbk_t[:], tabbk_d[:])
                for p in range(3):
                    nc.scalar.dma_start(tabaq_t[p][:], tabaq_d[p])
                    nc.scalar.dma_start(tabbq_t[p][:], tabbq_d[p])
                    nc.sync.dma_start(sels_t[p][:], sels_d[p])
                    nc.sync.dma_start(selst_t[p][:], selst_d[p])
                for g2 in range(KVH):
                    nc.sync.dma_start(vext[g2][:, :, HD:HD + 1], ocol_d[:])

                def load_rest():
                    pass

                # --- norm + RoPE chain, split into two issue stages ---------
                def stage_a(ps, sq, rinv):
                    """ssq matmul (PE) + sqrt (Act) + recip (DVE)."""
                    ssq = ssq_pool.tile([4, 512], f32, tag="ssq")
                    nc.tensor.matmul(ssq[0:1, :], ones128[:], sq[:],
                                     start=True, stop=True)
                    rms = tmpb_pool.tile([1, 512], f32, tag="rms")
                    nc.scalar.activation(rms[:], ssq[0:1, :], AF.Sqrt,
                                         bias=eps_t[:], scale=1.0 / HD)
                    nc.vector.reciprocal(rinv[:], rms[:])

                def stage_b(ps, qsh, rinv, dst, hsl, ta, tb, ve):
                    """rinv broadcast (PE) + normalize + RoPE (Act/DVE).
                    ps/qsh are SBUF copies of the pre-norm projection and its
                    RoPE partner-row swap (DMA'd off the critical DVE chain).
                    The RMSNorm weights are folded into the RoPE tables
                    host-side, so one plain 1/rms broadcast serves both.
                    """
                    bc = bc_pool.tile([128, 512], f32, tag="bc")
                    nc.tensor.matmul(bc[:], ones128T[:], rinv[:],
                                     start=True, stop=True)
                    bcs = tmpb_pool.tile([128, 512], f32, tag="bcs")
                    nc.scalar.copy(bcs[:], bc[:])
                    sl = tmpb_pool.tile([128, 512], bf16, tag="slt")
                    ve.tensor_mul(sl[:], ps[:], bcs[:])
                    qsn = tmpb_pool.tile([128, 512], bf16, tag="qsn")
                    ve.tensor_mul(qsn[:], qsh[:], bcs[:])
                    # dst = sl*P + qsn*Q, P/Q full-height tables with the
                    # norm weights and the rotation signs folded in host-side
                    tm1 = tmpb_pool.tile([128, 512], bf16, tag="tm1")
                    ve.tensor_mul(tm1[:], sl[:], ta[:, hsl])
                    tm2 = tmpb_pool.tile([128, 512], bf16, tag="tm2")
                    ve.tensor_mul(tm2[:], qsn[:], tb[:, hsl])
                    ve.tensor_add(dst[:, hsl], tm1[:], tm2[:])

                # Deferred two-stage pipeline over accumulation units: the
                # PE ops of stage A/B for unit u are emitted after the accum
                # matmuls of units u+1 / u+2, so PE never waits on Act/DVE.
                chainq = []   # entries: [stage, a_thunk, b_thunk]

                def step_chain_b():
                    if chainq and chainq[0][0] == 1:
                        e = chainq.pop(0)
                        e[2]()

                def step_chain():
                    for e in chainq:
                        if e[0] == 0:
                            e[0] = 1
                            e[1]()
                            return

                def flush_chains():
                    while chainq:
                        step_chain_b()
                        step_chain()
                        if chainq and chainq[0][0] == 1:
                            e = chainq.pop(0)
                            e[2]()

                chain_no = [0]

                def make_chain(ps, dst, hsl, ta, tb):
                    # alternate DVE/Pool for RoPE; last chains stay on DVE so
                    # the Pool queue is clear for phase-2 affine_selects
                    ve = (nc.gpsimd if (chain_no[0] % 2 == 1
                                        and chain_no[0] < 14) else nc.vector)
                    chain_no[0] += 1
                    # SBUF copy frees the PSUM tile and lets the RoPE partner
                    # swap run as a DMA concurrent with the norm chain; the
                    # square runs on DVE from the copy (keeps Act to
                    # copies+sqrt only -> no activation-table churn)
                    psb = tmp_pool.tile([128, 512], f32, tag="psb")
                    nc.scalar.copy(psb[:], ps[:])
                    sq = tmp_pool.tile([128, 512], f32r, tag="sq")
                    nc.vector.tensor_mul(sq[:], psb[:], psb[:])
                    qsh = tmp_pool.tile([128, 512], f32, tag="qsh")
                    nc.sync.dma_start(qsh[0:64, :], psb[64:128, :])
                    nc.sync.dma_start(qsh[64:128, :], psb[0:64, :])
                    rinv = tmp_pool.tile([1, 512], f32r, tag="rinv")
                    push = [0, lambda: stage_a(psb, sq, rinv),
                            lambda: stage_b(psb, qsh, rinv, dst, hsl,
                                            ta, tb, ve)]
                    chainq.append(push)

                # ---- k phase (first: x-paced; half 0 in token-quarters) ----
                psk = [psk_pool.tile([128, 512], f32, tag=f"psk{i}",
                                     name=f"psk{i}") for i in range(KVH)]
                # fused kt-streamed opening: k half0 + q set0 half0 consume
                # x chunks as they arrive; q matmuls lag QLAG chunks so
                # their weights have landed
                QLAG = 11
                qps0 = [psq_pool.tile([128, 512], f32, tag="psq",
                                      name="qp0") for g in range(3)]
                for kt in range(KT + QLAG):
                    if kt < KT:
                        for i in range(KVH):
                            nc.tensor.matmul(
                                psk[i][:], wk_t[i][:, kt, :],
                                xt_t[kt][:, 0:512],
                                start=(kt == 0), stop=(kt == KT - 1))
                    if kt >= QLAG:
                        kq = kt - QLAG
                        for g in range(3):
                            nc.tensor.matmul(
                                qps0[g][:], wq_tiles[g][:, kq, :],
                                xt_t[kq][:, 0:512],
                                start=(kq == 0), stop=(kq == KT - 1))
                load_rest()
                for i in range(KVH):
                    make_chain(psk[i], ktl[i], slice(0, 512),
                               tabak_t, tabbk_t)
                for i in range(KVH):
                    for kt in range(KT):
                        nc.tensor.matmul(
                            psk[i][:], wk_t[i][:, kt, :],
                            xt_t[kt][:, 512:1024],
                            start=(kt == 0), stop=(kt == KT - 1))
                    step_chain_b()
                    step_chain()
                    make_chain(psk[i], ktl[i], slice(512, 1024),
                               tabak_t, tabbk_t)

                # ---- q phase: heads packed 8x96 -> 6x128 ------------------
                # Two sets of 4 heads, 3 full 128-row groups each: 288 accum
                # matmuls instead of 384.  Per-head RMSNorm sums via selector
                # matmuls; RoPE in packed layout with per-group-pattern P/Q
                # tables; results DMA-repacked into the per-head padded qt
                # tiles so phase 2 is unchanged (zero k-pads keep the unset
                # qt pad rows harmless in the QK contraction).
                thunkq = []   # (append_step, fn) run one per step, lag >= 1
                stepc = [0]

                def step_thunk():
                    stepc[0] += 1
                    if thunkq and thunkq[0][0] < stepc[0]:
                        thunkq.pop(0)[1]()

                RUNS48 = []   # (start, partner_start, head4, is_odd)
                for h4 in range(4):
                    RUNS48.append((96 * h4, 96 * h4 + 48, h4, 0))
                    RUNS48.append((96 * h4 + 48, 96 * h4, h4, 1))

                for s in range(2):
                    for hf in range(2):
                        hsl = slice(hf * 512, (hf + 1) * 512)
                        sqs, psbs, qshs = [], [], []
                        for g in range(3):
                            if s == 0 and hf == 0:
                                ps = qps0[g]
                            else:
                                ps = psq_pool.tile([128, 512], f32,
                                                   tag="psq", name="ps")
                                for kt in range(KT):
                                    nc.tensor.matmul(
                                        ps[:], wq_tiles[3 * s + g][:, kt, :],
                                        xt_t[kt][:, hsl],
                                        start=(kt == 0),
                                        stop=(kt == KT - 1))
                                if hf == 1:
                                    load_wq(3 * s + g + 3)
                            step_chain_b()
                            step_chain()
                            step_thunk()
                            psb = tmp4_pool.tile([128, 512], f32, tag="psb",
                                                 name="psb")
                            nc.scalar.copy(psb[:], ps[:])
                            sq = tmp_pool.tile([128, 512], f32r, tag="sq",
                                               name="sq")
                            nc.vector.tensor_mul(sq[:], psb[:], psb[:])
                            sqs.append(sq)
                            psbs.append(psb)
                        for g in range(3):
                            qshs.append(tmp4_pool.tile(
                                [128, 512], f32, tag="qsh", name="qsh"))
                        # partner-row swap, cross-group pieces
                        for (a, ap, h4, odd) in RUNS48:
                            pos = 0
                            while pos < 48:
                                dg, dp = divmod(a + pos, 128)
                                sg, sp = divmod(ap + pos, 128)
                                ln = min(48 - pos, 128 - dp, 128 - sp)
                                nc.sync.dma_start(
                                    qshs[dg][dp:dp + ln, :],
                                    psbs[sg][sp:sp + ln, :])
                                pos += ln
                        rinv4 = tmp_pool.tile([4, 512], f32r, tag="rinv4",
                                              name="rinv4")

                        def a_set(sqs=sqs, rinv4=rinv4):
                            ssq = ssq_pool.tile([4, 512], f32, tag="ssq",
                                                name="ssq")
                            for g in range(3):
                                nc.tensor.matmul(
                                    ssq[:], sels_t[g][:], sqs[g][:],
                                    start=(g == 0), stop=(g == 2))
                            rms4 = tmpb_pool.tile([4, 512], f32, tag="rms4",
                                                  name="rms4")
                            nc.scalar.activation(rms4[:], ssq[:], AF.Sqrt,
                                                 bias=eps4_t[:],
                                                 scale=1.0 / HD)
                            nc.vector.reciprocal(rinv4[:], rms4[:])
                        thunkq.append((stepc[0], a_set))

                        for g in range(3):
                            def b_g(g=g, s=s, hsl=hsl, psb=psbs[g],
                                    qsh=qshs[g], rinv4=rinv4):
                                bc = bc_pool.tile([128, 512], f32, tag="bc",
                                                  name="bc")
                                nc.tensor.matmul(bc[:], selst_t[g][:],
                                                 rinv4[:],
                                                 start=True, stop=True)
                                bcs = tmpb_pool.tile([128, 512], f32,
                                                     tag="bcs", name="bcs")
                                nc.scalar.copy(bcs[:], bc[:])
                                sl = tmpb_pool.tile([128, 512], bf16,
                                                    tag="slt", name="sl")
                                nc.vector.tensor_mul(sl[:], psb[:], bcs[:])
                                qsn = tmpb_pool.tile([128, 512], bf16,
                                                     tag="qsn", name="qsn")
                                nc.vector.tensor_mul(qsn[:], qsh[:], bcs[:])
                                tm1 = tmpb_pool.tile([128, 512], bf16,
                                                     tag="tm1", name="tm1")
                                nc.vector.tensor_mul(tm1[:], sl[:],
                                                     tabaq_t[g][:, hsl])
                                tm2 = tmpb_pool.tile([128, 512], bf16,
                                                     tag="tm2", name="tm2")
                                nc.vector.tensor_mul(tm2[:], qsn[:],
                                                     tabbq_t[g][:, hsl])
                                qp = tmpb_pool.tile([128, 512], bf16,
                                                    tag="slt", name="qp")
                                nc.vector.tensor_add(qp[:], tm1[:], tm2[:])
                                # repack into per-head padded qt layout
                                base = 128 * g
                                for (a, ap, h4, odd) in RUNS48:
                                    lo = max(a, base)
                                    hi = min(a + 48, base + 128)
                                    if lo >= hi:
                                        continue
                                    o0 = lo - a
                                    d0 = (64 + o0) if odd else o0
                                    nc.sync.dma_start(
                                        qt[4 * s + h4][d0:d0 + hi - lo, hsl],
                                        qp[lo - base:hi - base, :])
                            thunkq.append((stepc[0], b_g))

                # ---- v phase last: no norm chains -- the trailing q chains
                # drain on Act/DVE behind these accums, so the phase-2 scope
                # switch sees no backlog
                wv_t = []
                for i in range(KVH):
                    t = wq_pool.tile([128, KT, 128], bf16, tag="wq",
                                     name="wv")
                    nc.scalar.dma_start(t[:, :, 0:HD], wvt_d[i])
                    wv_t.append(t)
                for hf in range(2):
                    hsl = slice(hf * 512, (hf + 1) * 512)
                    for i in range(KVH):
                        vp = psq_pool.tile([128, 512], f32, tag="psq")
                        vps = vp[0:HD, :]
                        for kt in range(KT):
                            nc.tensor.matmul(
                                vps, wv_t[i][:, kt, 0:HD],
                                xt_t[kt][:, hsl],
                                start=(kt == 0), stop=(kt == KT - 1))
                        vt = tab_pool.tile([HD, 512], f32, tag="vt",
                                           name="vt")
                        nc.scalar.copy(vt[:], vp[0:HD, :])
                        step_chain_b()
                        step_chain()
                        step_thunk()
                        step_thunk()
                        for c in range(4):
                            tp = vtr_pool.tile([128, HD], f32, tag="tp")
                            nc.tensor.transpose(
                                tp[:], vt[:, c * 128:(c + 1) * 128],
                                ident[0:HD, 0:HD])
                            itok = hf * 4 + c
                            nc.scalar.copy(vext[i][:, itok, 0:HD], tp[:])
                        step_thunk()
                flush_chains()
                while thunkq:
                    thunkq.pop(0)[1]()

            # -------- Phase 2+3: attention with o_proj interleaved ----------
            # q-chunks outer, heads inner: once all 8 heads finish chunk qc,
            # tokens [512qc, 512qc+512) are fully attended and their o_proj
            # row-tiles are emitted as PE gap-fillers while the Act engine
            # works through the next chunk group's exps.
            with ExitStack() as s2:
                pt_pool = s2.enter_context(tc.tile_pool(name="pt", bufs=5))
                tmp2_pool = s2.enter_context(tc.tile_pool(name="tmp2", bufs=3))
                wo_pool = s2.enter_context(tc.tile_pool(name="wo3", bufs=1))
                ob_pool = s2.enter_context(tc.tile_pool(name="ob", bufs=4))
                sc_pool = s2.enter_context(
                    tc.tile_pool(name="sc", bufs=3, space="PSUM"))
                po_pool = s2.enter_context(
                    tc.tile_pool(name="po", bufs=2, space="PSUM"))
                bc2_pool = s2.enter_context(
                    tc.tile_pool(name="bc2", bufs=1, space="PSUM"))
                ps3_pool = s2.enter_context(
                    tc.tile_pool(name="ps3", bufs=2, space="PSUM"))

                wo_t = [wo_pool.tile([128, NG3, 512], bf16, tag=f"wo3{j}",
                                     name=f"wo3{j}") for j in range(NJ)]

                # flattened (qc, h, kt2) item list; sc/exp/select emitted
                # D_PIPE items ahead of the corresponding P@V matmul.
                items = []
                for qc in range(NQC):
                    for h in range(QH):
                        for kt2 in range(NKP * qc + NKP):
                            items.append((h, qc, kt2))

                po_t = {}
                pts = {}
                norm_q = []
                ph3_q = []
                ob_t = {}

                def emit_sc(idx):
                    h, qc, kt2 = items[idx]
                    g2 = h // (QH // KVH)
                    jsl = slice(qc * QCS, (qc + 1) * QCS)
                    sc = sc_pool.tile([128, QCS], f32, tag="sc")
                    nc.tensor.matmul(
                        sc[:], ktl[g2][:, kt2 * 128:(kt2 + 1) * 128],
                        qt[h][:, jsl], start=True, stop=True)
                    pt = pt_pool.tile([128, QCS], bf16, tag="pt")
                    nc.scalar.activation(pt[:], sc[:], AF.Exp, scale=SCALE)
                    if kt2 >= NKP * qc:
                        nc.gpsimd.affine_select(
                            pt[:], pt[:], pattern=[[1, QCS]],
                            compare_op=mybir.AluOpType.is_ge,
                            fill=0.0,
                            base=qc * QCS - kt2 * 128,
                            channel_multiplier=-1)
                    pts[idx] = pt

                def emit_pv(idx):
                    h, qc, kt2 = items[idx]
                    g2 = h // (QH // KVH)
                    if kt2 == 0:
                        po_t[(h, qc)] = po_pool.tile([HD + 1, QCS], f32,
                                                     tag="po", name="po")
                    po = po_t[(h, qc)]
                    nc.tensor.matmul(
                        po[:], vext[g2][:, kt2, :], pts.pop(idx)[:],
                        start=(kt2 == 0),
                        stop=(kt2 == NKP * qc + NKP - 1))
                    if kt2 == NKP * qc + NKP - 1:
                        rinv2 = tmp2_pool.tile([1, QCS], f32r, tag="rinv2")
                        nc.vector.reciprocal(rinv2[:], po[HD:HD + 1, :])
                        norm_q.append((h, qc, rinv2))

                def emit_norm():
                    if not norm_q:
                        return
                    h, qc, rinv2 = norm_q.pop(0)
                    jsl = slice(qc * QCS, (qc + 1) * QCS)
                    po = po_t.pop((h, qc))
                    bc2 = bc2_pool.tile([HD, QCS], f32, tag="bc2")
                    nc.tensor.matmul(bc2[:], ones96[:], rinv2[:],
                                     start=True, stop=True)
                    bc2s = tmp2_pool.tile([HD, QCS], f32, tag="bc2s")
                    nc.scalar.copy(bc2s[:], bc2[:])
                    at_tmp = tmp2_pool.tile([HD, QCS], bf16, tag="at_tmp")
                    nc.vector.tensor_mul(at_tmp[:], po[0:HD, :], bc2s[:])
                    # repack rows 96h..96h+96 into the 6x128 contraction tiles
                    r0 = h * HD
                    while r0 < (h + 1) * HD:
                        t, p = divmod(r0, 128)
                        rows = min(128 - p, (h + 1) * HD - r0)
                        nc.sync.dma_start(
                            atp[t][p:p + rows, jsl],
                            at_tmp[r0 - h * HD:r0 - h * HD + rows, :])
                        r0 += rows
                    if h == QH - 1:
                        for j3 in range(NJ):
                            for i in range(NKP * qc, NKP * qc + NKP):
                                ph3_q.append((i, j3))

                def emit_ph3():
                    if not ph3_q:
                        return
                    i, j3 = ph3_q.pop(0)
                    isl = slice(i * 128, (i + 1) * 128)
                    ps3 = ps3_pool.tile([128, 512], f32, tag="ps3")
                    for t in range(NG3):
                        nc.tensor.matmul(
                            ps3[:], atp[t][:, isl], wo_t[j3][:, t, :],
                            start=(t == 0), stop=(t == NG3 - 1))
                    jsl = slice(j3 * 512, (j3 + 1) * 512)
                    ob = ob_pool.tile([128, 512], f32, tag="ob", name="ob")
                    nc.vector.tensor_copy(ob[:], ps3[:])
                    nc.sync.dma_start(out_d[isl, jsl], ob[:])

                for idx in range(len(items)):
                    if idx % 4 == 0 and idx // 4 < NJ * 2:
                        k = idx // 4
                        j, h = divmod(k, 2)
                        nc.scalar.dma_start(wo_t[j][:, 3 * h:3 * h + 3, :],
                                            wot_d[k])
                    emit_sc(idx)
                    if idx >= D_PIPE2:
                        emit_pv(idx - D_PIPE2)
                        emit_norm()
                    if idx % 3 == 2 and len(ph3_q) > 2:
                        emit_ph3()
                for idx in range(len(items) - D_PIPE2, len(items)):
                    emit_pv(idx)
                    emit_norm()
                while norm_q:
                    emit_norm()
                while ph3_q:
                    emit_ph3()

    nc.compile()
    return nc


def get_nc():
    if "nc" not in _BUILD_CACHE:
        _BUILD_CACHE["nc"] = _build_nc()
    return _BUILD_CACHE["nc"]


def _permpad_rows(w96):
    """(96, N) head rows -> (128, N): evens at 0:48, odds at 64:112, pad 0."""
    out = np.zeros((128, w96.shape[1]), np.float32)
    out[0:48] = w96[0::2]
    out[64:112] = w96[1::2]
    return out


def _lhsT_tiles(wT, m):
    """(D, m) -> (128, KT, m) lhsT tile layout (contraction on partitions)."""
    return np.ascontiguousarray(
        wT.reshape(KT, 128, m).transpose(1, 0, 2)).astype(np.float32)


def prepare_in_maps(x, wq, wk, wv, wo, q_norm_w, k_norm_w, cos, sin):
    import ml_dtypes
    bf16 = ml_dtypes.bfloat16

    x = np.asarray(x, np.float32)
    wq = np.asarray(wq, np.float32)
    wk = np.asarray(wk, np.float32)
    wv = np.asarray(wv, np.float32)
    wo = np.asarray(wo, np.float32)
    cos = np.asarray(cos, np.float32)
    sin = np.asarray(sin, np.float32)
    q_norm_w = np.asarray(q_norm_w, np.float32)
    k_norm_w = np.asarray(k_norm_w, np.float32)

    def _fold_tabs(nw):
        # P multiplies the in-place operand sl, Q the partner-swapped qsn:
        #   evens rows: out = a*we*ce - b*wo*se -> P=we*ce, Q=-wo*se
        #   odds rows:  out = b*wo*co + a*we*so -> P=wo*co, Q=+we*so
        nwe = nw[0::2][:, None]
        nwo = nw[1::2][:, None]
        ta = np.zeros((128, T), np.float32)
        tb = np.zeros((128, T), np.float32)
        ta[0:48] = nwe * cos[:, 0::2].T
        ta[64:112] = nwo * cos[:, 1::2].T
        tb[0:48] = -nwo * sin[:, 0::2].T
        tb[64:112] = nwe * sin[:, 1::2].T
        return ta, tb

    tabak, tabbk = _fold_tabs(k_norm_w)

    # packed-layout q tables/selectors: set-local row R (of 384) -> head
    # h4 = R//96, local l = R%96; l<48 = even component e=l, else odd o=l-48
    tabaq = np.zeros((3, 128, T), np.float32)
    tabbq = np.zeros((3, 128, T), np.float32)
    sels = np.zeros((3, 128, 4), np.float32)
    for p in range(3):
        for r in range(128):
            R = 128 * p + r
            h4, l = divmod(R, 96)
            sels[p, r, h4] = 1.0
            if l < 48:
                e = l
                tabaq[p, r] = q_norm_w[2 * e] * cos[:, 2 * e]
                tabbq[p, r] = -q_norm_w[2 * e + 1] * sin[:, 2 * e]
            else:
                o = l - 48
                tabaq[p, r] = q_norm_w[2 * o + 1] * cos[:, 2 * o + 1]
                tabbq[p, r] = q_norm_w[2 * o] * sin[:, 2 * o + 1]
    selst = np.ascontiguousarray(sels.transpose(0, 2, 1))

    xts = []
    for b in range(B):
        xT = np.ascontiguousarray(x[b].T)  # (D, T)
        xts.append(np.ascontiguousarray(
            xT.reshape(KT, 128, T).transpose(1, 0, 2)).astype(bf16))

    in_maps = []
    for c in range(NCORES):
        b, g = divmod(c, G)
        # packed q weights: 8 heads x 96 rows (evens then odds per head)
        # -> 6 full 128-row groups
        wqp = np.zeros((QH * HD, D), np.float32)
        for i in range(QH):
            hw_ = wq[(g * QH + i) * HD:(g * QH + i + 1) * HD]
            wqp[96 * i:96 * i + 48] = hw_[0::2]
            wqp[96 * i + 48:96 * i + 96] = hw_[1::2]
        wqt = np.stack([
            _lhsT_tiles(np.ascontiguousarray(
                wqp[128 * j:128 * (j + 1)].T), 128)
            for j in range(6)]).astype(bf16)
        wkt = np.stack([
            _lhsT_tiles(_permpad_rows(
                wk[(g * KVH + i) * HD:(g * KVH + i + 1) * HD]).T, 128)
            for i in range(KVH)]).astype(bf16)
        wvt = np.stack([
            _lhsT_tiles(np.ascontiguousarray(
                wv[(g * KVH + i) * HD:(g * KVH + i + 1) * HD].T), HD)
            for i in range(KVH)]).astype(bf16)
        # packed o_proj weights as 12 half-tile pieces (NJ*2, 128, 3, 512):
        # piece 2j+h = output column chunk j, contraction groups 3h..3h+2
        woT = np.ascontiguousarray(
            wo[:, g * QH * HD:(g + 1) * QH * HD].T)  # (768, 3072)
        wot = np.zeros((NJ * 2, 128, 3, 512), np.float32)
        for j in range(NJ):
            for h in range(2):
                for tp in range(3):
                    r0 = 128 * (3 * h + tp)
                    wot[2 * j + h][:, tp, :] = \
                        woT[r0:r0 + 128, 512 * j:512 * (j + 1)]
        wot = wot.astype(bf16)
        in_maps.append({
            "xt": xts[b], "wqt": wqt, "wkt": wkt, "wvt": wvt, "wot": wot,
            "tabaq": tabaq.astype(bf16), "tabbq": tabbq.astype(bf16),
            "tabak": tabak.astype(bf16), "tabbk": tabbk.astype(bf16),
            "sels": sels, "selst": selst,

            "o128": np.ones((128, 1), np.float32),
            "o128T": np.ones((1, 128), np.float32),
            "o96": np.ones((1, HD), np.float32),
            "ocol": np.ones((128, KTOK), bf16),
        })
    return in_maps


def kernel(**inputs):
    from concourse import bass_utils

    nc = get_nc()
    in_maps = prepare_in_maps(
        inputs["x"], inputs["wq"], inputs["wk"], inputs["wv"], inputs["wo"],
        inputs["q_norm_w"], inputs["k_norm_w"], inputs["cos"], inputs["sin"])
    trace = bool(int(os.environ.get("BASS_KERNEL_TRACE", "0")))
    res = bass_utils.run_bass_kernel_spmd(
        nc, in_maps, core_ids=list(range(NCORES)), trace=trace)
    _BUILD_CACHE["last_result"] = res
    partials = [np.asarray(r["out"]) for r in res.results]
    out = np.empty((B, T, D), np.float32)
    for b in range(B):
        out[b] = np.sum(np.stack(partials[b * G:(b + 1) * G]), axis=0,
                        dtype=np.float64).astype(np.float32)
    return out
